# revision 69
# baseline (speedup 1.0000x reference)
"""Trainium2 Bass kernel for nn_AttnGNNLayer (EdgeConv-style GNN layer).

Data-parallel over the B*M=512 group axis: 64 groups per core on 8 cores.

Per-group pipeline (K=64 points, knn=16):
  - distance proxy q = x^T x - xx/2 (one ones-row accum matmul; xx from a
    tall (96,128) Square + one f32 matmul against a packed selector)
  - top-16 neighbor indices via DVE max8 / match_replace / max_index
  - one-hot gather matrix S[j, n*16+k]: idx rows DMA'd to (2,1024) u16,
    broadcast-DMA'd to all 128 partitions, then ONE tensor_scalar is_equal
    vs a per-partition iota (4x-mode eligible; runs on the Pool engine,
    which may only touch SBUF - GPSIMD cannot access PSUM on silicon)
  - edge conv: gather matmul accumulates bd@S + bdv@R (R = kron(I,1_16)),
    folding the center term v into psum so the windowed reduce_max (DVE,
    merged (128,1024) 2-bank tiles) directly yields max_k(u[idx]+v); the
    bn+relu then applies at psum egress
  - all 1x1 convs batched over all 64*64=4096 points per core on PE

Emission is software-pipelined: per 8-pair chunk, phase_a (knn) x8, then
b1 (S + e1) / b2 (e2) interleaved with a stagger of 4 so every engine's
in-order stream has other pairs' work between dependent ops; phase_c
(calib/gate/expansion windows) is deferred past the next chunk's phase_a.
Weight constants ship in early/late blobs (f32 + bf16) so the pair loop
does not wait on late-stage conv weights.
"""

import functools
import os
import sys

for _p in ("/opt/trn_rl_repo", "/root/.axon_site/_ro/trn_rl_repo"):
    if os.path.isdir(_p) and _p not in sys.path:
        sys.path.append(_p)

import numpy as np

import concourse.bass as bass
import concourse.mybir as mybir
import concourse.tile as tile
from concourse import bacc, bass_utils

F32 = mybir.dt.float32
BF16 = mybir.dt.bfloat16
U16 = mybir.dt.uint16

B, M, K, KNN = 2, 256, 64, 16
G = 64            # groups per core
NPAIR = G // 2    # 32 pair tiles (2 groups packed in 128 partitions)
NCORES = 8
NEG = -1.0e30
EPS = 1e-5

AF = mybir.ActivationFunctionType
ALU = mybir.AluOpType

# (name, partitions, width) of every constant packed into the blob, in order
_BLOB_LAYOUT = [
    ("iota_col", 128, 1),
    ("neg_iota_col", 128, 1),
    ("ones_row", 1, 1024),
    ("sel96", 96, 32),
    ("s1", 128, 1), ("b1", 128, 1), ("s2", 128, 1), ("b2", 128, 1),
    ("ca1_s", 64, 1), ("ca1_bias", 64, 1),
    ("cb2_blk1", 128, 1), ("cb2_blk2", 64, 1),
    ("e1s", 128, 2), ("e1bias", 128, 2),
    ("e2s", 128, 4), ("e2bias", 128, 4),
]
_BLOB_OFF = {}
_off = 0
for _n, _pp, _w in _BLOB_LAYOUT:
    _BLOB_OFF[_n] = _off
    _off += _w
BLOB_W = _off

# late-stage weights (final 256ch x 64 stage): separate DMA issued after the
# input DMAs so the pair loop can start sooner
_BLOBL_LAYOUT = [
    ("rd0", 128, 256), ("rd1", 128, 256), ("rd2", 128, 256), ("rd3", 128, 256),
    ("rds", 128, 2), ("rdb", 128, 2),
    ("sc1_0", 128, 256), ("sc1_1", 128, 256), ("sc1b", 128, 2),
    ("sc2_0", 128, 256), ("sc2_1", 128, 256), ("sc2b", 128, 2),
    ("n1s", 128, 2), ("n1b", 128, 2), ("n2s", 128, 2), ("n2b", 128, 2),
]
_BLOBL_OFF = {}
_offl = 0
for _n, _pp, _w in _BLOBL_LAYOUT:
    _BLOBL_OFF[_n] = _offl
    _offl += _w
BLOBL_W = _offl

# bf16 constants: pair-loop weights (early) and conv weights (late)
_BLOB16_LAYOUT = [
    ("b16_R2", 128, 1024),
    ("b16_wu1", 64, 64), ("b16_wv1", 64, 64),
    ("b16_wu2", 64, 128), ("b16_wv2", 64, 128),
]
_BLOB16_OFF = {}
_o16 = 0
for _n, _pp, _w in _BLOB16_LAYOUT:
    _BLOB16_OFF[_n] = _o16
    _o16 += _w
BLOB16_W = _o16

_BLOB16L_LAYOUT = [
    ("b16_ca1_a", 64, 64), ("b16_ca1_b", 128, 64), ("b16_ca2", 64, 192),
    ("b16_x1a", 64, 256), ("b16_x1b", 128, 256),
    ("b16_x2a", 128, 512), ("b16_x2b", 128, 512),
]
_BLOB16L_OFF = {}
_o16l = 0
for _n, _pp, _w in _BLOB16L_LAYOUT:
    _BLOB16L_OFF[_n] = _o16l
    _o16l += _w
BLOB16L_W = _o16l


def _np_consts(iw):
    """All constant tensors (iota + host-prepped weights)."""
    f = np.float32
    c = {}
    iota = np.arange(64, dtype=f)
    c["iota_col"] = np.concatenate([iota, iota]).reshape(128, 1)
    c["neg_iota_col"] = -c["iota_col"]
    selg = np.zeros((2, 128), dtype=f)
    selg[0, :64] = 1.0
    selg[1, 64:] = 1.0
    c["selg"] = selg
    c["ones_row"] = np.ones((1, 1024), dtype=f)
    # sel96[c*32+blk, blk] = -0.5: one matmul turns xsq96 (96,128) into
    # -xx/2 for all 4096 points as a (32,128) psum tile
    sel96 = np.zeros((96, 32), dtype=f)
    for _c in range(3):
        for _b in range(32):
            sel96[_c * 32 + _b, _b] = -0.5
    c["sel96"] = sel96
    # replication matrix: R2[p, n*16+k] = (n == p % 64); v-fold accumuland
    _R = np.repeat(np.eye(64, dtype=f), KNN, axis=1)
    c["R2"] = np.vstack([_R, _R])

    e1_w = iw["e1_w"].astype(f)
    W1, W2 = e1_w[:, :64], e1_w[:, 64:]
    c["wu1"] = W1.T.copy()
    c["wv1"] = (W2 - W1).T.copy()
    e2_w = iw["e2_w"].astype(f)
    W21, W22 = e2_w[:, :64], e2_w[:, 64:]
    c["wu2"] = W21.T.copy()
    c["wv2"] = (W22 - W21).T.copy()

    def bn_sb(g, b):
        return (g / np.sqrt(1.0 + EPS)).astype(f), b.astype(f)

    def pair_col(v):
        return np.concatenate([v, v]).reshape(128, 1).astype(f)

    s1, b1 = bn_sb(iw["e1_g"], iw["e1_b"])
    c["s1"], c["b1"] = pair_col(s1), pair_col(b1)
    s2, b2 = bn_sb(iw["e2_g"], iw["e2_b"])
    c["s2"], c["b2"] = s2.reshape(128, 1), b2.reshape(128, 1)

    cal1_w = iw["cal1_w"].astype(f)
    c["ca1_a"] = cal1_w[:, :64].T.copy()
    c["ca1_b"] = cal1_w[:, 64:].T.copy()
    cs, cbv = bn_sb(iw["cal1_g"], iw["cal1_b"])
    c["ca1_s"], c["ca1_bias"] = cs.reshape(64, 1), cbv.reshape(64, 1)

    c["ca2"] = iw["cal2_w"].astype(f).T.copy()
    cb2 = iw["cal2_bias"].astype(f)
    c["cb2_blk1"] = cb2[:128].reshape(128, 1)
    c["cb2_blk2"] = cb2[128:].reshape(64, 1)

    exp1_w = iw["exp1_w"].astype(f)
    c["x1a"] = exp1_w[:, :64].T.copy()
    c["x1b"] = exp1_w[:, 64:].T.copy()
    es, eb = bn_sb(iw["exp1_g"], iw["exp1_b"])
    c["e1s"] = es.reshape(2, 128).T.copy()
    c["e1bias"] = eb.reshape(2, 128).T.copy()

    exp2_w = iw["exp2_w"].astype(f)
    c["x2a"] = exp2_w[:, :128].T.copy()
    c["x2b"] = exp2_w[:, 128:].T.copy()
    es2, eb2 = bn_sb(iw["exp2_g"], iw["exp2_b"])
    c["e2s"] = es2.reshape(4, 128).T.copy()
    c["e2bias"] = eb2.reshape(4, 128).T.copy()

    rdT = iw["red_w"].astype(f).T.reshape(4, 128, 256)
    for i in range(4):
        c[f"rd{i}"] = rdT[i].copy()
    rs, rb = bn_sb(iw["red_g"], iw["red_b"])
    c["rds"] = rs.reshape(2, 128).T.copy()
    c["rdb"] = rb.reshape(2, 128).T.copy()

    sc1T = iw["sc1_w"].astype(f).T.reshape(2, 128, 256)
    c["sc1_0"], c["sc1_1"] = sc1T[0].copy(), sc1T[1].copy()
    c["sc1b"] = iw["sc1_b"].astype(f).reshape(2, 128).T.copy()
    sc2T = iw["sc2_w"].astype(f).T.reshape(2, 128, 256)
    c["sc2_0"], c["sc2_1"] = sc2T[0].copy(), sc2T[1].copy()
    c["sc2b"] = iw["sc2_b"].astype(f).reshape(2, 128).T.copy()

    n1s, n1b = bn_sb(iw["sc_n1_g"], iw["sc_n1_b"])
    c["n1s"] = (2.0 * n1s).reshape(2, 128).T.copy()
    c["n1b"] = n1b.reshape(2, 128).T.copy()
    n2s, n2b = bn_sb(iw["sc_n2_g"], iw["sc_n2_b"])
    c["n2s"] = n2s.reshape(2, 128).T.copy()
    c["n2b"] = n2b.reshape(2, 128).T.copy()
    return c


def _pack_blob(c):
    blob = np.zeros((128, BLOB_W), dtype=np.float32)
    for name, p, w in _BLOB_LAYOUT:
        v = c[name]
        assert v.shape == (p, w), (name, v.shape, (p, w))
        blob[:p, _BLOB_OFF[name]:_BLOB_OFF[name] + w] = v
    return blob


def _pack_blob_late(c):
    blob = np.zeros((128, BLOBL_W), dtype=np.float32)
    for name, p, w in _BLOBL_LAYOUT:
        v = c[name]
        assert v.shape == (p, w), (name, v.shape, (p, w))
        blob[:p, _BLOBL_OFF[name]:_BLOBL_OFF[name] + w] = v
    return blob


def _pack_blob16(c):
    import ml_dtypes
    src16 = {"b16_R2": c["R2"],
             "b16_wu1": c["wu1"], "b16_wv1": c["wv1"],
             "b16_wu2": c["wu2"], "b16_wv2": c["wv2"],
             "b16_ca1_a": c["ca1_a"], "b16_ca1_b": c["ca1_b"],
             "b16_ca2": c["ca2"], "b16_x1a": c["x1a"], "b16_x1b": c["x1b"],
             "b16_x2a": c["x2a"], "b16_x2b": c["x2b"]}
    blob = np.zeros((128, BLOB16_W), dtype=ml_dtypes.bfloat16)
    for name, p, w in _BLOB16_LAYOUT:
        v = src16[name]
        assert v.shape == (p, w), (name, v.shape, (p, w))
        blob[:p, _BLOB16_OFF[name]:_BLOB16_OFF[name] + w] = v.astype(
            ml_dtypes.bfloat16)
    blobl = np.zeros((128, BLOB16L_W), dtype=ml_dtypes.bfloat16)
    for name, p, w in _BLOB16L_LAYOUT:
        v = src16[name]
        assert v.shape == (p, w), (name, v.shape, (p, w))
        blobl[:p, _BLOB16L_OFF[name]:_BLOB16L_OFF[name] + w] = v.astype(
            ml_dtypes.bfloat16)
    return blob, blobl


def _emit(tc, I, out_ap, ctx):
    nc = tc.nc

    cp = ctx.enter_context(tc.tile_pool(name="const", bufs=1))
    wide = ctx.enter_context(tc.tile_pool(name="wide", bufs=1))
    MERGE = int(os.environ.get("K_MERGE", "1"))
    nbig = int(os.environ.get("K_NBIG", "2")) if MERGE else 6
    pp_big = ctx.enter_context(
        tc.tile_pool(name="ps_big", bufs=nbig, space="PSUM"))
    pp_big2 = ctx.enter_context(
        tc.tile_pool(name="ps_big2", bufs=int(os.environ.get("K_NBIG2", "2")),
                     space="PSUM"))
    pp_med = ctx.enter_context(
        tc.tile_pool(name="ps_med", bufs=int(os.environ.get("K_NMED", "2")),
                     space="PSUM"))
    wk = ctx.enter_context(
        tc.tile_pool(name="work", bufs=int(os.environ.get("K_WK", "10"))))
    wk2 = ctx.enter_context(
        tc.tile_pool(name="work2", bufs=int(os.environ.get("K_WK2", "8"))))
    ring = ctx.enter_context(tc.tile_pool(name="ring", bufs=1))

    # ---- DMA order = HWDGE processing order: the knn-critical pieces
    # (x96 for xx, xt3 rows, f32 blob with sel96/iota) go first so the
    # distance/top-k chain starts ~4us earlier; bf16 weights and the input
    # x (first needed by the e1 u/v matmuls) follow; late-stage weights
    # last.
    x96 = wide.tile([96, 128], F32, tag="w96")
    nc.sync.dma_start(out=x96,
                      in_=bass.AP(tensor=I["xt3"].tensor, offset=0,
                                  ap=[[4096, 3], [128, 32], [1, 128]]))
    blob = cp.tile([128, BLOB_W], F32, tag="blob")
    nc.sync.dma_start(out=blob, in_=I["blob"])
    sb = {}
    for name, p, w in _BLOB_LAYOUT:
        sb[name] = blob[0:p, _BLOB_OFF[name]:_BLOB_OFF[name] + w]
    blob16 = cp.tile([128, BLOB16_W], BF16, tag="blob16")
    for name, p, w in _BLOB16_LAYOUT:
        sb[name] = blob16[0:p, _BLOB16_OFF[name]:_BLOB16_OFF[name] + w]
    blob16l = cp.tile([128, BLOB16L_W], BF16, tag="blob16l")
    for name, p, w in _BLOB16L_LAYOUT:
        sb[name] = blob16l[0:p, _BLOB16L_OFF[name]:_BLOB16L_OFF[name] + w]
    xsq96 = wide.tile([96, 128], F32, tag="w96b")
    nc.scalar.activation(out=xsq96, in_=x96, func=AF.Square)
    nxp32 = pp_med.tile([32, 128], F32, tag="med")
    nc.tensor.matmul(nxp32, sb["sel96"], xsq96)
    nxs = wide.tile([32, 128], F32, tag="w96c")
    nc.scalar.activation(out=nxs, in_=nxp32, func=AF.Copy)

    B4 = wide.tile([4, 4096], F32, tag="wC")
    A4 = wide.tile([4, 4096], F32, tag="wB")
    nc.sync.dma_start(out=B4[0:3, :], in_=I["xt3"])
    nc.sync.dma_start(out=B4[3:4, :], in_=nxs)
    nc.sync.dma_start(out=A4[0:3, :], in_=I["xt3"])
    nc.sync.dma_start(out=A4[3:4, :],
                      in_=bass.AP(tensor=I["blob"].tensor,
                                  offset=_BLOB_OFF["ones_row"],
                                  ap=[[0, 1], [0, 4], [1, 1024]]))

    # bf16 weights, input x, then late-stage weights
    nc.sync.dma_start(out=blob16, in_=I["blob16"])
    x = wide.tile([64, 4096], BF16, tag="wD")
    for t in range(8):
        nc.sync.dma_start(out=x[:, t * 512:(t + 1) * 512],
                          in_=I["xt16"][:, t * 512:(t + 1) * 512])
    blobl = cp.tile([128, BLOBL_W], F32, tag="blobl")
    nc.sync.dma_start(out=blobl, in_=I["blobl"])
    nc.sync.dma_start(out=blob16l, in_=I["blob16l"])
    for name, p, w in _BLOBL_LAYOUT:
        sb[name] = blobl[0:p, _BLOBL_OFF[name]:_BLOBL_OFF[name] + w]

    # all pairwise-distance tiles upfront (prologue is DMA-bound, engines
    # idle): qt_all[:, pi*64:...] = q for pair pi; frees the psum med ring
    # and the ACT stream from per-pair distance work
    QTALL = int(os.environ.get("K_QTALL", "1"))
    qt_all = None
    if QTALL:
        qt_all = wide.tile([128, NPAIR * 64], F32, tag="wQT")
        for pi in range(NPAIR):
            _cs1 = slice((2 * pi) * 64, (2 * pi + 1) * 64)
            _cs2 = slice((2 * pi + 1) * 64, (2 * pi + 2) * 64)
            _pdp = pp_med.tile([128, 64], F32, tag="med")
            nc.tensor.matmul(_pdp[0:64, :], A4[:, _cs1], B4[:, _cs1])
            nc.tensor.matmul(_pdp[64:128, :], A4[:, _cs2], B4[:, _cs2])
            nc.scalar.activation(out=qt_all[:, pi * 64:(pi + 1) * 64],
                                 in_=_pdp, func=AF.Copy)

    # gated activations accumulated across all groups (for batched convs)
    x1all = wide.tile([64, 4096], BF16, tag="wE")
    x2all = wide.tile([128, 4096], BF16, tag="wF")
    # final per-group features (512ch as 4 blocks x 64 groups)
    xfin = cp.tile([128, 4, G], F32, tag="xfin")

    BDM = int(os.environ.get("K_BDM", "1"))
    bd_ring = []
    bdv_ring = []
    bdd_ring = []
    for ri in range(int(os.environ.get("K_BD", "6"))):
        if BDM:
            bddt = cp.tile([128, 256], BF16, tag=f"bddring{ri}")
            nc.gpsimd.memset(bddt, 0.0)
            bdd_ring.append(bddt)
            bd_ring.append(bddt[:, 0:128])
            bdv_ring.append(bddt[:, 128:256])
        else:
            bdt = cp.tile([128, 128], BF16, tag=f"bdring{ri}")
            nc.gpsimd.memset(bdt, 0.0)
            bd_ring.append(bdt)
            bdvt = cp.tile([128, 128], BF16, tag=f"bdvring{ri}")
            nc.gpsimd.memset(bdvt, 0.0)
            bdv_ring.append(bdvt)

    ADDP = int(os.environ.get("K_ADDP", "0"))
    E2_MODE = os.environ.get("K_E2M", "a")  # a | b | c | bc
    PCTAIL = int(os.environ.get("K_PCTAIL", "0"))
    SPOOL = int(os.environ.get("K_SPOOL", "1"))
    RELUP = int(os.environ.get("K_RELUP", "0"))
    RELUX2 = int(os.environ.get("K_RELUX2", "0"))

    def _relu_sb(out, in_, s_col, b_col, pool=None):
        # relu(s*in + b) from sbuf: 2 pool ops, or 1 act op
        if RELUP if pool is None else pool:
            tmp = wk.tile(list(in_.shape), F32, tag="rtmp")
            nc.gpsimd.tensor_scalar(out=tmp, in0=in_, scalar1=s_col,
                                    scalar2=b_col, op0=ALU.mult, op1=ALU.add)
            nc.gpsimd.tensor_scalar(out=out, in0=tmp, scalar1=0.0,
                                    scalar2=None, op0=ALU.max)
        else:
            nc.scalar.activation(out=out, in_=in_, func=AF.Relu,
                                 bias=b_col, scale=s_col)
    F1 = int(os.environ.get("K_F1", "1"))
    F2 = int(os.environ.get("K_F2", "1"))
    QTP = int(os.environ.get("K_QTP", "0"))

    def _tadd(out, a, b):
        if ADDP:
            nc.gpsimd.tensor_tensor(out, a, b, op=ALU.add)
        else:
            nc.vector.tensor_add(out, a, b)

    # windowed-max placement: offload part of the (128, 32n, 16k) max blocks
    # from DVE (TensorReduce) to the mostly-idle Pool engine (pairwise-max
    # tree, in place on the psum tile)
    E1_POOL = int(os.environ.get("K_E1P", "0"))
    E2_POOL_ROUNDS = tuple(
        int(v) for v in os.environ.get("K_E2P", "0,0").split(","))
    PC_POOL = int(os.environ.get("K_PCP", "0"))

    def _wmax(gp, out, pool_rounds, k=KNN):
        """max over k of gp (128, 512) viewed as (p, n, k) -> out (128, 512/k).

        pool_rounds pairwise-max rounds run on the Pool engine (in place on
        the psum tile); the remaining window is reduced on DVE."""
        g3 = gp.rearrange("p (n k) -> p n k", k=k)
        w = k
        for _ in range(pool_rounds):
            h = w // 2
            nc.gpsimd.tensor_tensor(g3[:, :, 0:h], g3[:, :, 0:h],
                                    g3[:, :, h:w], op=ALU.max)
            w = h
        nc.vector.reduce_max(out=out, in_=g3[:, :, 0:w],
                             axis=mybir.AxisListType.X)

    # ---------------- per-pair loops (chunked for DMA latency hiding) ----
    CHUNK = int(os.environ.get("K_CHUNK", "8"))
    NIXB = int(os.environ.get("K_NIXB", "8"))
    ixb_ring = [None] * NIXB
    ixr_all = ring.tile([2 * CHUNK, 1024], U16, tag="ixr_all")

    def phase_a(pi):
        g1, g2 = 2 * pi, 2 * pi + 1
        cs1 = slice(g1 * 64, (g1 + 1) * 64)
        cs2 = slice(g2 * 64, (g2 + 1) * 64)

        # q = x_m.x_n - xx_n/2 for both groups -> (128, 64)
        # (row-constant -xx_m/2 term dropped: doesn't change row top-k)
        if QTALL:
            qt = qt_all[:, pi * 64:(pi + 1) * 64]
        else:
            pdp = pp_med.tile([128, 64], F32, tag="med")
            for h, cs in ((0, cs1), (1, cs2)):
                nc.tensor.matmul(pdp[h * 64:(h + 1) * 64, :],
                                 A4[:, cs], B4[:, cs])
            qt = wk.tile([128, 64], F32, tag="qt")
            if QTP:
                nc.gpsimd.tensor_copy(qt, pdp)
            else:
                nc.scalar.activation(out=qt, in_=pdp, func=AF.Copy)

        # top-16 indices per point row
        mx = wk.tile([128, 16], F32, tag="mx")
        ix = wk.tile([128, 16], U16, tag="ix")
        qt2 = wk.tile([128, 64], F32, tag="qt2")
        nc.vector.max(out=mx[:, 0:8], in_=qt)
        nc.vector.max_index(out=ix[:, 0:8], in_max=mx[:, 0:8], in_values=qt)
        nc.vector.match_replace(out=qt2, in_to_replace=mx[:, 0:8],
                                in_values=qt, imm_value=NEG)
        nc.vector.max(out=mx[:, 8:16], in_=qt2)
        nc.vector.max_index(out=ix[:, 8:16], in_max=mx[:, 8:16], in_values=qt2)

        # idx row form (2, 1024) u16, then broadcast to all 128 partitions so
        # the one-hot compare can run at 4x (2-byte sbuf in/out)
        s2 = 2 * (pi % CHUNK)
        ixrows = ixr_all[s2:s2 + 2, :]
        nc.sync.dma_start(out=ixrows, in_=ix)
        ixb = ring.tile([128, 1024], U16, tag=f"ixb{pi % NIXB}")
        ixb_ring[pi % NIXB] = ixb
        nc.sync.dma_start(
            out=ixb, in_=ixrows.unsqueeze(1).broadcast_to((2, 64, 1024)))

    S01_ring = [None] * CHUNK

    def phase_b1(pi):
        """S-matrix + e1 edge conv for pair pi."""
        g1, g2 = 2 * pi, 2 * pi + 1
        cs1 = slice(g1 * 64, (g1 + 1) * 64)
        cs2 = slice(g2 * 64, (g2 + 1) * 64)
        ixb = ixb_ring[pi % NIXB]
        S01 = wk2.tile([128, 1024], BF16, tag="S01")
        S01_ring[pi % CHUNK] = S01
        # S01 = (ix_bcast == iota_p): 4x dve op (all operands 2-byte sbuf)
        seng = nc.gpsimd if (SPOOL == 1 or (SPOOL == 2 and pi % 2)) \
            else nc.vector
        seng.tensor_scalar(out=S01, in0=ixb,
                           scalar1=sb["iota_col"], scalar2=None,
                           op0=ALU.is_equal)

        bd = bd_ring[pi % len(bd_ring)]
        if F1:
            # u and v-fold matmuls into one psum tile; merged copies
            uvv = pp_med.tile([128, 256], F32, tag="med")
            nc.tensor.matmul(uvv[0:64, 0:64], x[:, cs1], sb["b16_wu1"])
            nc.tensor.matmul(uvv[64:128, 64:128], x[:, cs2], sb["b16_wu1"])
            nc.tensor.matmul(uvv[0:64, 128:192], x[:, cs1], sb["b16_wv1"])
            nc.tensor.matmul(uvv[64:128, 192:256], x[:, cs2], sb["b16_wv1"])
            bdv = bdv_ring[pi % len(bdv_ring)]
            if BDM:
                # one copy per partition half: {u block, v block} as a
                # strided access pattern on both sides.
                # col = a*128 + b*64 + c: a selects u/v, b selects group
                bdd = bdd_ring[pi % len(bdd_ring)]
                sv = uvv.rearrange("p (a b c) -> p a b c", a=2, b=2)
                dv = bdd.rearrange("p (a b c) -> p a b c", a=2, b=2)
                nc.scalar.activation(out=dv[0:64, :, 0:1, :],
                                     in_=sv[0:64, :, 0:1, :], func=AF.Copy)
                nc.scalar.activation(out=dv[64:128, :, 1:2, :],
                                     in_=sv[64:128, :, 1:2, :], func=AF.Copy)
            else:
                nc.scalar.activation(
                    out=bd[0:64, 0:64], in_=uvv[0:64, 0:64], func=AF.Copy)
                nc.scalar.activation(
                    out=bd[64:128, 64:128], in_=uvv[64:128, 64:128],
                    func=AF.Copy)
                nc.scalar.activation(
                    out=bdv[0:64, 0:64], in_=uvv[0:64, 128:192], func=AF.Copy)
                nc.scalar.activation(
                    out=bdv[64:128, 64:128], in_=uvv[64:128, 192:256],
                    func=AF.Copy)
        else:
            uv1 = pp_med.tile([128, 128], F32, tag="med")
            nc.tensor.matmul(uv1[0:64, 0:64], x[:, cs1], sb["b16_wu1"])
            nc.tensor.matmul(uv1[64:128, 64:128], x[:, cs2], sb["b16_wu1"])
            nc.scalar.activation(out=bd[0:64, 0:64], in_=uv1[0:64, 0:64],
                                 func=AF.Copy)
            nc.scalar.activation(out=bd[64:128, 64:128],
                                 in_=uv1[64:128, 64:128], func=AF.Copy)
        m1 = wk.tile([128, 64], F32, tag="m1")
        if MERGE:
            g1p = pp_big2.tile([128, 1024], F32, tag="big2")
            for half in range(2):
                csl = slice(half * 512, (half + 1) * 512)
                gh = g1p[:, csl]
                if F1:
                    nc.tensor.matmul(gh, bd, S01[:, csl], start=True,
                                     stop=False)
                    nc.tensor.matmul(gh, bdv, sb["b16_R2"][:, csl],
                                     start=False, stop=True)
                else:
                    nc.tensor.matmul(gh, bd, S01[:, csl])
            nc.vector.reduce_max(
                out=m1, in_=g1p.rearrange("p (n k) -> p n k", k=KNN),
                axis=mybir.AxisListType.X)
        else:
            for half in range(2):
                csl = slice(half * 512, (half + 1) * 512)
                g1p = pp_big.tile([128, 512], F32, tag="big")
                if F1:
                    nc.tensor.matmul(g1p, bd, S01[:, csl], start=True,
                                     stop=False)
                    nc.tensor.matmul(g1p, bdv, sb["b16_R2"][:, csl],
                                     start=False, stop=True)
                else:
                    nc.tensor.matmul(g1p, bd, S01[:, csl])
                _wmax(g1p, m1[:, half * 32:(half + 1) * 32], E1_POOL)
        if F1:
            _relu_sb(x1all[:, cs1], m1[0:64, :], sb["s1"][0:64],
                     sb["b1"][0:64])
            _relu_sb(x1all[:, cs2], m1[64:128, :], sb["s1"][64:128],
                     sb["b1"][64:128])
        else:
            v1 = pp_med.tile([128, 64], F32, tag="med")
            nc.tensor.matmul(v1[0:64, :], sb["b16_wv1"], x[:, cs1])
            nc.tensor.matmul(v1[64:128, :], sb["b16_wv1"], x[:, cs2])
            t1a = wk.tile([64, 64], F32, tag="t1a")
            _tadd(t1a, m1[0:64, :], v1[0:64, :])
            t1b = wk.tile([128, 64], F32, tag="t1b")
            _tadd(t1b[64:128, :], m1[64:128, :], v1[64:128, :])
            nc.scalar.activation(out=x1all[:, cs1], in_=t1a, func=AF.Relu,
                                 bias=sb["b1"][0:64], scale=sb["s1"][0:64])
            nc.scalar.activation(out=x1all[:, cs2], in_=t1b[64:128, :],
                                 func=AF.Relu,
                                 bias=sb["b1"][64:128], scale=sb["s1"][64:128])

    def phase_b2(pi):
        """e2 edge conv for pair pi (consumes x1all + S01)."""
        g1, g2 = 2 * pi, 2 * pi + 1
        cs1 = slice(g1 * 64, (g1 + 1) * 64)
        cs2 = slice(g2 * 64, (g2 + 1) * 64)
        S01 = S01_ring[pi % CHUNK]
        for h, cs in ((0, cs1), (1, cs2)):
            xg = x1all[:, cs]
            psl = slice(h * 64, (h + 1) * 64)
            if E2_MODE == "a" and F2:
                uvp = pp_med.tile([64, 256], F32, tag="med")
                nc.tensor.matmul(uvp[:, 0:128], xg, sb["b16_wu2"])
                nc.tensor.matmul(uvp[:, 128:256], xg, sb["b16_wv2"])
                uvs = wk.tile([128, 256], BF16, tag="uvs")
                nc.scalar.activation(out=uvs[psl, :], in_=uvp, func=AF.Copy)
                uT2 = uvs[:, 0:128]
                vv2 = uvs[:, 128:256]
            else:
                uT2 = wk.tile([128, 128], BF16, tag="uT2")
                uT2p = pp_med.tile([64, 128], F32, tag="med")
                nc.tensor.matmul(uT2p, xg, sb["b16_wu2"])
                nc.scalar.activation(out=uT2[psl, :], in_=uT2p, func=AF.Copy)
            if E2_MODE == "a":
                m2 = wk.tile([128, 64], F32, tag="m2")
                if MERGE:
                    g2p = pp_big2.tile([128, 1024], F32, tag="big2")
                    for half in range(2):
                        csl = slice(half * 512, (half + 1) * 512)
                        gh = g2p[:, csl]
                        if F2:
                            nc.tensor.matmul(gh, uT2[psl, :], S01[psl, csl],
                                             start=True, stop=False)
                            nc.tensor.matmul(gh, vv2[psl, :],
                                             sb["b16_R2"][psl, csl],
                                             start=False, stop=True)
                        else:
                            nc.tensor.matmul(gh, uT2[psl, :], S01[psl, csl])
                    nc.vector.reduce_max(
                        out=m2, in_=g2p.rearrange("p (n k) -> p n k", k=KNN),
                        axis=mybir.AxisListType.X)
                else:
                    for half in range(2):
                        csl = slice(half * 512, (half + 1) * 512)
                        g2p = pp_big.tile([128, 512], F32, tag="big")
                        if F2:
                            nc.tensor.matmul(g2p, uT2[psl, :], S01[psl, csl],
                                             start=True, stop=False)
                            nc.tensor.matmul(g2p, vv2[psl, :],
                                             sb["b16_R2"][psl, csl],
                                             start=False, stop=True)
                        else:
                            nc.tensor.matmul(g2p, uT2[psl, :], S01[psl, csl])
                        _wmax(g2p, m2[:, half * 32:(half + 1) * 32],
                              E2_POOL_ROUNDS[half])
                if F2:
                    _relu_sb(x2all[:, cs], m2, sb["s2"], sb["b2"],
                             pool=RELUX2)
                else:
                    v2 = pp_med.tile([128, 64], F32, tag="med")
                    nc.tensor.matmul(v2, sb["b16_wv2"], xg)
                    t2 = wk.tile([128, 64], F32, tag="t2")
                    _tadd(t2, m2, v2)
                    nc.scalar.activation(out=x2all[:, cs], in_=t2,
                                         func=AF.Relu,
                                         bias=sb["b2"], scale=sb["s2"])
                continue
            # v-folded path: g2p = uT2.S + vT2.R, then relu(bn) at psum
            # egress (valid pre-max: bn scale > 0), max-tree on sbuf bf16
            vv2p = pp_med.tile([64, 128], F32, tag="med")
            nc.tensor.matmul(vv2p, xg, sb["b16_wv2"])
            vv2 = wk.tile([64, 128], BF16, tag="vv2")
            nc.scalar.activation(out=vv2, in_=vv2p, func=AF.Copy)
            for half in range(2):
                csl = slice(half * 512, (half + 1) * 512)
                g2p = pp_big.tile([128, 512], F32, tag="big")
                nc.tensor.matmul(g2p, uT2[psl, :], S01[psl, csl],
                                 start=True, stop=False)
                nc.tensor.matmul(g2p, vv2, sb["b16_R2"][0:64, csl],
                                 start=False, stop=True)
                x2pre = wk.tile([128, 512], BF16, tag="x2pre")
                nc.scalar.activation(out=x2pre, in_=g2p, func=AF.Relu,
                                     bias=sb["b2"], scale=sb["s2"])
                p3 = x2pre.rearrange("p (n k) -> p n k", k=KNN)
                eng = nc.gpsimd if E2_MODE == "c" or (
                    E2_MODE == "bc" and half == 1) else nc.vector
                eng.tensor_tensor(p3[:, :, 0:8], p3[:, :, 0:8],
                                  p3[:, :, 8:16], op=ALU.max)
                eng.tensor_tensor(p3[:, :, 0:4], p3[:, :, 0:4],
                                  p3[:, :, 4:8], op=ALU.max)
                eng.tensor_tensor(p3[:, :, 0:2], p3[:, :, 0:2],
                                  p3[:, :, 2:4], op=ALU.max)
                osub = slice(cs.start + half * 32, cs.start + half * 32 + 32)
                eng.tensor_tensor(x2all[:, osub], p3[:, :, 0:1].squeeze(-1),
                                  p3[:, :, 1:2].squeeze(-1), op=ALU.max)


    # ---------------- batched calib/gate/expansion (per 512-col window) --
    c1all = wide.tile([64, 4096], BF16, tag="wG")
    sigA = wide.tile([64, 4096], BF16, tag="wA")
    sigX2 = wide.tile([128, 4096], BF16, tag="wH")
    pcr = ctx.enter_context(
        tc.tile_pool(name="pcring", bufs=int(os.environ.get("K_NPC", "3"))))
    ee0 = wide.tile([128, 4096], BF16, tag="wK")
    ee1 = wide.tile([128, 4096], BF16, tag="wL")
    ee = [ee0, ee1]

    def phase_c(j):
        if PCSPLIT:
            phase_c_part(j, 0)
            phase_c_part(j, 1)
        else:
            phase_c_part(j, None)

    def phase_c_part(j, part):
        if part is None:
            csl = slice(j * 512, (j + 1) * 512)
            fsl = slice(j * 8, (j + 1) * 8)
        else:
            csl = slice(j * 512 + part * 256, j * 512 + part * 256 + 256)
            fsl = slice(j * 8 + part * 4, j * 8 + part * 4 + 4)
        W = csl.stop - csl.start
        c1p = pp_big.tile([64, W], F32, tag="big")
        nc.tensor.matmul(c1p, sb["b16_ca1_a"], x1all[:, csl], start=True,
                         stop=False)
        nc.tensor.matmul(c1p, sb["b16_ca1_b"], x2all[:, csl], start=False,
                         stop=True)
        nc.scalar.activation(out=c1all[:, csl], in_=c1p, func=AF.Relu,
                             bias=sb["ca1_bias"], scale=sb["ca1_s"])
        sp1 = pp_big.tile([128, W], F32, tag="big")
        nc.tensor.matmul(sp1, sb["b16_ca2"][:, 0:128], c1all[:, csl])
        nc.scalar.activation(out=sigA[:, csl], in_=sp1[0:64, :],
                             func=AF.Sigmoid, bias=sb["cb2_blk1"][0:64])
        nc.scalar.activation(out=sigX2[0:64, csl], in_=sp1[64:128, :],
                             func=AF.Sigmoid, bias=sb["cb2_blk1"][64:128])
        sp2 = pp_big.tile([64, W], F32, tag="big")
        nc.tensor.matmul(sp2, sb["b16_ca2"][:, 128:192], c1all[:, csl])
        nc.scalar.activation(out=sigX2[64:128, csl], in_=sp2, func=AF.Sigmoid,
                             bias=sb["cb2_blk2"])
        p1t = pcr.tile([64, W], BF16, tag="p1r")
        p2t = pcr.tile([128, W], BF16, tag="p2r")
        nc.gpsimd.tensor_mul(p1t, x1all[:, csl], sigA[:, csl])
        nc.gpsimd.tensor_mul(p2t, x2all[:, csl], sigX2[:, csl])
        for b in range(2):
            ep = pp_big.tile([128, W], F32, tag="big")
            osl = slice(b * 128, (b + 1) * 128)
            nc.tensor.matmul(ep, sb["b16_x1a"][:, osl], p1t,
                             start=True, stop=False)
            nc.tensor.matmul(ep, sb["b16_x1b"][:, osl], p2t,
                             start=False, stop=True)
            nc.scalar.activation(out=ee[b][:, csl], in_=ep, func=AF.Relu,
                                 bias=sb["e1bias"][:, b:b + 1],
                                 scale=sb["e1s"][:, b:b + 1])
        tailwin = PCTAIL and j >= 6
        for b in range(4):
            if tailwin and b >= 2:
                xp = pp_big2.tile([128, W], F32, tag="big2")
            else:
                xp = pp_big.tile([128, W], F32, tag="big")
            osl = slice(b * 128, (b + 1) * 128)
            nc.tensor.matmul(xp, sb["b16_x2a"][:, osl], ee[0][:, csl],
                             start=True, stop=False)
            nc.tensor.matmul(xp, sb["b16_x2b"][:, osl], ee[1][:, csl],
                             start=False, stop=True)
            xm = wk2.tile([128, W // 64], F32, tag="xm")
            _wmax(xp, xm, PC_POOL, k=64)
            nc.scalar.activation(out=xfin[:, b, fsl], in_=xm,
                                 func=AF.Relu,
                                 bias=sb["e2bias"][:, b:b + 1],
                                 scale=sb["e2s"][:, b:b + 1])

    STAG = int(os.environ.get("K_STAG", "5"))
    PCSPLIT = int(os.environ.get("K_PCSPLIT", "0"))
    FLAT = int(os.environ.get("K_FLAT", "1"))
    if FLAT:
        # one continuous pipeline over all 32 pairs: phase_a leads b1 by LA,
        # b2 trails b1 by STAG, each phase_c window fires as its 4 pairs
        # complete. No chunk boundaries, so the pipeline never drains.
        # Ring safety: ixb/ixr slots (8) are rewritten 8-LA b1-steps after
        # their reader; S01 slots (WK2) rewritten WK2-STAG steps after.
        LA = int(os.environ.get("K_LA", "3"))
        for pi in range(LA):
            phase_a(pi)

        def _after_b2(done):
            if done % 4 == 3:
                phase_c(done // 4)
        for pi in range(NPAIR):
            phase_b1(pi)
            if pi + LA < NPAIR:
                phase_a(pi + LA)
            if pi >= STAG:
                phase_b2(pi - STAG)
                _after_b2(pi - STAG)
        for pi in range(NPAIR - STAG, NPAIR):
            phase_b2(pi)
            _after_b2(pi)
    else:
        nwin = CHUNK // 4
        pending_c = []
        for chunk in range(NPAIR // CHUNK):
            base = chunk * CHUNK
            for pi in range(base, base + CHUNK):
                phase_a(pi)
            for w in pending_c:
                phase_c(w)
            pending_c = []
            for i in range(CHUNK):
                phase_b1(base + i)
                if i >= STAG:
                    phase_b2(base + i - STAG)
            for i in range(CHUNK - STAG, CHUNK):
                phase_b2(base + i)
            pending_c = list(range(nwin * chunk, nwin * chunk + nwin))
        for w in pending_c:
            phase_c(w)

    # ---------------- final stage (256ch x 64 group-cols) ---------------
    tt = wk.tile([128, 2, G], F32, tag="tt")
    FSPLIT = int(os.environ.get("K_FSPLIT", "0"))
    for b in range(2):
        osl = slice(b * 128, (b + 1) * 128)
        rp = pp_med.tile([128, G], F32, tag="med")
        rngs = (slice(0, 48), slice(48, 64)) if FSPLIT else (slice(0, G),)
        for rng in rngs:
            for cb in range(4):
                nc.tensor.matmul(rp[:, rng], sb[f"rd{cb}"][:, osl],
                                 xfin[:, cb, rng],
                                 start=(cb == 0), stop=(cb == 3))
        rr = wk.tile([128, G], F32, tag="rr")
        nc.scalar.activation(out=rr, in_=rp, func=AF.Relu,
                             bias=sb["rdb"][:, b:b + 1],
                             scale=sb["rds"][:, b:b + 1])
        nc.vector.tensor_scalar(out=tt[:, b, :], in0=rr,
                                scalar1=sb["n1s"][:, b:b + 1],
                                scalar2=sb["n1b"][:, b:b + 1],
                                op0=ALU.mult, op1=ALU.add)
    hh = wk.tile([128, 2, G], F32, tag="hh")
    for b in range(2):
        osl = slice(b * 128, (b + 1) * 128)
        hp = pp_med.tile([128, G], F32, tag="med")
        for cb in range(2):
            nc.tensor.matmul(hp, sb[f"sc1_{cb}"][:, osl], tt[:, cb, :],
                             start=(cb == 0), stop=(cb == 1))
        nc.scalar.activation(out=hh[:, b, :], in_=hp, func=AF.Relu,
                             bias=sb["sc1b"][:, b:b + 1])
    for b in range(2):
        osl = slice(b * 128, (b + 1) * 128)
        h2p = pp_med.tile([128, G], F32, tag="med")
        for cb in range(2):
            nc.tensor.matmul(h2p, sb[f"sc2_{cb}"][:, osl], hh[:, cb, :],
                             start=(cb == 0), stop=(cb == 1))
        s2sum = wk.tile([128, G], F32, tag="s2sum")
        nc.vector.tensor_scalar(out=s2sum, in0=h2p,
                                scalar1=sb["sc2b"][:, b:b + 1], scalar2=None,
                                op0=ALU.add)
        s2t = wk.tile([128, G], F32, tag="s2t")
        nc.vector.tensor_add(s2t, s2sum, tt[:, b, :])
        osb = wk.tile([128, G], F32, tag="osb")
        nc.vector.tensor_scalar(out=osb, in0=s2t,
                                scalar1=sb["n2s"][:, b:b + 1],
                                scalar2=sb["n2b"][:, b:b + 1],
                                op0=ALU.mult, op1=ALU.add)
        nc.sync.dma_start(out=out_ap[b * 128:(b + 1) * 128, :], in_=osb)


@functools.lru_cache(maxsize=1)
def _build():
    nc = bacc.Bacc("TRN2", target_bir_lowering=False, debug=False,
                   num_devices=NCORES)
    I = {}
    I["xt16"] = nc.dram_tensor("xt16", (64, 4096), BF16,
                               kind="ExternalInput").ap()
    I["xt3"] = nc.dram_tensor("xt3", (3, 4096), F32,
                              kind="ExternalInput").ap()
    I["blob"] = nc.dram_tensor("blob", (128, BLOB_W), F32,
                               kind="ExternalInput").ap()
    I["blobl"] = nc.dram_tensor("blobl", (128, BLOBL_W), F32,
                                kind="ExternalInput").ap()
    I["blob16"] = nc.dram_tensor("blob16", (128, BLOB16_W), BF16,
                                 kind="ExternalInput").ap()
    I["blob16l"] = nc.dram_tensor("blob16l", (128, BLOB16L_W), BF16,
                                  kind="ExternalInput").ap()
    out_ap = nc.dram_tensor("out", (256, G), F32, kind="ExternalOutput").ap()
    from contextlib import ExitStack
    with tile.TileContext(nc) as tc, ExitStack() as ctx:
        _emit(tc, I, out_ap, ctx)
    nc.compile()
    return nc


def kernel(**inputs):
    nc = _build()
    consts = _np_consts(inputs)
    blob = _pack_blob(consts)
    blobl = _pack_blob_late(consts)
    blob16v, blob16lv = _pack_blob16(consts)

    xyz = inputs["xyz"].astype(np.float32)      # (2, 256, 64, 3)
    feats = inputs["feats"].astype(np.float32)  # (2, 256, 64, 61)
    xf_full = np.concatenate([xyz, feats], axis=-1).reshape(512 * 64, 64)

    in_maps = []
    for c in range(NCORES):
        import ml_dtypes
        sh = xf_full[c * 4096:(c + 1) * 4096, :]
        in_maps.append({
            "blob": blob,
            "blobl": blobl,
            "blob16": blob16v,
            "blob16l": blob16lv,
            "xt16": np.ascontiguousarray(sh.T.astype(ml_dtypes.bfloat16)),
            "xt3": np.ascontiguousarray(sh.T[0:3, :]),
        })

    trace = bool(int(os.environ.get("KERNEL_TRACE", "0")))
    try:
        res = bass_utils.run_bass_kernel_spmd(
            nc, in_maps, core_ids=list(range(NCORES)), trace=trace)
    except ModuleNotFoundError:
        res = bass_utils.run_bass_kernel_spmd(
            nc, in_maps, core_ids=list(range(NCORES)))
    if trace and res.exec_time_ns is not None:
        print(f"HW exec time: {res.exec_time_ns} ns")
        if res.instructions_and_trace is not None:
            print(f"trace: {res.instructions_and_trace[1]}")
        kernel.last_results = res

    out = np.empty((2, 256, 256), dtype=np.float32)
    for c in range(NCORES):
        o = res.results[c]["out"]              # (256, 64)
        b, mlo = divmod(c * G, 256)
        out[b, :, mlo:mlo + G] = o
    return out


if __name__ == "__main__":
    print("building bass graph...")
    nc = _build()
    print("graph built ok")



# revision 72
# speedup vs baseline: 1.0119x; 1.0119x over previous
"""Trainium2 Bass kernel for nn_AttnGNNLayer (EdgeConv-style GNN layer).

Data-parallel over the B*M=512 group axis: 64 groups per core on 8 cores.

Per-group pipeline (K=64 points, knn=16):
  - distance proxy q = x^T x - xx/2 (one ones-row accum matmul; xx from a
    tall (96,128) Square + one f32 matmul against a packed selector)
  - top-16 neighbor indices via DVE max8 / match_replace / max_index
  - one-hot gather matrix S[j, n*16+k]: idx rows DMA'd to (2,1024) u16,
    broadcast-DMA'd to all 128 partitions, then ONE tensor_scalar is_equal
    vs a per-partition iota (4x-mode eligible; runs on the Pool engine,
    which may only touch SBUF - GPSIMD cannot access PSUM on silicon)
  - edge conv: gather matmul accumulates bd@S + bdv@R (R = kron(I,1_16)),
    folding the center term v into psum so the windowed reduce_max (DVE,
    merged (128,1024) 2-bank tiles) directly yields max_k(u[idx]+v); the
    bn+relu then applies at psum egress
  - all 1x1 convs batched over all 64*64=4096 points per core on PE

Emission is software-pipelined: per 8-pair chunk, phase_a (knn) x8, then
b1 (S + e1) / b2 (e2) interleaved with a stagger of 4 so every engine's
in-order stream has other pairs' work between dependent ops; phase_c
(calib/gate/expansion windows) is deferred past the next chunk's phase_a.
Weight constants ship in early/late blobs (f32 + bf16) so the pair loop
does not wait on late-stage conv weights.
"""

import functools
import os
import sys

for _p in ("/opt/trn_rl_repo", "/root/.axon_site/_ro/trn_rl_repo"):
    if os.path.isdir(_p) and _p not in sys.path:
        sys.path.append(_p)

import numpy as np

import concourse.bass as bass
import concourse.mybir as mybir
import concourse.tile as tile
from concourse import bacc, bass_utils

F32 = mybir.dt.float32
BF16 = mybir.dt.bfloat16
U16 = mybir.dt.uint16

B, M, K, KNN = 2, 256, 64, 16
G = 64            # groups per core
NPAIR = G // 2    # 32 pair tiles (2 groups packed in 128 partitions)
NCORES = 8
NEG = -1.0e30
EPS = 1e-5

AF = mybir.ActivationFunctionType
ALU = mybir.AluOpType

# (name, partitions, width) of every constant packed into the blob, in order
_BLOB_LAYOUT = [
    ("iota_col", 128, 1),
    ("neg_iota_col", 128, 1),
    ("ones_row", 1, 1024),
    ("sel96", 96, 32),
    ("s1", 128, 1), ("b1", 128, 1), ("s2", 128, 1), ("b2", 128, 1),
    ("ca1_s", 64, 1), ("ca1_bias", 64, 1),
    ("cb2_blk1", 128, 1), ("cb2_blk2", 64, 1),
    ("e1s", 128, 2), ("e1bias", 128, 2),
    ("e2s", 128, 4), ("e2bias", 128, 4),
]
_BLOB_OFF = {}
_off = 0
for _n, _pp, _w in _BLOB_LAYOUT:
    _BLOB_OFF[_n] = _off
    _off += _w
BLOB_W = _off

# late-stage weights (final 256ch x 64 stage): separate DMA issued after the
# input DMAs so the pair loop can start sooner
_BLOBL_LAYOUT = [
    ("rd0", 128, 256), ("rd1", 128, 256), ("rd2", 128, 256), ("rd3", 128, 256),
    ("rds", 128, 2), ("rdb", 128, 2),
    ("sc1_0", 128, 256), ("sc1_1", 128, 256), ("sc1b", 128, 2),
    ("sc2_0", 128, 256), ("sc2_1", 128, 256), ("sc2b", 128, 2),
    ("n1s", 128, 2), ("n1b", 128, 2), ("n2s", 128, 2), ("n2b", 128, 2),
]
_BLOBL_OFF = {}
_offl = 0
for _n, _pp, _w in _BLOBL_LAYOUT:
    _BLOBL_OFF[_n] = _offl
    _offl += _w
BLOBL_W = _offl

# bf16 constants: pair-loop weights (early) and conv weights (late)
_BLOB16_LAYOUT = [
    ("b16_R2", 128, 1024),
    ("b16_wu1", 64, 64), ("b16_wv1", 64, 64),
    ("b16_wu2", 64, 128), ("b16_wv2", 64, 128),
]
_BLOB16_OFF = {}
_o16 = 0
for _n, _pp, _w in _BLOB16_LAYOUT:
    _BLOB16_OFF[_n] = _o16
    _o16 += _w
BLOB16_W = _o16

_BLOB16L_LAYOUT = [
    ("b16_ca1_a", 64, 64), ("b16_ca1_b", 128, 64), ("b16_ca2", 64, 192),
    ("b16_x1a", 64, 256), ("b16_x1b", 128, 256),
    ("b16_x2a", 128, 512), ("b16_x2b", 128, 512),
]
_BLOB16L_OFF = {}
_o16l = 0
for _n, _pp, _w in _BLOB16L_LAYOUT:
    _BLOB16L_OFF[_n] = _o16l
    _o16l += _w
BLOB16L_W = _o16l


def _np_consts(iw):
    """All constant tensors (iota + host-prepped weights)."""
    f = np.float32
    c = {}
    iota = np.arange(64, dtype=f)
    c["iota_col"] = np.concatenate([iota, iota]).reshape(128, 1)
    c["neg_iota_col"] = -c["iota_col"]
    selg = np.zeros((2, 128), dtype=f)
    selg[0, :64] = 1.0
    selg[1, 64:] = 1.0
    c["selg"] = selg
    c["ones_row"] = np.ones((1, 1024), dtype=f)
    # sel96[c*32+blk, blk] = -0.5: one matmul turns xsq96 (96,128) into
    # -xx/2 for all 4096 points as a (32,128) psum tile
    sel96 = np.zeros((96, 32), dtype=f)
    for _c in range(3):
        for _b in range(32):
            sel96[_c * 32 + _b, _b] = -0.5
    c["sel96"] = sel96
    # replication matrix: R2[p, n*16+k] = (n == p % 64); v-fold accumuland
    _R = np.repeat(np.eye(64, dtype=f), KNN, axis=1)
    c["R2"] = np.vstack([_R, _R])

    e1_w = iw["e1_w"].astype(f)
    W1, W2 = e1_w[:, :64], e1_w[:, 64:]
    c["wu1"] = W1.T.copy()
    c["wv1"] = (W2 - W1).T.copy()
    e2_w = iw["e2_w"].astype(f)
    W21, W22 = e2_w[:, :64], e2_w[:, 64:]
    c["wu2"] = W21.T.copy()
    c["wv2"] = (W22 - W21).T.copy()

    def bn_sb(g, b):
        return (g / np.sqrt(1.0 + EPS)).astype(f), b.astype(f)

    def pair_col(v):
        return np.concatenate([v, v]).reshape(128, 1).astype(f)

    s1, b1 = bn_sb(iw["e1_g"], iw["e1_b"])
    c["s1"], c["b1"] = pair_col(s1), pair_col(b1)
    s2, b2 = bn_sb(iw["e2_g"], iw["e2_b"])
    c["s2"], c["b2"] = s2.reshape(128, 1), b2.reshape(128, 1)

    cal1_w = iw["cal1_w"].astype(f)
    c["ca1_a"] = cal1_w[:, :64].T.copy()
    c["ca1_b"] = cal1_w[:, 64:].T.copy()
    cs, cbv = bn_sb(iw["cal1_g"], iw["cal1_b"])
    c["ca1_s"], c["ca1_bias"] = cs.reshape(64, 1), cbv.reshape(64, 1)

    c["ca2"] = iw["cal2_w"].astype(f).T.copy()
    cb2 = iw["cal2_bias"].astype(f)
    c["cb2_blk1"] = cb2[:128].reshape(128, 1)
    c["cb2_blk2"] = cb2[128:].reshape(64, 1)

    exp1_w = iw["exp1_w"].astype(f)
    c["x1a"] = exp1_w[:, :64].T.copy()
    c["x1b"] = exp1_w[:, 64:].T.copy()
    es, eb = bn_sb(iw["exp1_g"], iw["exp1_b"])
    c["e1s"] = es.reshape(2, 128).T.copy()
    c["e1bias"] = eb.reshape(2, 128).T.copy()

    exp2_w = iw["exp2_w"].astype(f)
    c["x2a"] = exp2_w[:, :128].T.copy()
    c["x2b"] = exp2_w[:, 128:].T.copy()
    es2, eb2 = bn_sb(iw["exp2_g"], iw["exp2_b"])
    c["e2s"] = es2.reshape(4, 128).T.copy()
    c["e2bias"] = eb2.reshape(4, 128).T.copy()

    rdT = iw["red_w"].astype(f).T.reshape(4, 128, 256)
    for i in range(4):
        c[f"rd{i}"] = rdT[i].copy()
    rs, rb = bn_sb(iw["red_g"], iw["red_b"])
    c["rds"] = rs.reshape(2, 128).T.copy()
    c["rdb"] = rb.reshape(2, 128).T.copy()

    sc1T = iw["sc1_w"].astype(f).T.reshape(2, 128, 256)
    c["sc1_0"], c["sc1_1"] = sc1T[0].copy(), sc1T[1].copy()
    c["sc1b"] = iw["sc1_b"].astype(f).reshape(2, 128).T.copy()
    sc2T = iw["sc2_w"].astype(f).T.reshape(2, 128, 256)
    c["sc2_0"], c["sc2_1"] = sc2T[0].copy(), sc2T[1].copy()
    c["sc2b"] = iw["sc2_b"].astype(f).reshape(2, 128).T.copy()

    n1s, n1b = bn_sb(iw["sc_n1_g"], iw["sc_n1_b"])
    c["n1s"] = (2.0 * n1s).reshape(2, 128).T.copy()
    c["n1b"] = n1b.reshape(2, 128).T.copy()
    n2s, n2b = bn_sb(iw["sc_n2_g"], iw["sc_n2_b"])
    c["n2s"] = n2s.reshape(2, 128).T.copy()
    c["n2b"] = n2b.reshape(2, 128).T.copy()
    return c


def _pack_blob(c):
    blob = np.zeros((128, BLOB_W), dtype=np.float32)
    for name, p, w in _BLOB_LAYOUT:
        v = c[name]
        assert v.shape == (p, w), (name, v.shape, (p, w))
        blob[:p, _BLOB_OFF[name]:_BLOB_OFF[name] + w] = v
    return blob


def _pack_blob_late(c):
    blob = np.zeros((128, BLOBL_W), dtype=np.float32)
    for name, p, w in _BLOBL_LAYOUT:
        v = c[name]
        assert v.shape == (p, w), (name, v.shape, (p, w))
        blob[:p, _BLOBL_OFF[name]:_BLOBL_OFF[name] + w] = v
    return blob


def _pack_blob16(c):
    import ml_dtypes
    src16 = {"b16_R2": c["R2"],
             "b16_wu1": c["wu1"], "b16_wv1": c["wv1"],
             "b16_wu2": c["wu2"], "b16_wv2": c["wv2"],
             "b16_ca1_a": c["ca1_a"], "b16_ca1_b": c["ca1_b"],
             "b16_ca2": c["ca2"], "b16_x1a": c["x1a"], "b16_x1b": c["x1b"],
             "b16_x2a": c["x2a"], "b16_x2b": c["x2b"]}
    blob = np.zeros((128, BLOB16_W), dtype=ml_dtypes.bfloat16)
    for name, p, w in _BLOB16_LAYOUT:
        v = src16[name]
        assert v.shape == (p, w), (name, v.shape, (p, w))
        blob[:p, _BLOB16_OFF[name]:_BLOB16_OFF[name] + w] = v.astype(
            ml_dtypes.bfloat16)
    blobl = np.zeros((128, BLOB16L_W), dtype=ml_dtypes.bfloat16)
    for name, p, w in _BLOB16L_LAYOUT:
        v = src16[name]
        assert v.shape == (p, w), (name, v.shape, (p, w))
        blobl[:p, _BLOB16L_OFF[name]:_BLOB16L_OFF[name] + w] = v.astype(
            ml_dtypes.bfloat16)
    return blob, blobl


def _emit(tc, I, out_ap, ctx):
    nc = tc.nc

    cp = ctx.enter_context(tc.tile_pool(name="const", bufs=1))
    wide = ctx.enter_context(tc.tile_pool(name="wide", bufs=1))
    MERGE = int(os.environ.get("K_MERGE", "1"))
    nbig = int(os.environ.get("K_NBIG", "2")) if MERGE else 6
    pp_big = ctx.enter_context(
        tc.tile_pool(name="ps_big", bufs=nbig, space="PSUM"))
    pp_big2 = ctx.enter_context(
        tc.tile_pool(name="ps_big2", bufs=int(os.environ.get("K_NBIG2", "2")),
                     space="PSUM"))
    pp_med = ctx.enter_context(
        tc.tile_pool(name="ps_med", bufs=int(os.environ.get("K_NMED", "2")),
                     space="PSUM"))
    wk = ctx.enter_context(
        tc.tile_pool(name="work", bufs=int(os.environ.get("K_WK", "10"))))
    wk2 = ctx.enter_context(
        tc.tile_pool(name="work2", bufs=int(os.environ.get("K_WK2", "8"))))
    ring = ctx.enter_context(tc.tile_pool(name="ring", bufs=1))

    # ---- DMA order = HWDGE processing order: the knn-critical pieces
    # (x96 for xx, xt3 rows, f32 blob with sel96/iota) go first so the
    # distance/top-k chain starts ~4us earlier; bf16 weights and the input
    # x (first needed by the e1 u/v matmuls) follow; late-stage weights
    # last.
    x96 = wide.tile([96, 128], F32, tag="w96")
    nc.sync.dma_start(out=x96,
                      in_=bass.AP(tensor=I["xt3"].tensor, offset=0,
                                  ap=[[4096, 3], [128, 32], [1, 128]]))
    blob = cp.tile([128, BLOB_W], F32, tag="blob")
    nc.sync.dma_start(out=blob, in_=I["blob"])
    sb = {}
    for name, p, w in _BLOB_LAYOUT:
        sb[name] = blob[0:p, _BLOB_OFF[name]:_BLOB_OFF[name] + w]
    blob16 = cp.tile([128, BLOB16_W], BF16, tag="blob16")
    for name, p, w in _BLOB16_LAYOUT:
        sb[name] = blob16[0:p, _BLOB16_OFF[name]:_BLOB16_OFF[name] + w]
    blob16l = cp.tile([128, BLOB16L_W], BF16, tag="blob16l")
    for name, p, w in _BLOB16L_LAYOUT:
        sb[name] = blob16l[0:p, _BLOB16L_OFF[name]:_BLOB16L_OFF[name] + w]
    xsq96 = wide.tile([96, 128], F32, tag="w96b")
    nc.scalar.activation(out=xsq96, in_=x96, func=AF.Square)
    nxp32 = pp_med.tile([32, 128], F32, tag="med")
    nc.tensor.matmul(nxp32, sb["sel96"], xsq96)
    nxs = wide.tile([32, 128], F32, tag="w96c")
    nc.scalar.activation(out=nxs, in_=nxp32, func=AF.Copy)

    B4 = wide.tile([4, 4096], F32, tag="wC")
    A4 = wide.tile([4, 4096], F32, tag="wB")
    nc.sync.dma_start(out=B4[0:3, :], in_=I["xt3"])
    nc.sync.dma_start(out=B4[3:4, :], in_=nxs)
    nc.sync.dma_start(out=A4[0:3, :], in_=I["xt3"])
    nc.sync.dma_start(out=A4[3:4, :],
                      in_=bass.AP(tensor=I["blob"].tensor,
                                  offset=_BLOB_OFF["ones_row"],
                                  ap=[[0, 1], [0, 4], [1, 1024]]))

    # bf16 weights, input x, then late-stage weights
    nc.sync.dma_start(out=blob16, in_=I["blob16"])
    x = wide.tile([64, 4096], BF16, tag="wD")
    for t in range(8):
        nc.sync.dma_start(out=x[:, t * 512:(t + 1) * 512],
                          in_=I["xt16"][:, t * 512:(t + 1) * 512])
    blobl = cp.tile([128, BLOBL_W], F32, tag="blobl")
    nc.sync.dma_start(out=blobl, in_=I["blobl"])
    nc.sync.dma_start(out=blob16l, in_=I["blob16l"])
    for name, p, w in _BLOBL_LAYOUT:
        sb[name] = blobl[0:p, _BLOBL_OFF[name]:_BLOBL_OFF[name] + w]

    # all pairwise-distance tiles upfront (prologue is DMA-bound, engines
    # idle): qt_all[:, pi*64:...] = q for pair pi; frees the psum med ring
    # and the ACT stream from per-pair distance work
    QTALL = int(os.environ.get("K_QTALL", "1"))
    qt_all = None
    if QTALL:
        qt_all = wide.tile([128, NPAIR * 64], F32, tag="wQT")
        for pi in range(NPAIR):
            _cs1 = slice((2 * pi) * 64, (2 * pi + 1) * 64)
            _cs2 = slice((2 * pi + 1) * 64, (2 * pi + 2) * 64)
            _pdp = pp_med.tile([128, 64], F32, tag="med")
            nc.tensor.matmul(_pdp[0:64, :], A4[:, _cs1], B4[:, _cs1])
            nc.tensor.matmul(_pdp[64:128, :], A4[:, _cs2], B4[:, _cs2])
            nc.scalar.activation(out=qt_all[:, pi * 64:(pi + 1) * 64],
                                 in_=_pdp, func=AF.Copy)

    # gated activations accumulated across all groups (for batched convs)
    x1all = wide.tile([64, 4096], BF16, tag="wE")
    x2all = wide.tile([128, 4096], BF16, tag="wF")
    # final per-group features (512ch as 4 blocks x 64 groups)
    xfin = cp.tile([128, 4, G], F32, tag="xfin")

    BDM = int(os.environ.get("K_BDM", "1"))
    bd_ring = []
    bdv_ring = []
    bdd_ring = []
    for ri in range(int(os.environ.get("K_BD", "6"))):
        if BDM:
            bddt = cp.tile([128, 256], BF16, tag=f"bddring{ri}")
            nc.gpsimd.memset(bddt, 0.0)
            bdd_ring.append(bddt)
            bd_ring.append(bddt[:, 0:128])
            bdv_ring.append(bddt[:, 128:256])
        else:
            bdt = cp.tile([128, 128], BF16, tag=f"bdring{ri}")
            nc.gpsimd.memset(bdt, 0.0)
            bd_ring.append(bdt)
            bdvt = cp.tile([128, 128], BF16, tag=f"bdvring{ri}")
            nc.gpsimd.memset(bdvt, 0.0)
            bdv_ring.append(bdvt)

    ADDP = int(os.environ.get("K_ADDP", "0"))
    E2_MODE = os.environ.get("K_E2M", "a")  # a | b | c | bc
    PCTAIL = int(os.environ.get("K_PCTAIL", "0"))
    SPOOL = int(os.environ.get("K_SPOOL", "1"))
    RELUP = int(os.environ.get("K_RELUP", "0"))
    RELUX2 = int(os.environ.get("K_RELUX2", "0"))

    def _relu_sb(out, in_, s_col, b_col, pool=None):
        # relu(s*in + b) from sbuf: 2 pool ops, or 1 act op
        if RELUP if pool is None else pool:
            tmp = wk.tile(list(in_.shape), F32, tag="rtmp")
            nc.gpsimd.tensor_scalar(out=tmp, in0=in_, scalar1=s_col,
                                    scalar2=b_col, op0=ALU.mult, op1=ALU.add)
            nc.gpsimd.tensor_scalar(out=out, in0=tmp, scalar1=0.0,
                                    scalar2=None, op0=ALU.max)
        else:
            nc.scalar.activation(out=out, in_=in_, func=AF.Relu,
                                 bias=b_col, scale=s_col)
    F1 = int(os.environ.get("K_F1", "1"))
    F2 = int(os.environ.get("K_F2", "1"))
    QTP = int(os.environ.get("K_QTP", "0"))

    def _tadd(out, a, b):
        if ADDP:
            nc.gpsimd.tensor_tensor(out, a, b, op=ALU.add)
        else:
            nc.vector.tensor_add(out, a, b)

    # windowed-max placement: offload part of the (128, 32n, 16k) max blocks
    # from DVE (TensorReduce) to the mostly-idle Pool engine (pairwise-max
    # tree, in place on the psum tile)
    E1_POOL = int(os.environ.get("K_E1P", "0"))
    E2_POOL_ROUNDS = tuple(
        int(v) for v in os.environ.get("K_E2P", "0,0").split(","))
    PC_POOL = int(os.environ.get("K_PCP", "0"))

    def _wmax(gp, out, pool_rounds, k=KNN):
        """max over k of gp (128, 512) viewed as (p, n, k) -> out (128, 512/k).

        pool_rounds pairwise-max rounds run on the Pool engine (in place on
        the psum tile); the remaining window is reduced on DVE."""
        g3 = gp.rearrange("p (n k) -> p n k", k=k)
        w = k
        for _ in range(pool_rounds):
            h = w // 2
            nc.gpsimd.tensor_tensor(g3[:, :, 0:h], g3[:, :, 0:h],
                                    g3[:, :, h:w], op=ALU.max)
            w = h
        nc.vector.reduce_max(out=out, in_=g3[:, :, 0:w],
                             axis=mybir.AxisListType.X)

    # ---------------- per-pair loops (chunked for DMA latency hiding) ----
    CHUNK = int(os.environ.get("K_CHUNK", "8"))
    NIXB = int(os.environ.get("K_NIXB", "8"))
    ixb_ring = [None] * NIXB
    ixr_all = ring.tile([2 * CHUNK, 1024], U16, tag="ixr_all")

    def phase_a(pi):
        g1, g2 = 2 * pi, 2 * pi + 1
        cs1 = slice(g1 * 64, (g1 + 1) * 64)
        cs2 = slice(g2 * 64, (g2 + 1) * 64)

        # q = x_m.x_n - xx_n/2 for both groups -> (128, 64)
        # (row-constant -xx_m/2 term dropped: doesn't change row top-k)
        if QTALL:
            qt = qt_all[:, pi * 64:(pi + 1) * 64]
        else:
            pdp = pp_med.tile([128, 64], F32, tag="med")
            for h, cs in ((0, cs1), (1, cs2)):
                nc.tensor.matmul(pdp[h * 64:(h + 1) * 64, :],
                                 A4[:, cs], B4[:, cs])
            qt = wk.tile([128, 64], F32, tag="qt")
            if QTP:
                nc.gpsimd.tensor_copy(qt, pdp)
            else:
                nc.scalar.activation(out=qt, in_=pdp, func=AF.Copy)

        # top-16 indices per point row
        mx = wk.tile([128, 16], F32, tag="mx")
        ix = wk.tile([128, 16], U16, tag="ix")
        qt2 = wk.tile([128, 64], F32, tag="qt2")
        nc.vector.max(out=mx[:, 0:8], in_=qt)
        nc.vector.max_index(out=ix[:, 0:8], in_max=mx[:, 0:8], in_values=qt)
        nc.vector.match_replace(out=qt2, in_to_replace=mx[:, 0:8],
                                in_values=qt, imm_value=NEG)
        nc.vector.max(out=mx[:, 8:16], in_=qt2)
        nc.vector.max_index(out=ix[:, 8:16], in_max=mx[:, 8:16], in_values=qt2)

        # idx row form (2, 1024) u16, then broadcast to all 128 partitions so
        # the one-hot compare can run at 4x (2-byte sbuf in/out)
        s2 = 2 * (pi % CHUNK)
        ixrows = ixr_all[s2:s2 + 2, :]
        nc.sync.dma_start(out=ixrows, in_=ix)
        ixb = ring.tile([128, 1024], U16, tag=f"ixb{pi % NIXB}")
        ixb_ring[pi % NIXB] = ixb
        nc.sync.dma_start(
            out=ixb, in_=ixrows.unsqueeze(1).broadcast_to((2, 64, 1024)))

    S01_ring = [None] * CHUNK

    def phase_b1(pi):
        """S-matrix + e1 edge conv for pair pi."""
        g1, g2 = 2 * pi, 2 * pi + 1
        cs1 = slice(g1 * 64, (g1 + 1) * 64)
        cs2 = slice(g2 * 64, (g2 + 1) * 64)
        ixb = ixb_ring[pi % NIXB]
        S01 = wk2.tile([128, 1024], BF16, tag="S01")
        S01_ring[pi % CHUNK] = S01
        # S01 = (ix_bcast == iota_p): 4x dve op (all operands 2-byte sbuf)
        seng = nc.gpsimd if (SPOOL == 1 or (SPOOL == 2 and pi % 2)) \
            else nc.vector
        seng.tensor_scalar(out=S01, in0=ixb,
                           scalar1=sb["iota_col"], scalar2=None,
                           op0=ALU.is_equal)

        bd = bd_ring[pi % len(bd_ring)]
        if F1:
            # u and v-fold matmuls into one psum tile; merged copies
            uvv = pp_med.tile([128, 256], F32, tag="med")
            nc.tensor.matmul(uvv[0:64, 0:64], x[:, cs1], sb["b16_wu1"])
            nc.tensor.matmul(uvv[64:128, 64:128], x[:, cs2], sb["b16_wu1"])
            nc.tensor.matmul(uvv[0:64, 128:192], x[:, cs1], sb["b16_wv1"])
            nc.tensor.matmul(uvv[64:128, 192:256], x[:, cs2], sb["b16_wv1"])
            bdv = bdv_ring[pi % len(bdv_ring)]
            if BDM:
                # one copy per partition half: {u block, v block} as a
                # strided access pattern on both sides.
                # col = a*128 + b*64 + c: a selects u/v, b selects group
                bdd = bdd_ring[pi % len(bdd_ring)]
                sv = uvv.rearrange("p (a b c) -> p a b c", a=2, b=2)
                dv = bdd.rearrange("p (a b c) -> p a b c", a=2, b=2)
                nc.scalar.activation(out=dv[0:64, :, 0:1, :],
                                     in_=sv[0:64, :, 0:1, :], func=AF.Copy)
                nc.scalar.activation(out=dv[64:128, :, 1:2, :],
                                     in_=sv[64:128, :, 1:2, :], func=AF.Copy)
            else:
                nc.scalar.activation(
                    out=bd[0:64, 0:64], in_=uvv[0:64, 0:64], func=AF.Copy)
                nc.scalar.activation(
                    out=bd[64:128, 64:128], in_=uvv[64:128, 64:128],
                    func=AF.Copy)
                nc.scalar.activation(
                    out=bdv[0:64, 0:64], in_=uvv[0:64, 128:192], func=AF.Copy)
                nc.scalar.activation(
                    out=bdv[64:128, 64:128], in_=uvv[64:128, 192:256],
                    func=AF.Copy)
        else:
            uv1 = pp_med.tile([128, 128], F32, tag="med")
            nc.tensor.matmul(uv1[0:64, 0:64], x[:, cs1], sb["b16_wu1"])
            nc.tensor.matmul(uv1[64:128, 64:128], x[:, cs2], sb["b16_wu1"])
            nc.scalar.activation(out=bd[0:64, 0:64], in_=uv1[0:64, 0:64],
                                 func=AF.Copy)
            nc.scalar.activation(out=bd[64:128, 64:128],
                                 in_=uv1[64:128, 64:128], func=AF.Copy)
        m1 = wk.tile([128, 64], F32, tag="m1")
        if MERGE:
            g1p = pp_big2.tile([128, 1024], F32, tag="big2")
            for half in range(2):
                csl = slice(half * 512, (half + 1) * 512)
                gh = g1p[:, csl]
                if F1:
                    nc.tensor.matmul(gh, bd, S01[:, csl], start=True,
                                     stop=False)
                    nc.tensor.matmul(gh, bdv, sb["b16_R2"][:, csl],
                                     start=False, stop=True)
                else:
                    nc.tensor.matmul(gh, bd, S01[:, csl])
            nc.vector.reduce_max(
                out=m1, in_=g1p.rearrange("p (n k) -> p n k", k=KNN),
                axis=mybir.AxisListType.X)
        else:
            for half in range(2):
                csl = slice(half * 512, (half + 1) * 512)
                g1p = pp_big.tile([128, 512], F32, tag="big")
                if F1:
                    nc.tensor.matmul(g1p, bd, S01[:, csl], start=True,
                                     stop=False)
                    nc.tensor.matmul(g1p, bdv, sb["b16_R2"][:, csl],
                                     start=False, stop=True)
                else:
                    nc.tensor.matmul(g1p, bd, S01[:, csl])
                _wmax(g1p, m1[:, half * 32:(half + 1) * 32], E1_POOL)
        if F1:
            _relu_sb(x1all[:, cs1], m1[0:64, :], sb["s1"][0:64],
                     sb["b1"][0:64])
            _relu_sb(x1all[:, cs2], m1[64:128, :], sb["s1"][64:128],
                     sb["b1"][64:128])
        else:
            v1 = pp_med.tile([128, 64], F32, tag="med")
            nc.tensor.matmul(v1[0:64, :], sb["b16_wv1"], x[:, cs1])
            nc.tensor.matmul(v1[64:128, :], sb["b16_wv1"], x[:, cs2])
            t1a = wk.tile([64, 64], F32, tag="t1a")
            _tadd(t1a, m1[0:64, :], v1[0:64, :])
            t1b = wk.tile([128, 64], F32, tag="t1b")
            _tadd(t1b[64:128, :], m1[64:128, :], v1[64:128, :])
            nc.scalar.activation(out=x1all[:, cs1], in_=t1a, func=AF.Relu,
                                 bias=sb["b1"][0:64], scale=sb["s1"][0:64])
            nc.scalar.activation(out=x1all[:, cs2], in_=t1b[64:128, :],
                                 func=AF.Relu,
                                 bias=sb["b1"][64:128], scale=sb["s1"][64:128])

    def phase_b2(pi):
        """e2 edge conv for pair pi (consumes x1all + S01)."""
        g1, g2 = 2 * pi, 2 * pi + 1
        cs1 = slice(g1 * 64, (g1 + 1) * 64)
        cs2 = slice(g2 * 64, (g2 + 1) * 64)
        S01 = S01_ring[pi % CHUNK]
        for h, cs in ((0, cs1), (1, cs2)):
            xg = x1all[:, cs]
            psl = slice(h * 64, (h + 1) * 64)
            if E2_MODE == "a" and F2:
                uvp = pp_med.tile([64, 256], F32, tag="med")
                nc.tensor.matmul(uvp[:, 0:128], xg, sb["b16_wu2"])
                nc.tensor.matmul(uvp[:, 128:256], xg, sb["b16_wv2"])
                uvs = wk.tile([128, 256], BF16, tag="uvs")
                nc.scalar.activation(out=uvs[psl, :], in_=uvp, func=AF.Copy)
                uT2 = uvs[:, 0:128]
                vv2 = uvs[:, 128:256]
            else:
                uT2 = wk.tile([128, 128], BF16, tag="uT2")
                uT2p = pp_med.tile([64, 128], F32, tag="med")
                nc.tensor.matmul(uT2p, xg, sb["b16_wu2"])
                nc.scalar.activation(out=uT2[psl, :], in_=uT2p, func=AF.Copy)
            if E2_MODE == "a":
                m2 = wk.tile([128, 64], F32, tag="m2")
                if MERGE:
                    g2p = pp_big2.tile([128, 1024], F32, tag="big2")
                    for half in range(2):
                        csl = slice(half * 512, (half + 1) * 512)
                        gh = g2p[:, csl]
                        if F2:
                            nc.tensor.matmul(gh, uT2[psl, :], S01[psl, csl],
                                             start=True, stop=False)
                            nc.tensor.matmul(gh, vv2[psl, :],
                                             sb["b16_R2"][psl, csl],
                                             start=False, stop=True)
                        else:
                            nc.tensor.matmul(gh, uT2[psl, :], S01[psl, csl])
                    nc.vector.reduce_max(
                        out=m2, in_=g2p.rearrange("p (n k) -> p n k", k=KNN),
                        axis=mybir.AxisListType.X)
                else:
                    for half in range(2):
                        csl = slice(half * 512, (half + 1) * 512)
                        g2p = pp_big.tile([128, 512], F32, tag="big")
                        if F2:
                            nc.tensor.matmul(g2p, uT2[psl, :], S01[psl, csl],
                                             start=True, stop=False)
                            nc.tensor.matmul(g2p, vv2[psl, :],
                                             sb["b16_R2"][psl, csl],
                                             start=False, stop=True)
                        else:
                            nc.tensor.matmul(g2p, uT2[psl, :], S01[psl, csl])
                        _wmax(g2p, m2[:, half * 32:(half + 1) * 32],
                              E2_POOL_ROUNDS[half])
                if F2:
                    _relu_sb(x2all[:, cs], m2, sb["s2"], sb["b2"],
                             pool=RELUX2)
                else:
                    v2 = pp_med.tile([128, 64], F32, tag="med")
                    nc.tensor.matmul(v2, sb["b16_wv2"], xg)
                    t2 = wk.tile([128, 64], F32, tag="t2")
                    _tadd(t2, m2, v2)
                    nc.scalar.activation(out=x2all[:, cs], in_=t2,
                                         func=AF.Relu,
                                         bias=sb["b2"], scale=sb["s2"])
                continue
            # v-folded path: g2p = uT2.S + vT2.R, then relu(bn) at psum
            # egress (valid pre-max: bn scale > 0), max-tree on sbuf bf16
            vv2p = pp_med.tile([64, 128], F32, tag="med")
            nc.tensor.matmul(vv2p, xg, sb["b16_wv2"])
            vv2 = wk.tile([64, 128], BF16, tag="vv2")
            nc.scalar.activation(out=vv2, in_=vv2p, func=AF.Copy)
            for half in range(2):
                csl = slice(half * 512, (half + 1) * 512)
                g2p = pp_big.tile([128, 512], F32, tag="big")
                nc.tensor.matmul(g2p, uT2[psl, :], S01[psl, csl],
                                 start=True, stop=False)
                nc.tensor.matmul(g2p, vv2, sb["b16_R2"][0:64, csl],
                                 start=False, stop=True)
                x2pre = wk.tile([128, 512], BF16, tag="x2pre")
                nc.scalar.activation(out=x2pre, in_=g2p, func=AF.Relu,
                                     bias=sb["b2"], scale=sb["s2"])
                p3 = x2pre.rearrange("p (n k) -> p n k", k=KNN)
                eng = nc.gpsimd if E2_MODE == "c" or (
                    E2_MODE == "bc" and half == 1) else nc.vector
                eng.tensor_tensor(p3[:, :, 0:8], p3[:, :, 0:8],
                                  p3[:, :, 8:16], op=ALU.max)
                eng.tensor_tensor(p3[:, :, 0:4], p3[:, :, 0:4],
                                  p3[:, :, 4:8], op=ALU.max)
                eng.tensor_tensor(p3[:, :, 0:2], p3[:, :, 0:2],
                                  p3[:, :, 2:4], op=ALU.max)
                osub = slice(cs.start + half * 32, cs.start + half * 32 + 32)
                eng.tensor_tensor(x2all[:, osub], p3[:, :, 0:1].squeeze(-1),
                                  p3[:, :, 1:2].squeeze(-1), op=ALU.max)


    # ---------------- batched calib/gate/expansion (per 512-col window) --
    c1all = wide.tile([64, 4096], BF16, tag="wG")
    sigA = wide.tile([64, 4096], BF16, tag="wA")
    sigX2 = wide.tile([128, 4096], BF16, tag="wH")
    pcr = ctx.enter_context(
        tc.tile_pool(name="pcring", bufs=int(os.environ.get("K_NPC", "3"))))
    ee0 = wide.tile([128, 4096], BF16, tag="wK")
    ee1 = wide.tile([128, 4096], BF16, tag="wL")
    ee = [ee0, ee1]

    def phase_c(j):
        if PCSPLIT:
            phase_c_part(j, 0)
            phase_c_part(j, 1)
        else:
            phase_c_part(j, None)

    def phase_c_part(j, part):
        if part is None:
            csl = slice(j * 512, (j + 1) * 512)
            fsl = slice(j * 8, (j + 1) * 8)
        else:
            csl = slice(j * 512 + part * 256, j * 512 + part * 256 + 256)
            fsl = slice(j * 8 + part * 4, j * 8 + part * 4 + 4)
        W = csl.stop - csl.start
        c1p = pp_big.tile([64, W], F32, tag="big")
        nc.tensor.matmul(c1p, sb["b16_ca1_a"], x1all[:, csl], start=True,
                         stop=False)
        nc.tensor.matmul(c1p, sb["b16_ca1_b"], x2all[:, csl], start=False,
                         stop=True)
        nc.scalar.activation(out=c1all[:, csl], in_=c1p, func=AF.Relu,
                             bias=sb["ca1_bias"], scale=sb["ca1_s"])
        sp1 = pp_big.tile([128, W], F32, tag="big")
        nc.tensor.matmul(sp1, sb["b16_ca2"][:, 0:128], c1all[:, csl])
        nc.scalar.activation(out=sigA[:, csl], in_=sp1[0:64, :],
                             func=AF.Sigmoid, bias=sb["cb2_blk1"][0:64])
        nc.scalar.activation(out=sigX2[0:64, csl], in_=sp1[64:128, :],
                             func=AF.Sigmoid, bias=sb["cb2_blk1"][64:128])
        sp2 = pp_big.tile([64, W], F32, tag="big")
        nc.tensor.matmul(sp2, sb["b16_ca2"][:, 128:192], c1all[:, csl])
        nc.scalar.activation(out=sigX2[64:128, csl], in_=sp2, func=AF.Sigmoid,
                             bias=sb["cb2_blk2"])
        p1t = pcr.tile([64, W], BF16, tag="p1r")
        p2t = pcr.tile([128, W], BF16, tag="p2r")
        nc.gpsimd.tensor_mul(p1t, x1all[:, csl], sigA[:, csl])
        nc.gpsimd.tensor_mul(p2t, x2all[:, csl], sigX2[:, csl])
        for b in range(2):
            ep = pp_big.tile([128, W], F32, tag="big")
            osl = slice(b * 128, (b + 1) * 128)
            nc.tensor.matmul(ep, sb["b16_x1a"][:, osl], p1t,
                             start=True, stop=False)
            nc.tensor.matmul(ep, sb["b16_x1b"][:, osl], p2t,
                             start=False, stop=True)
            nc.scalar.activation(out=ee[b][:, csl], in_=ep, func=AF.Relu,
                                 bias=sb["e1bias"][:, b:b + 1],
                                 scale=sb["e1s"][:, b:b + 1])
        tailwin = PCTAIL and j >= 6
        for b in range(4):
            if tailwin and b >= 2:
                xp = pp_big2.tile([128, W], F32, tag="big2")
            else:
                xp = pp_big.tile([128, W], F32, tag="big")
            osl = slice(b * 128, (b + 1) * 128)
            nc.tensor.matmul(xp, sb["b16_x2a"][:, osl], ee[0][:, csl],
                             start=True, stop=False)
            nc.tensor.matmul(xp, sb["b16_x2b"][:, osl], ee[1][:, csl],
                             start=False, stop=True)
            xm = wk2.tile([128, W // 64], F32, tag="xm")
            _wmax(xp, xm, PC_POOL, k=64)
            nc.scalar.activation(out=xfin[:, b, fsl], in_=xm,
                                 func=AF.Relu,
                                 bias=sb["e2bias"][:, b:b + 1],
                                 scale=sb["e2s"][:, b:b + 1])

    STAG = int(os.environ.get("K_STAG", "5"))
    PCSPLIT = int(os.environ.get("K_PCSPLIT", "0"))
    FLAT = int(os.environ.get("K_FLAT", "1"))
    if FLAT:
        # one continuous pipeline over all 32 pairs: phase_a leads b1 by LA,
        # b2 trails b1 by STAG, each phase_c window fires as its 4 pairs
        # complete. No chunk boundaries, so the pipeline never drains.
        # Ring safety: ixb/ixr slots (8) are rewritten 8-LA b1-steps after
        # their reader; S01 slots (WK2) rewritten WK2-STAG steps after.
        LA = int(os.environ.get("K_LA", "3"))
        for pi in range(LA):
            phase_a(pi)

        PCD = int(os.environ.get("K_PCD", "3"))

        def _after_b2(done):
            # window w is emitted at done == 4*w + 3 + PCD
            if done >= PCD + 3 and (done - PCD - 3) % 4 == 0:
                phase_c((done - PCD - 3) // 4)
            if done == NPAIR - 1:
                for w in range((done - PCD - 3) // 4 + 1, NPAIR // 4):
                    phase_c(w)
        for pi in range(NPAIR):
            phase_b1(pi)
            if pi + LA < NPAIR:
                phase_a(pi + LA)
            if pi >= STAG:
                phase_b2(pi - STAG)
                _after_b2(pi - STAG)
        for pi in range(NPAIR - STAG, NPAIR):
            phase_b2(pi)
            _after_b2(pi)
    else:
        nwin = CHUNK // 4
        pending_c = []
        for chunk in range(NPAIR // CHUNK):
            base = chunk * CHUNK
            for pi in range(base, base + CHUNK):
                phase_a(pi)
            for w in pending_c:
                phase_c(w)
            pending_c = []
            for i in range(CHUNK):
                phase_b1(base + i)
                if i >= STAG:
                    phase_b2(base + i - STAG)
            for i in range(CHUNK - STAG, CHUNK):
                phase_b2(base + i)
            pending_c = list(range(nwin * chunk, nwin * chunk + nwin))
        for w in pending_c:
            phase_c(w)

    # ---------------- final stage (256ch x 64 group-cols) ---------------
    tt = wk.tile([128, 2, G], F32, tag="tt")
    FSPLIT = int(os.environ.get("K_FSPLIT", "0"))
    for b in range(2):
        osl = slice(b * 128, (b + 1) * 128)
        rp = pp_med.tile([128, G], F32, tag="med")
        rngs = (slice(0, 48), slice(48, 64)) if FSPLIT else (slice(0, G),)
        for rng in rngs:
            for cb in range(4):
                nc.tensor.matmul(rp[:, rng], sb[f"rd{cb}"][:, osl],
                                 xfin[:, cb, rng],
                                 start=(cb == 0), stop=(cb == 3))
        rr = wk.tile([128, G], F32, tag="rr")
        nc.scalar.activation(out=rr, in_=rp, func=AF.Relu,
                             bias=sb["rdb"][:, b:b + 1],
                             scale=sb["rds"][:, b:b + 1])
        nc.vector.tensor_scalar(out=tt[:, b, :], in0=rr,
                                scalar1=sb["n1s"][:, b:b + 1],
                                scalar2=sb["n1b"][:, b:b + 1],
                                op0=ALU.mult, op1=ALU.add)
    hh = wk.tile([128, 2, G], F32, tag="hh")
    for b in range(2):
        osl = slice(b * 128, (b + 1) * 128)
        hp = pp_med.tile([128, G], F32, tag="med")
        for cb in range(2):
            nc.tensor.matmul(hp, sb[f"sc1_{cb}"][:, osl], tt[:, cb, :],
                             start=(cb == 0), stop=(cb == 1))
        nc.scalar.activation(out=hh[:, b, :], in_=hp, func=AF.Relu,
                             bias=sb["sc1b"][:, b:b + 1])
    for b in range(2):
        osl = slice(b * 128, (b + 1) * 128)
        h2p = pp_med.tile([128, G], F32, tag="med")
        for cb in range(2):
            nc.tensor.matmul(h2p, sb[f"sc2_{cb}"][:, osl], hh[:, cb, :],
                             start=(cb == 0), stop=(cb == 1))
        s2sum = wk.tile([128, G], F32, tag="s2sum")
        nc.vector.tensor_scalar(out=s2sum, in0=h2p,
                                scalar1=sb["sc2b"][:, b:b + 1], scalar2=None,
                                op0=ALU.add)
        s2t = wk.tile([128, G], F32, tag="s2t")
        nc.vector.tensor_add(s2t, s2sum, tt[:, b, :])
        osb = wk.tile([128, G], F32, tag="osb")
        nc.vector.tensor_scalar(out=osb, in0=s2t,
                                scalar1=sb["n2s"][:, b:b + 1],
                                scalar2=sb["n2b"][:, b:b + 1],
                                op0=ALU.mult, op1=ALU.add)
        nc.sync.dma_start(out=out_ap[b * 128:(b + 1) * 128, :], in_=osb)


@functools.lru_cache(maxsize=1)
def _build():
    nc = bacc.Bacc("TRN2", target_bir_lowering=False, debug=False,
                   num_devices=NCORES)
    I = {}
    I["xt16"] = nc.dram_tensor("xt16", (64, 4096), BF16,
                               kind="ExternalInput").ap()
    I["xt3"] = nc.dram_tensor("xt3", (3, 4096), F32,
                              kind="ExternalInput").ap()
    I["blob"] = nc.dram_tensor("blob", (128, BLOB_W), F32,
                               kind="ExternalInput").ap()
    I["blobl"] = nc.dram_tensor("blobl", (128, BLOBL_W), F32,
                                kind="ExternalInput").ap()
    I["blob16"] = nc.dram_tensor("blob16", (128, BLOB16_W), BF16,
                                 kind="ExternalInput").ap()
    I["blob16l"] = nc.dram_tensor("blob16l", (128, BLOB16L_W), BF16,
                                  kind="ExternalInput").ap()
    out_ap = nc.dram_tensor("out", (256, G), F32, kind="ExternalOutput").ap()
    from contextlib import ExitStack
    with tile.TileContext(nc) as tc, ExitStack() as ctx:
        _emit(tc, I, out_ap, ctx)
    nc.compile()
    return nc


def kernel(**inputs):
    nc = _build()
    consts = _np_consts(inputs)
    blob = _pack_blob(consts)
    blobl = _pack_blob_late(consts)
    blob16v, blob16lv = _pack_blob16(consts)

    xyz = inputs["xyz"].astype(np.float32)      # (2, 256, 64, 3)
    feats = inputs["feats"].astype(np.float32)  # (2, 256, 64, 61)
    xf_full = np.concatenate([xyz, feats], axis=-1).reshape(512 * 64, 64)

    in_maps = []
    for c in range(NCORES):
        import ml_dtypes
        sh = xf_full[c * 4096:(c + 1) * 4096, :]
        in_maps.append({
            "blob": blob,
            "blobl": blobl,
            "blob16": blob16v,
            "blob16l": blob16lv,
            "xt16": np.ascontiguousarray(sh.T.astype(ml_dtypes.bfloat16)),
            "xt3": np.ascontiguousarray(sh.T[0:3, :]),
        })

    trace = bool(int(os.environ.get("KERNEL_TRACE", "0")))
    try:
        res = bass_utils.run_bass_kernel_spmd(
            nc, in_maps, core_ids=list(range(NCORES)), trace=trace)
    except ModuleNotFoundError:
        res = bass_utils.run_bass_kernel_spmd(
            nc, in_maps, core_ids=list(range(NCORES)))
    if trace and res.exec_time_ns is not None:
        print(f"HW exec time: {res.exec_time_ns} ns")
        if res.instructions_and_trace is not None:
            print(f"trace: {res.instructions_and_trace[1]}")
        kernel.last_results = res

    out = np.empty((2, 256, 256), dtype=np.float32)
    for c in range(NCORES):
        o = res.results[c]["out"]              # (256, 64)
        b, mlo = divmod(c * G, 256)
        out[b, :, mlo:mlo + G] = o
    return out


if __name__ == "__main__":
    print("building bass graph...")
    nc = _build()
    print("graph built ok")



# revision 73
# speedup vs baseline: 1.0240x; 1.0119x over previous
"""Trainium2 Bass kernel for nn_AttnGNNLayer (EdgeConv-style GNN layer).

Data-parallel over the B*M=512 group axis: 64 groups per core on 8 cores.

Per-group pipeline (K=64 points, knn=16):
  - distance proxy q = x^T x - xx/2 (one ones-row accum matmul; xx from a
    tall (96,128) Square + one f32 matmul against a packed selector)
  - top-16 neighbor indices via DVE max8 / match_replace / max_index
  - one-hot gather matrix S[j, n*16+k]: idx rows DMA'd to (2,1024) u16,
    broadcast-DMA'd to all 128 partitions, then ONE tensor_scalar is_equal
    vs a per-partition iota (4x-mode eligible; runs on the Pool engine,
    which may only touch SBUF - GPSIMD cannot access PSUM on silicon)
  - edge conv: gather matmul accumulates bd@S + bdv@R (R = kron(I,1_16)),
    folding the center term v into psum so the windowed reduce_max (DVE,
    merged (128,1024) 2-bank tiles) directly yields max_k(u[idx]+v); the
    bn+relu then applies at psum egress
  - all 1x1 convs batched over all 64*64=4096 points per core on PE

Emission is software-pipelined: per 8-pair chunk, phase_a (knn) x8, then
b1 (S + e1) / b2 (e2) interleaved with a stagger of 4 so every engine's
in-order stream has other pairs' work between dependent ops; phase_c
(calib/gate/expansion windows) is deferred past the next chunk's phase_a.
Weight constants ship in early/late blobs (f32 + bf16) so the pair loop
does not wait on late-stage conv weights.
"""

import functools
import os
import sys

for _p in ("/opt/trn_rl_repo", "/root/.axon_site/_ro/trn_rl_repo"):
    if os.path.isdir(_p) and _p not in sys.path:
        sys.path.append(_p)

import numpy as np

import concourse.bass as bass
import concourse.mybir as mybir
import concourse.tile as tile
from concourse import bacc, bass_utils

F32 = mybir.dt.float32
BF16 = mybir.dt.bfloat16
U16 = mybir.dt.uint16

B, M, K, KNN = 2, 256, 64, 16
G = 64            # groups per core
NPAIR = G // 2    # 32 pair tiles (2 groups packed in 128 partitions)
NCORES = 8
NEG = -1.0e30
EPS = 1e-5

AF = mybir.ActivationFunctionType
ALU = mybir.AluOpType

# (name, partitions, width) of every constant packed into the blob, in order
_BLOB_LAYOUT = [
    ("iota_col", 128, 1),
    ("neg_iota_col", 128, 1),
    ("ones_row", 1, 1024),
    ("sel96", 96, 32),
    ("s1", 128, 1), ("b1", 128, 1), ("s2", 128, 1), ("b2", 128, 1),
    ("ca1_s", 64, 1), ("ca1_bias", 64, 1),
    ("cb2_blk1", 128, 1), ("cb2_blk2", 64, 1),
    ("e1s", 128, 2), ("e1bias", 128, 2),
    ("e2s", 128, 4), ("e2bias", 128, 4),
]
_BLOB_OFF = {}
_off = 0
for _n, _pp, _w in _BLOB_LAYOUT:
    _BLOB_OFF[_n] = _off
    _off += _w
BLOB_W = _off

# late-stage weights (final 256ch x 64 stage): separate DMA issued after the
# input DMAs so the pair loop can start sooner
_BLOBL_LAYOUT = [
    ("rd0", 128, 256), ("rd1", 128, 256), ("rd2", 128, 256), ("rd3", 128, 256),
    ("rds", 128, 2), ("rdb", 128, 2),
    ("sc1_0", 128, 256), ("sc1_1", 128, 256), ("sc1b", 128, 2),
    ("sc2_0", 128, 256), ("sc2_1", 128, 256), ("sc2b", 128, 2),
    ("n1s", 128, 2), ("n1b", 128, 2), ("n2s", 128, 2), ("n2b", 128, 2),
]
_BLOBL_OFF = {}
_offl = 0
for _n, _pp, _w in _BLOBL_LAYOUT:
    _BLOBL_OFF[_n] = _offl
    _offl += _w
BLOBL_W = _offl

# bf16 constants: pair-loop weights (early) and conv weights (late)
_BLOB16_LAYOUT = [
    ("b16_R2", 128, 1024),
    ("b16_wu1", 64, 64), ("b16_wv1", 64, 64),
    ("b16_wu2", 64, 128), ("b16_wv2", 64, 128),
]
_BLOB16_OFF = {}
_o16 = 0
for _n, _pp, _w in _BLOB16_LAYOUT:
    _BLOB16_OFF[_n] = _o16
    _o16 += _w
BLOB16_W = _o16

_BLOB16L_LAYOUT = [
    ("b16_ca1_a", 64, 64), ("b16_ca1_b", 128, 64), ("b16_ca2", 64, 192),
    ("b16_x1a", 64, 256), ("b16_x1b", 128, 256),
    ("b16_x2a", 128, 512), ("b16_x2b", 128, 512),
]
_BLOB16L_OFF = {}
_o16l = 0
for _n, _pp, _w in _BLOB16L_LAYOUT:
    _BLOB16L_OFF[_n] = _o16l
    _o16l += _w
BLOB16L_W = _o16l


def _np_consts(iw):
    """All constant tensors (iota + host-prepped weights)."""
    f = np.float32
    c = {}
    iota = np.arange(64, dtype=f)
    c["iota_col"] = np.concatenate([iota, iota]).reshape(128, 1)
    c["neg_iota_col"] = -c["iota_col"]
    selg = np.zeros((2, 128), dtype=f)
    selg[0, :64] = 1.0
    selg[1, 64:] = 1.0
    c["selg"] = selg
    c["ones_row"] = np.ones((1, 1024), dtype=f)
    # sel96[c*32+blk, blk] = -0.5: one matmul turns xsq96 (96,128) into
    # -xx/2 for all 4096 points as a (32,128) psum tile
    sel96 = np.zeros((96, 32), dtype=f)
    for _c in range(3):
        for _b in range(32):
            sel96[_c * 32 + _b, _b] = -0.5
    c["sel96"] = sel96
    # replication matrix: R2[p, n*16+k] = (n == p % 64); v-fold accumuland
    _R = np.repeat(np.eye(64, dtype=f), KNN, axis=1)
    c["R2"] = np.vstack([_R, _R])

    e1_w = iw["e1_w"].astype(f)
    W1, W2 = e1_w[:, :64], e1_w[:, 64:]
    c["wu1"] = W1.T.copy()
    c["wv1"] = (W2 - W1).T.copy()
    e2_w = iw["e2_w"].astype(f)
    W21, W22 = e2_w[:, :64], e2_w[:, 64:]
    c["wu2"] = W21.T.copy()
    c["wv2"] = (W22 - W21).T.copy()

    def bn_sb(g, b):
        return (g / np.sqrt(1.0 + EPS)).astype(f), b.astype(f)

    def pair_col(v):
        return np.concatenate([v, v]).reshape(128, 1).astype(f)

    s1, b1 = bn_sb(iw["e1_g"], iw["e1_b"])
    c["s1"], c["b1"] = pair_col(s1), pair_col(b1)
    s2, b2 = bn_sb(iw["e2_g"], iw["e2_b"])
    c["s2"], c["b2"] = s2.reshape(128, 1), b2.reshape(128, 1)

    cal1_w = iw["cal1_w"].astype(f)
    c["ca1_a"] = cal1_w[:, :64].T.copy()
    c["ca1_b"] = cal1_w[:, 64:].T.copy()
    cs, cbv = bn_sb(iw["cal1_g"], iw["cal1_b"])
    c["ca1_s"], c["ca1_bias"] = cs.reshape(64, 1), cbv.reshape(64, 1)

    c["ca2"] = iw["cal2_w"].astype(f).T.copy()
    cb2 = iw["cal2_bias"].astype(f)
    c["cb2_blk1"] = cb2[:128].reshape(128, 1)
    c["cb2_blk2"] = cb2[128:].reshape(64, 1)

    exp1_w = iw["exp1_w"].astype(f)
    c["x1a"] = exp1_w[:, :64].T.copy()
    c["x1b"] = exp1_w[:, 64:].T.copy()
    es, eb = bn_sb(iw["exp1_g"], iw["exp1_b"])
    c["e1s"] = es.reshape(2, 128).T.copy()
    c["e1bias"] = eb.reshape(2, 128).T.copy()

    exp2_w = iw["exp2_w"].astype(f)
    c["x2a"] = exp2_w[:, :128].T.copy()
    c["x2b"] = exp2_w[:, 128:].T.copy()
    es2, eb2 = bn_sb(iw["exp2_g"], iw["exp2_b"])
    c["e2s"] = es2.reshape(4, 128).T.copy()
    c["e2bias"] = eb2.reshape(4, 128).T.copy()

    rdT = iw["red_w"].astype(f).T.reshape(4, 128, 256)
    for i in range(4):
        c[f"rd{i}"] = rdT[i].copy()
    rs, rb = bn_sb(iw["red_g"], iw["red_b"])
    c["rds"] = rs.reshape(2, 128).T.copy()
    c["rdb"] = rb.reshape(2, 128).T.copy()

    sc1T = iw["sc1_w"].astype(f).T.reshape(2, 128, 256)
    c["sc1_0"], c["sc1_1"] = sc1T[0].copy(), sc1T[1].copy()
    c["sc1b"] = iw["sc1_b"].astype(f).reshape(2, 128).T.copy()
    sc2T = iw["sc2_w"].astype(f).T.reshape(2, 128, 256)
    c["sc2_0"], c["sc2_1"] = sc2T[0].copy(), sc2T[1].copy()
    c["sc2b"] = iw["sc2_b"].astype(f).reshape(2, 128).T.copy()

    n1s, n1b = bn_sb(iw["sc_n1_g"], iw["sc_n1_b"])
    c["n1s"] = (2.0 * n1s).reshape(2, 128).T.copy()
    c["n1b"] = n1b.reshape(2, 128).T.copy()
    n2s, n2b = bn_sb(iw["sc_n2_g"], iw["sc_n2_b"])
    c["n2s"] = n2s.reshape(2, 128).T.copy()
    c["n2b"] = n2b.reshape(2, 128).T.copy()
    return c


def _pack_blob(c):
    blob = np.zeros((128, BLOB_W), dtype=np.float32)
    for name, p, w in _BLOB_LAYOUT:
        v = c[name]
        assert v.shape == (p, w), (name, v.shape, (p, w))
        blob[:p, _BLOB_OFF[name]:_BLOB_OFF[name] + w] = v
    return blob


def _pack_blob_late(c):
    blob = np.zeros((128, BLOBL_W), dtype=np.float32)
    for name, p, w in _BLOBL_LAYOUT:
        v = c[name]
        assert v.shape == (p, w), (name, v.shape, (p, w))
        blob[:p, _BLOBL_OFF[name]:_BLOBL_OFF[name] + w] = v
    return blob


def _pack_blob16(c):
    import ml_dtypes
    src16 = {"b16_R2": c["R2"],
             "b16_wu1": c["wu1"], "b16_wv1": c["wv1"],
             "b16_wu2": c["wu2"], "b16_wv2": c["wv2"],
             "b16_ca1_a": c["ca1_a"], "b16_ca1_b": c["ca1_b"],
             "b16_ca2": c["ca2"], "b16_x1a": c["x1a"], "b16_x1b": c["x1b"],
             "b16_x2a": c["x2a"], "b16_x2b": c["x2b"]}
    blob = np.zeros((128, BLOB16_W), dtype=ml_dtypes.bfloat16)
    for name, p, w in _BLOB16_LAYOUT:
        v = src16[name]
        assert v.shape == (p, w), (name, v.shape, (p, w))
        blob[:p, _BLOB16_OFF[name]:_BLOB16_OFF[name] + w] = v.astype(
            ml_dtypes.bfloat16)
    blobl = np.zeros((128, BLOB16L_W), dtype=ml_dtypes.bfloat16)
    for name, p, w in _BLOB16L_LAYOUT:
        v = src16[name]
        assert v.shape == (p, w), (name, v.shape, (p, w))
        blobl[:p, _BLOB16L_OFF[name]:_BLOB16L_OFF[name] + w] = v.astype(
            ml_dtypes.bfloat16)
    return blob, blobl


def _emit(tc, I, out_ap, ctx):
    nc = tc.nc

    cp = ctx.enter_context(tc.tile_pool(name="const", bufs=1))
    wide = ctx.enter_context(tc.tile_pool(name="wide", bufs=1))
    MERGE = int(os.environ.get("K_MERGE", "1"))
    nbig = int(os.environ.get("K_NBIG", "2")) if MERGE else 6
    pp_big = ctx.enter_context(
        tc.tile_pool(name="ps_big", bufs=nbig, space="PSUM"))
    pp_big2 = ctx.enter_context(
        tc.tile_pool(name="ps_big2", bufs=int(os.environ.get("K_NBIG2", "2")),
                     space="PSUM"))
    pp_med = ctx.enter_context(
        tc.tile_pool(name="ps_med", bufs=int(os.environ.get("K_NMED", "2")),
                     space="PSUM"))
    wk = ctx.enter_context(
        tc.tile_pool(name="work", bufs=int(os.environ.get("K_WK", "10"))))
    wk2 = ctx.enter_context(
        tc.tile_pool(name="work2", bufs=int(os.environ.get("K_WK2", "8"))))
    ring = ctx.enter_context(tc.tile_pool(name="ring", bufs=1))

    # ---- DMA order = HWDGE processing order: the knn-critical pieces
    # (x96 for xx, xt3 rows, f32 blob with sel96/iota) go first so the
    # distance/top-k chain starts ~4us earlier; bf16 weights and the input
    # x (first needed by the e1 u/v matmuls) follow; late-stage weights
    # last.
    x96 = wide.tile([96, 128], F32, tag="w96")
    nc.sync.dma_start(out=x96,
                      in_=bass.AP(tensor=I["xt3"].tensor, offset=0,
                                  ap=[[4096, 3], [128, 32], [1, 128]]))
    blob = cp.tile([128, BLOB_W], F32, tag="blob")
    nc.sync.dma_start(out=blob, in_=I["blob"])
    sb = {}
    for name, p, w in _BLOB_LAYOUT:
        sb[name] = blob[0:p, _BLOB_OFF[name]:_BLOB_OFF[name] + w]
    blob16 = cp.tile([128, BLOB16_W], BF16, tag="blob16")
    for name, p, w in _BLOB16_LAYOUT:
        sb[name] = blob16[0:p, _BLOB16_OFF[name]:_BLOB16_OFF[name] + w]
    blob16l = cp.tile([128, BLOB16L_W], BF16, tag="blob16l")
    for name, p, w in _BLOB16L_LAYOUT:
        sb[name] = blob16l[0:p, _BLOB16L_OFF[name]:_BLOB16L_OFF[name] + w]
    xsq96 = wide.tile([96, 128], F32, tag="w96b")
    nc.scalar.activation(out=xsq96, in_=x96, func=AF.Square)
    nxp32 = pp_med.tile([32, 128], F32, tag="med")
    nc.tensor.matmul(nxp32, sb["sel96"], xsq96)
    nxs = wide.tile([32, 128], F32, tag="w96c")
    nc.scalar.activation(out=nxs, in_=nxp32, func=AF.Copy)

    B4 = wide.tile([4, 4096], F32, tag="wC")
    A4 = wide.tile([4, 4096], F32, tag="wB")
    nc.sync.dma_start(out=B4[0:3, :], in_=I["xt3"])
    nc.sync.dma_start(out=B4[3:4, :], in_=nxs)
    nc.sync.dma_start(out=A4[0:3, :], in_=I["xt3"])
    nc.sync.dma_start(out=A4[3:4, :],
                      in_=bass.AP(tensor=I["blob"].tensor,
                                  offset=_BLOB_OFF["ones_row"],
                                  ap=[[0, 1], [0, 4], [1, 1024]]))

    # bf16 weights, input x, then late-stage weights
    nc.sync.dma_start(out=blob16, in_=I["blob16"])
    x = wide.tile([64, 4096], BF16, tag="wD")
    for t in range(8):
        nc.sync.dma_start(out=x[:, t * 512:(t + 1) * 512],
                          in_=I["xt16"][:, t * 512:(t + 1) * 512])
    blobl = cp.tile([128, BLOBL_W], F32, tag="blobl")
    nc.sync.dma_start(out=blobl, in_=I["blobl"])
    nc.sync.dma_start(out=blob16l, in_=I["blob16l"])
    for name, p, w in _BLOBL_LAYOUT:
        sb[name] = blobl[0:p, _BLOBL_OFF[name]:_BLOBL_OFF[name] + w]

    # all pairwise-distance tiles upfront (prologue is DMA-bound, engines
    # idle): qt_all[:, pi*64:...] = q for pair pi; frees the psum med ring
    # and the ACT stream from per-pair distance work
    QTALL = int(os.environ.get("K_QTALL", "1"))
    qt_all = None
    if QTALL:
        qt_all = wide.tile([128, NPAIR * 64], F32, tag="wQT")
        for pi in range(NPAIR):
            _cs1 = slice((2 * pi) * 64, (2 * pi + 1) * 64)
            _cs2 = slice((2 * pi + 1) * 64, (2 * pi + 2) * 64)
            _pdp = pp_med.tile([128, 64], F32, tag="med")
            nc.tensor.matmul(_pdp[0:64, :], A4[:, _cs1], B4[:, _cs1])
            nc.tensor.matmul(_pdp[64:128, :], A4[:, _cs2], B4[:, _cs2])
            nc.scalar.activation(out=qt_all[:, pi * 64:(pi + 1) * 64],
                                 in_=_pdp, func=AF.Copy)

    # gated activations accumulated across all groups (for batched convs)
    x1all = wide.tile([64, 4096], BF16, tag="wE")
    x2all = wide.tile([128, 4096], BF16, tag="wF")
    # final per-group features (512ch as 4 blocks x 64 groups)
    xfin = cp.tile([128, 4, G], F32, tag="xfin")

    BDM = int(os.environ.get("K_BDM", "1"))
    bd_ring = []
    bdv_ring = []
    bdd_ring = []
    for ri in range(int(os.environ.get("K_BD", "6"))):
        if BDM:
            bddt = cp.tile([128, 256], BF16, tag=f"bddring{ri}")
            nc.gpsimd.memset(bddt, 0.0)
            bdd_ring.append(bddt)
            bd_ring.append(bddt[:, 0:128])
            bdv_ring.append(bddt[:, 128:256])
        else:
            bdt = cp.tile([128, 128], BF16, tag=f"bdring{ri}")
            nc.gpsimd.memset(bdt, 0.0)
            bd_ring.append(bdt)
            bdvt = cp.tile([128, 128], BF16, tag=f"bdvring{ri}")
            nc.gpsimd.memset(bdvt, 0.0)
            bdv_ring.append(bdvt)

    ADDP = int(os.environ.get("K_ADDP", "0"))
    E2_MODE = os.environ.get("K_E2M", "a")  # a | b | c | bc
    PCTAIL = int(os.environ.get("K_PCTAIL", "0"))
    SPOOL = int(os.environ.get("K_SPOOL", "1"))
    RELUP = int(os.environ.get("K_RELUP", "0"))
    RELUX2 = int(os.environ.get("K_RELUX2", "0"))

    def _relu_sb(out, in_, s_col, b_col, pool=None):
        # relu(s*in + b) from sbuf: 2 pool ops, or 1 act op
        if RELUP if pool is None else pool:
            tmp = wk.tile(list(in_.shape), F32, tag="rtmp")
            nc.gpsimd.tensor_scalar(out=tmp, in0=in_, scalar1=s_col,
                                    scalar2=b_col, op0=ALU.mult, op1=ALU.add)
            nc.gpsimd.tensor_scalar(out=out, in0=tmp, scalar1=0.0,
                                    scalar2=None, op0=ALU.max)
        else:
            nc.scalar.activation(out=out, in_=in_, func=AF.Relu,
                                 bias=b_col, scale=s_col)
    F1 = int(os.environ.get("K_F1", "1"))
    F2 = int(os.environ.get("K_F2", "1"))
    QTP = int(os.environ.get("K_QTP", "0"))

    def _tadd(out, a, b):
        if ADDP:
            nc.gpsimd.tensor_tensor(out, a, b, op=ALU.add)
        else:
            nc.vector.tensor_add(out, a, b)

    # windowed-max placement: offload part of the (128, 32n, 16k) max blocks
    # from DVE (TensorReduce) to the mostly-idle Pool engine (pairwise-max
    # tree, in place on the psum tile)
    E1_POOL = int(os.environ.get("K_E1P", "0"))
    E2_POOL_ROUNDS = tuple(
        int(v) for v in os.environ.get("K_E2P", "0,0").split(","))
    PC_POOL = int(os.environ.get("K_PCP", "0"))

    def _wmax(gp, out, pool_rounds, k=KNN):
        """max over k of gp (128, 512) viewed as (p, n, k) -> out (128, 512/k).

        pool_rounds pairwise-max rounds run on the Pool engine (in place on
        the psum tile); the remaining window is reduced on DVE."""
        g3 = gp.rearrange("p (n k) -> p n k", k=k)
        w = k
        for _ in range(pool_rounds):
            h = w // 2
            nc.gpsimd.tensor_tensor(g3[:, :, 0:h], g3[:, :, 0:h],
                                    g3[:, :, h:w], op=ALU.max)
            w = h
        nc.vector.reduce_max(out=out, in_=g3[:, :, 0:w],
                             axis=mybir.AxisListType.X)

    # ---------------- per-pair loops (chunked for DMA latency hiding) ----
    CHUNK = int(os.environ.get("K_CHUNK", "8"))
    NIXB = int(os.environ.get("K_NIXB", "8"))
    ixb_ring = [None] * NIXB
    ixr_all = ring.tile([2 * CHUNK, 1024], U16, tag="ixr_all")

    def phase_a(pi):
        g1, g2 = 2 * pi, 2 * pi + 1
        cs1 = slice(g1 * 64, (g1 + 1) * 64)
        cs2 = slice(g2 * 64, (g2 + 1) * 64)

        # q = x_m.x_n - xx_n/2 for both groups -> (128, 64)
        # (row-constant -xx_m/2 term dropped: doesn't change row top-k)
        if QTALL:
            qt = qt_all[:, pi * 64:(pi + 1) * 64]
        else:
            pdp = pp_med.tile([128, 64], F32, tag="med")
            for h, cs in ((0, cs1), (1, cs2)):
                nc.tensor.matmul(pdp[h * 64:(h + 1) * 64, :],
                                 A4[:, cs], B4[:, cs])
            qt = wk.tile([128, 64], F32, tag="qt")
            if QTP:
                nc.gpsimd.tensor_copy(qt, pdp)
            else:
                nc.scalar.activation(out=qt, in_=pdp, func=AF.Copy)

        # top-16 indices per point row
        mx = wk.tile([128, 16], F32, tag="mx")
        ix = wk.tile([128, 16], U16, tag="ix")
        qt2 = wk.tile([128, 64], F32, tag="qt2")
        nc.vector.max(out=mx[:, 0:8], in_=qt)
        nc.vector.max_index(out=ix[:, 0:8], in_max=mx[:, 0:8], in_values=qt)
        nc.vector.match_replace(out=qt2, in_to_replace=mx[:, 0:8],
                                in_values=qt, imm_value=NEG)
        nc.vector.max(out=mx[:, 8:16], in_=qt2)
        nc.vector.max_index(out=ix[:, 8:16], in_max=mx[:, 8:16], in_values=qt2)

        # idx row form (2, 1024) u16, then broadcast to all 128 partitions so
        # the one-hot compare can run at 4x (2-byte sbuf in/out)
        s2 = 2 * (pi % CHUNK)
        ixrows = ixr_all[s2:s2 + 2, :]
        nc.sync.dma_start(out=ixrows, in_=ix)
        ixb = ring.tile([128, 1024], U16, tag=f"ixb{pi % NIXB}")
        ixb_ring[pi % NIXB] = ixb
        nc.sync.dma_start(
            out=ixb, in_=ixrows.unsqueeze(1).broadcast_to((2, 64, 1024)))

    S01_ring = [None] * CHUNK

    def phase_b1(pi):
        """S-matrix + e1 edge conv for pair pi."""
        g1, g2 = 2 * pi, 2 * pi + 1
        cs1 = slice(g1 * 64, (g1 + 1) * 64)
        cs2 = slice(g2 * 64, (g2 + 1) * 64)
        ixb = ixb_ring[pi % NIXB]
        S01 = wk2.tile([128, 1024], BF16, tag="S01")
        S01_ring[pi % CHUNK] = S01
        # S01 = (ix_bcast == iota_p): 4x dve op (all operands 2-byte sbuf)
        seng = nc.gpsimd if (SPOOL == 1 or (SPOOL == 2 and pi % 2)) \
            else nc.vector
        seng.tensor_scalar(out=S01, in0=ixb,
                           scalar1=sb["iota_col"], scalar2=None,
                           op0=ALU.is_equal)

        bd = bd_ring[pi % len(bd_ring)]
        if F1:
            # u and v-fold matmuls into one psum tile; merged copies
            uvv = pp_med.tile([128, 256], F32, tag="med")
            nc.tensor.matmul(uvv[0:64, 0:64], x[:, cs1], sb["b16_wu1"])
            nc.tensor.matmul(uvv[64:128, 64:128], x[:, cs2], sb["b16_wu1"])
            nc.tensor.matmul(uvv[0:64, 128:192], x[:, cs1], sb["b16_wv1"])
            nc.tensor.matmul(uvv[64:128, 192:256], x[:, cs2], sb["b16_wv1"])
            bdv = bdv_ring[pi % len(bdv_ring)]
            if BDM:
                # one copy per partition half: {u block, v block} as a
                # strided access pattern on both sides.
                # col = a*128 + b*64 + c: a selects u/v, b selects group
                bdd = bdd_ring[pi % len(bdd_ring)]
                sv = uvv.rearrange("p (a b c) -> p a b c", a=2, b=2)
                dv = bdd.rearrange("p (a b c) -> p a b c", a=2, b=2)
                nc.scalar.activation(out=dv[0:64, :, 0:1, :],
                                     in_=sv[0:64, :, 0:1, :], func=AF.Copy)
                nc.scalar.activation(out=dv[64:128, :, 1:2, :],
                                     in_=sv[64:128, :, 1:2, :], func=AF.Copy)
            else:
                nc.scalar.activation(
                    out=bd[0:64, 0:64], in_=uvv[0:64, 0:64], func=AF.Copy)
                nc.scalar.activation(
                    out=bd[64:128, 64:128], in_=uvv[64:128, 64:128],
                    func=AF.Copy)
                nc.scalar.activation(
                    out=bdv[0:64, 0:64], in_=uvv[0:64, 128:192], func=AF.Copy)
                nc.scalar.activation(
                    out=bdv[64:128, 64:128], in_=uvv[64:128, 192:256],
                    func=AF.Copy)
        else:
            uv1 = pp_med.tile([128, 128], F32, tag="med")
            nc.tensor.matmul(uv1[0:64, 0:64], x[:, cs1], sb["b16_wu1"])
            nc.tensor.matmul(uv1[64:128, 64:128], x[:, cs2], sb["b16_wu1"])
            nc.scalar.activation(out=bd[0:64, 0:64], in_=uv1[0:64, 0:64],
                                 func=AF.Copy)
            nc.scalar.activation(out=bd[64:128, 64:128],
                                 in_=uv1[64:128, 64:128], func=AF.Copy)
        m1 = wk.tile([128, 64], F32, tag="m1")
        if MERGE:
            g1p = pp_big2.tile([128, 1024], F32, tag="big2")
            for half in range(2):
                csl = slice(half * 512, (half + 1) * 512)
                gh = g1p[:, csl]
                if F1:
                    nc.tensor.matmul(gh, bd, S01[:, csl], start=True,
                                     stop=False)
                    nc.tensor.matmul(gh, bdv, sb["b16_R2"][:, csl],
                                     start=False, stop=True)
                else:
                    nc.tensor.matmul(gh, bd, S01[:, csl])
            nc.vector.reduce_max(
                out=m1, in_=g1p.rearrange("p (n k) -> p n k", k=KNN),
                axis=mybir.AxisListType.X)
        else:
            for half in range(2):
                csl = slice(half * 512, (half + 1) * 512)
                g1p = pp_big.tile([128, 512], F32, tag="big")
                if F1:
                    nc.tensor.matmul(g1p, bd, S01[:, csl], start=True,
                                     stop=False)
                    nc.tensor.matmul(g1p, bdv, sb["b16_R2"][:, csl],
                                     start=False, stop=True)
                else:
                    nc.tensor.matmul(g1p, bd, S01[:, csl])
                _wmax(g1p, m1[:, half * 32:(half + 1) * 32], E1_POOL)
        if F1:
            _relu_sb(x1all[:, cs1], m1[0:64, :], sb["s1"][0:64],
                     sb["b1"][0:64])
            _relu_sb(x1all[:, cs2], m1[64:128, :], sb["s1"][64:128],
                     sb["b1"][64:128])
        else:
            v1 = pp_med.tile([128, 64], F32, tag="med")
            nc.tensor.matmul(v1[0:64, :], sb["b16_wv1"], x[:, cs1])
            nc.tensor.matmul(v1[64:128, :], sb["b16_wv1"], x[:, cs2])
            t1a = wk.tile([64, 64], F32, tag="t1a")
            _tadd(t1a, m1[0:64, :], v1[0:64, :])
            t1b = wk.tile([128, 64], F32, tag="t1b")
            _tadd(t1b[64:128, :], m1[64:128, :], v1[64:128, :])
            nc.scalar.activation(out=x1all[:, cs1], in_=t1a, func=AF.Relu,
                                 bias=sb["b1"][0:64], scale=sb["s1"][0:64])
            nc.scalar.activation(out=x1all[:, cs2], in_=t1b[64:128, :],
                                 func=AF.Relu,
                                 bias=sb["b1"][64:128], scale=sb["s1"][64:128])

    def phase_b2(pi):
        """e2 edge conv for pair pi (consumes x1all + S01)."""
        g1, g2 = 2 * pi, 2 * pi + 1
        cs1 = slice(g1 * 64, (g1 + 1) * 64)
        cs2 = slice(g2 * 64, (g2 + 1) * 64)
        S01 = S01_ring[pi % CHUNK]
        for h, cs in ((0, cs1), (1, cs2)):
            xg = x1all[:, cs]
            psl = slice(h * 64, (h + 1) * 64)
            if E2_MODE == "a" and F2:
                uvp = pp_med.tile([64, 256], F32, tag="med")
                nc.tensor.matmul(uvp[:, 0:128], xg, sb["b16_wu2"])
                nc.tensor.matmul(uvp[:, 128:256], xg, sb["b16_wv2"])
                uvs = wk.tile([128, 256], BF16, tag="uvs")
                nc.scalar.activation(out=uvs[psl, :], in_=uvp, func=AF.Copy)
                uT2 = uvs[:, 0:128]
                vv2 = uvs[:, 128:256]
            else:
                uT2 = wk.tile([128, 128], BF16, tag="uT2")
                uT2p = pp_med.tile([64, 128], F32, tag="med")
                nc.tensor.matmul(uT2p, xg, sb["b16_wu2"])
                nc.scalar.activation(out=uT2[psl, :], in_=uT2p, func=AF.Copy)
            if E2_MODE == "a":
                m2 = wk.tile([128, 64], F32, tag="m2")
                if MERGE:
                    g2p = pp_big2.tile([128, 1024], F32, tag="big2")
                    for half in range(2):
                        csl = slice(half * 512, (half + 1) * 512)
                        gh = g2p[:, csl]
                        if F2:
                            nc.tensor.matmul(gh, uT2[psl, :], S01[psl, csl],
                                             start=True, stop=False)
                            nc.tensor.matmul(gh, vv2[psl, :],
                                             sb["b16_R2"][psl, csl],
                                             start=False, stop=True)
                        else:
                            nc.tensor.matmul(gh, uT2[psl, :], S01[psl, csl])
                    nc.vector.reduce_max(
                        out=m2, in_=g2p.rearrange("p (n k) -> p n k", k=KNN),
                        axis=mybir.AxisListType.X)
                else:
                    for half in range(2):
                        csl = slice(half * 512, (half + 1) * 512)
                        g2p = pp_big.tile([128, 512], F32, tag="big")
                        if F2:
                            nc.tensor.matmul(g2p, uT2[psl, :], S01[psl, csl],
                                             start=True, stop=False)
                            nc.tensor.matmul(g2p, vv2[psl, :],
                                             sb["b16_R2"][psl, csl],
                                             start=False, stop=True)
                        else:
                            nc.tensor.matmul(g2p, uT2[psl, :], S01[psl, csl])
                        _wmax(g2p, m2[:, half * 32:(half + 1) * 32],
                              E2_POOL_ROUNDS[half])
                if F2:
                    _relu_sb(x2all[:, cs], m2, sb["s2"], sb["b2"],
                             pool=RELUX2)
                else:
                    v2 = pp_med.tile([128, 64], F32, tag="med")
                    nc.tensor.matmul(v2, sb["b16_wv2"], xg)
                    t2 = wk.tile([128, 64], F32, tag="t2")
                    _tadd(t2, m2, v2)
                    nc.scalar.activation(out=x2all[:, cs], in_=t2,
                                         func=AF.Relu,
                                         bias=sb["b2"], scale=sb["s2"])
                continue
            # v-folded path: g2p = uT2.S + vT2.R, then relu(bn) at psum
            # egress (valid pre-max: bn scale > 0), max-tree on sbuf bf16
            vv2p = pp_med.tile([64, 128], F32, tag="med")
            nc.tensor.matmul(vv2p, xg, sb["b16_wv2"])
            vv2 = wk.tile([64, 128], BF16, tag="vv2")
            nc.scalar.activation(out=vv2, in_=vv2p, func=AF.Copy)
            for half in range(2):
                csl = slice(half * 512, (half + 1) * 512)
                g2p = pp_big.tile([128, 512], F32, tag="big")
                nc.tensor.matmul(g2p, uT2[psl, :], S01[psl, csl],
                                 start=True, stop=False)
                nc.tensor.matmul(g2p, vv2, sb["b16_R2"][0:64, csl],
                                 start=False, stop=True)
                x2pre = wk.tile([128, 512], BF16, tag="x2pre")
                nc.scalar.activation(out=x2pre, in_=g2p, func=AF.Relu,
                                     bias=sb["b2"], scale=sb["s2"])
                p3 = x2pre.rearrange("p (n k) -> p n k", k=KNN)
                eng = nc.gpsimd if E2_MODE == "c" or (
                    E2_MODE == "bc" and half == 1) else nc.vector
                eng.tensor_tensor(p3[:, :, 0:8], p3[:, :, 0:8],
                                  p3[:, :, 8:16], op=ALU.max)
                eng.tensor_tensor(p3[:, :, 0:4], p3[:, :, 0:4],
                                  p3[:, :, 4:8], op=ALU.max)
                eng.tensor_tensor(p3[:, :, 0:2], p3[:, :, 0:2],
                                  p3[:, :, 2:4], op=ALU.max)
                osub = slice(cs.start + half * 32, cs.start + half * 32 + 32)
                eng.tensor_tensor(x2all[:, osub], p3[:, :, 0:1].squeeze(-1),
                                  p3[:, :, 1:2].squeeze(-1), op=ALU.max)


    # ---------------- batched calib/gate/expansion (per 512-col window) --
    c1all = wide.tile([64, 4096], BF16, tag="wG")
    sigA = wide.tile([64, 4096], BF16, tag="wA")
    sigX2 = wide.tile([128, 4096], BF16, tag="wH")
    pcr = ctx.enter_context(
        tc.tile_pool(name="pcring", bufs=int(os.environ.get("K_NPC", "3"))))
    ee0 = wide.tile([128, 4096], BF16, tag="wK")
    ee1 = wide.tile([128, 4096], BF16, tag="wL")
    ee = [ee0, ee1]

    def phase_c(j):
        if PCSPLIT:
            phase_c_part(j, 0)
            phase_c_part(j, 1)
        else:
            phase_c_part(j, None)

    def phase_c_part(j, part):
        if part is None:
            csl = slice(j * 512, (j + 1) * 512)
            fsl = slice(j * 8, (j + 1) * 8)
        else:
            csl = slice(j * 512 + part * 256, j * 512 + part * 256 + 256)
            fsl = slice(j * 8 + part * 4, j * 8 + part * 4 + 4)
        W = csl.stop - csl.start
        c1p = pp_big.tile([64, W], F32, tag="big")
        nc.tensor.matmul(c1p, sb["b16_ca1_a"], x1all[:, csl], start=True,
                         stop=False)
        nc.tensor.matmul(c1p, sb["b16_ca1_b"], x2all[:, csl], start=False,
                         stop=True)
        nc.scalar.activation(out=c1all[:, csl], in_=c1p, func=AF.Relu,
                             bias=sb["ca1_bias"], scale=sb["ca1_s"])
        sp1 = pp_big.tile([128, W], F32, tag="big")
        nc.tensor.matmul(sp1, sb["b16_ca2"][:, 0:128], c1all[:, csl])
        nc.scalar.activation(out=sigA[:, csl], in_=sp1[0:64, :],
                             func=AF.Sigmoid, bias=sb["cb2_blk1"][0:64])
        nc.scalar.activation(out=sigX2[0:64, csl], in_=sp1[64:128, :],
                             func=AF.Sigmoid, bias=sb["cb2_blk1"][64:128])
        sp2 = pp_big.tile([64, W], F32, tag="big")
        nc.tensor.matmul(sp2, sb["b16_ca2"][:, 128:192], c1all[:, csl])
        nc.scalar.activation(out=sigX2[64:128, csl], in_=sp2, func=AF.Sigmoid,
                             bias=sb["cb2_blk2"])
        p1t = pcr.tile([64, W], BF16, tag="p1r")
        p2t = pcr.tile([128, W], BF16, tag="p2r")
        nc.gpsimd.tensor_mul(p1t, x1all[:, csl], sigA[:, csl])
        nc.gpsimd.tensor_mul(p2t, x2all[:, csl], sigX2[:, csl])
        for b in range(2):
            ep = pp_big.tile([128, W], F32, tag="big")
            osl = slice(b * 128, (b + 1) * 128)
            nc.tensor.matmul(ep, sb["b16_x1a"][:, osl], p1t,
                             start=True, stop=False)
            nc.tensor.matmul(ep, sb["b16_x1b"][:, osl], p2t,
                             start=False, stop=True)
            nc.scalar.activation(out=ee[b][:, csl], in_=ep, func=AF.Relu,
                                 bias=sb["e1bias"][:, b:b + 1],
                                 scale=sb["e1s"][:, b:b + 1])
        tailwin = PCTAIL and j >= 6
        for b in range(4):
            if tailwin and b >= 2:
                xp = pp_big2.tile([128, W], F32, tag="big2")
            else:
                xp = pp_big.tile([128, W], F32, tag="big")
            osl = slice(b * 128, (b + 1) * 128)
            nc.tensor.matmul(xp, sb["b16_x2a"][:, osl], ee[0][:, csl],
                             start=True, stop=False)
            nc.tensor.matmul(xp, sb["b16_x2b"][:, osl], ee[1][:, csl],
                             start=False, stop=True)
            xm = wk2.tile([128, W // 64], F32, tag="xm")
            _wmax(xp, xm, PC_POOL, k=64)
            nc.scalar.activation(out=xfin[:, b, fsl], in_=xm,
                                 func=AF.Relu,
                                 bias=sb["e2bias"][:, b:b + 1],
                                 scale=sb["e2s"][:, b:b + 1])

    STAG = int(os.environ.get("K_STAG", "5"))
    PCSPLIT = int(os.environ.get("K_PCSPLIT", "0"))
    FLAT = int(os.environ.get("K_FLAT", "1"))
    if FLAT:
        # one continuous pipeline over all 32 pairs: phase_a leads b1 by LA,
        # b2 trails b1 by STAG, each phase_c window fires as its 4 pairs
        # complete. No chunk boundaries, so the pipeline never drains.
        # Ring safety: ixb/ixr slots (8) are rewritten 8-LA b1-steps after
        # their reader; S01 slots (WK2) rewritten WK2-STAG steps after.
        LA = int(os.environ.get("K_LA", "4"))
        for pi in range(LA):
            phase_a(pi)

        PCD = int(os.environ.get("K_PCD", "5"))

        def _after_b2(done):
            # window w is emitted at done == 4*w + 3 + PCD
            if done >= PCD + 3 and (done - PCD - 3) % 4 == 0:
                phase_c((done - PCD - 3) // 4)
            if done == NPAIR - 1:
                for w in range((done - PCD - 3) // 4 + 1, NPAIR // 4):
                    phase_c(w)
        for pi in range(NPAIR):
            phase_b1(pi)
            if pi + LA < NPAIR:
                phase_a(pi + LA)
            if pi >= STAG:
                phase_b2(pi - STAG)
                _after_b2(pi - STAG)
        for pi in range(NPAIR - STAG, NPAIR):
            phase_b2(pi)
            _after_b2(pi)
    else:
        nwin = CHUNK // 4
        pending_c = []
        for chunk in range(NPAIR // CHUNK):
            base = chunk * CHUNK
            for pi in range(base, base + CHUNK):
                phase_a(pi)
            for w in pending_c:
                phase_c(w)
            pending_c = []
            for i in range(CHUNK):
                phase_b1(base + i)
                if i >= STAG:
                    phase_b2(base + i - STAG)
            for i in range(CHUNK - STAG, CHUNK):
                phase_b2(base + i)
            pending_c = list(range(nwin * chunk, nwin * chunk + nwin))
        for w in pending_c:
            phase_c(w)

    # ---------------- final stage (256ch x 64 group-cols) ---------------
    tt = wk.tile([128, 2, G], F32, tag="tt")
    FSPLIT = int(os.environ.get("K_FSPLIT", "0"))
    for b in range(2):
        osl = slice(b * 128, (b + 1) * 128)
        rp = pp_med.tile([128, G], F32, tag="med")
        rngs = (slice(0, 48), slice(48, 64)) if FSPLIT else (slice(0, G),)
        for rng in rngs:
            for cb in range(4):
                nc.tensor.matmul(rp[:, rng], sb[f"rd{cb}"][:, osl],
                                 xfin[:, cb, rng],
                                 start=(cb == 0), stop=(cb == 3))
        rr = wk.tile([128, G], F32, tag="rr")
        nc.scalar.activation(out=rr, in_=rp, func=AF.Relu,
                             bias=sb["rdb"][:, b:b + 1],
                             scale=sb["rds"][:, b:b + 1])
        nc.vector.tensor_scalar(out=tt[:, b, :], in0=rr,
                                scalar1=sb["n1s"][:, b:b + 1],
                                scalar2=sb["n1b"][:, b:b + 1],
                                op0=ALU.mult, op1=ALU.add)
    hh = wk.tile([128, 2, G], F32, tag="hh")
    for b in range(2):
        osl = slice(b * 128, (b + 1) * 128)
        hp = pp_med.tile([128, G], F32, tag="med")
        for cb in range(2):
            nc.tensor.matmul(hp, sb[f"sc1_{cb}"][:, osl], tt[:, cb, :],
                             start=(cb == 0), stop=(cb == 1))
        nc.scalar.activation(out=hh[:, b, :], in_=hp, func=AF.Relu,
                             bias=sb["sc1b"][:, b:b + 1])
    for b in range(2):
        osl = slice(b * 128, (b + 1) * 128)
        h2p = pp_med.tile([128, G], F32, tag="med")
        for cb in range(2):
            nc.tensor.matmul(h2p, sb[f"sc2_{cb}"][:, osl], hh[:, cb, :],
                             start=(cb == 0), stop=(cb == 1))
        s2sum = wk.tile([128, G], F32, tag="s2sum")
        nc.vector.tensor_scalar(out=s2sum, in0=h2p,
                                scalar1=sb["sc2b"][:, b:b + 1], scalar2=None,
                                op0=ALU.add)
        s2t = wk.tile([128, G], F32, tag="s2t")
        nc.vector.tensor_add(s2t, s2sum, tt[:, b, :])
        osb = wk.tile([128, G], F32, tag="osb")
        nc.vector.tensor_scalar(out=osb, in0=s2t,
                                scalar1=sb["n2s"][:, b:b + 1],
                                scalar2=sb["n2b"][:, b:b + 1],
                                op0=ALU.mult, op1=ALU.add)
        nc.sync.dma_start(out=out_ap[b * 128:(b + 1) * 128, :], in_=osb)


@functools.lru_cache(maxsize=1)
def _build():
    nc = bacc.Bacc("TRN2", target_bir_lowering=False, debug=False,
                   num_devices=NCORES)
    I = {}
    I["xt16"] = nc.dram_tensor("xt16", (64, 4096), BF16,
                               kind="ExternalInput").ap()
    I["xt3"] = nc.dram_tensor("xt3", (3, 4096), F32,
                              kind="ExternalInput").ap()
    I["blob"] = nc.dram_tensor("blob", (128, BLOB_W), F32,
                               kind="ExternalInput").ap()
    I["blobl"] = nc.dram_tensor("blobl", (128, BLOBL_W), F32,
                                kind="ExternalInput").ap()
    I["blob16"] = nc.dram_tensor("blob16", (128, BLOB16_W), BF16,
                                 kind="ExternalInput").ap()
    I["blob16l"] = nc.dram_tensor("blob16l", (128, BLOB16L_W), BF16,
                                  kind="ExternalInput").ap()
    out_ap = nc.dram_tensor("out", (256, G), F32, kind="ExternalOutput").ap()
    from contextlib import ExitStack
    with tile.TileContext(nc) as tc, ExitStack() as ctx:
        _emit(tc, I, out_ap, ctx)
    nc.compile()
    return nc


def kernel(**inputs):
    nc = _build()
    consts = _np_consts(inputs)
    blob = _pack_blob(consts)
    blobl = _pack_blob_late(consts)
    blob16v, blob16lv = _pack_blob16(consts)

    xyz = inputs["xyz"].astype(np.float32)      # (2, 256, 64, 3)
    feats = inputs["feats"].astype(np.float32)  # (2, 256, 64, 61)
    xf_full = np.concatenate([xyz, feats], axis=-1).reshape(512 * 64, 64)

    in_maps = []
    for c in range(NCORES):
        import ml_dtypes
        sh = xf_full[c * 4096:(c + 1) * 4096, :]
        in_maps.append({
            "blob": blob,
            "blobl": blobl,
            "blob16": blob16v,
            "blob16l": blob16lv,
            "xt16": np.ascontiguousarray(sh.T.astype(ml_dtypes.bfloat16)),
            "xt3": np.ascontiguousarray(sh.T[0:3, :]),
        })

    trace = bool(int(os.environ.get("KERNEL_TRACE", "0")))
    try:
        res = bass_utils.run_bass_kernel_spmd(
            nc, in_maps, core_ids=list(range(NCORES)), trace=trace)
    except ModuleNotFoundError:
        res = bass_utils.run_bass_kernel_spmd(
            nc, in_maps, core_ids=list(range(NCORES)))
    if trace and res.exec_time_ns is not None:
        print(f"HW exec time: {res.exec_time_ns} ns")
        if res.instructions_and_trace is not None:
            print(f"trace: {res.instructions_and_trace[1]}")
        kernel.last_results = res

    out = np.empty((2, 256, 256), dtype=np.float32)
    for c in range(NCORES):
        o = res.results[c]["out"]              # (256, 64)
        b, mlo = divmod(c * G, 256)
        out[b, :, mlo:mlo + G] = o
    return out


if __name__ == "__main__":
    print("building bass graph...")
    nc = _build()
    print("graph built ok")



# revision 74
# speedup vs baseline: 1.0285x; 1.0044x over previous
"""Trainium2 Bass kernel for nn_AttnGNNLayer (EdgeConv-style GNN layer).

Data-parallel over the B*M=512 group axis: 64 groups per core on 8 cores.

Per-group pipeline (K=64 points, knn=16):
  - distance proxy q = x^T x - xx/2 (one ones-row accum matmul; xx from a
    tall (96,128) Square + one f32 matmul against a packed selector)
  - top-16 neighbor indices via DVE max8 / match_replace / max_index
  - one-hot gather matrix S[j, n*16+k]: idx rows DMA'd to (2,1024) u16,
    broadcast-DMA'd to all 128 partitions, then ONE tensor_scalar is_equal
    vs a per-partition iota (4x-mode eligible; runs on the Pool engine,
    which may only touch SBUF - GPSIMD cannot access PSUM on silicon)
  - edge conv: gather matmul accumulates bd@S + bdv@R (R = kron(I,1_16)),
    folding the center term v into psum so the windowed reduce_max (DVE,
    merged (128,1024) 2-bank tiles) directly yields max_k(u[idx]+v); the
    bn+relu then applies at psum egress
  - all 1x1 convs batched over all 64*64=4096 points per core on PE

Emission is software-pipelined: per 8-pair chunk, phase_a (knn) x8, then
b1 (S + e1) / b2 (e2) interleaved with a stagger of 4 so every engine's
in-order stream has other pairs' work between dependent ops; phase_c
(calib/gate/expansion windows) is deferred past the next chunk's phase_a.
Weight constants ship in early/late blobs (f32 + bf16) so the pair loop
does not wait on late-stage conv weights.
"""

import functools
import os
import sys

for _p in ("/opt/trn_rl_repo", "/root/.axon_site/_ro/trn_rl_repo"):
    if os.path.isdir(_p) and _p not in sys.path:
        sys.path.append(_p)

import numpy as np

import concourse.bass as bass
import concourse.mybir as mybir
import concourse.tile as tile
from concourse import bacc, bass_utils

F32 = mybir.dt.float32
BF16 = mybir.dt.bfloat16
U16 = mybir.dt.uint16

B, M, K, KNN = 2, 256, 64, 16
G = 64            # groups per core
NPAIR = G // 2    # 32 pair tiles (2 groups packed in 128 partitions)
NCORES = 8
NEG = -1.0e30
EPS = 1e-5

AF = mybir.ActivationFunctionType
ALU = mybir.AluOpType

# (name, partitions, width) of every constant packed into the blob, in order
_BLOB_LAYOUT = [
    ("iota_col", 128, 1),
    ("neg_iota_col", 128, 1),
    ("ones_row", 1, 1024),
    ("sel96", 96, 32),
    ("s1", 128, 1), ("b1", 128, 1), ("s2", 128, 1), ("b2", 128, 1),
    ("ca1_s", 64, 1), ("ca1_bias", 64, 1),
    ("cb2_blk1", 128, 1), ("cb2_blk2", 64, 1),
    ("e1s", 128, 2), ("e1bias", 128, 2),
    ("e2s", 128, 4), ("e2bias", 128, 4),
]
_BLOB_OFF = {}
_off = 0
for _n, _pp, _w in _BLOB_LAYOUT:
    _BLOB_OFF[_n] = _off
    _off += _w
BLOB_W = _off

# late-stage weights (final 256ch x 64 stage): separate DMA issued after the
# input DMAs so the pair loop can start sooner
_BLOBL_LAYOUT = [
    ("rd0", 128, 256), ("rd1", 128, 256), ("rd2", 128, 256), ("rd3", 128, 256),
    ("rds", 128, 2), ("rdb", 128, 2),
    ("sc1_0", 128, 256), ("sc1_1", 128, 256), ("sc1b", 128, 2),
    ("sc2_0", 128, 256), ("sc2_1", 128, 256), ("sc2b", 128, 2),
    ("n1s", 128, 2), ("n1b", 128, 2), ("n2s", 128, 2), ("n2b", 128, 2),
]
_BLOBL_OFF = {}
_offl = 0
for _n, _pp, _w in _BLOBL_LAYOUT:
    _BLOBL_OFF[_n] = _offl
    _offl += _w
BLOBL_W = _offl

# bf16 constants: pair-loop weights (early) and conv weights (late)
_BLOB16_LAYOUT = [
    ("b16_R2", 128, 1024),
    ("b16_wu1", 64, 64), ("b16_wv1", 64, 64),
    ("b16_wu2", 64, 128), ("b16_wv2", 64, 128),
]
_BLOB16_OFF = {}
_o16 = 0
for _n, _pp, _w in _BLOB16_LAYOUT:
    _BLOB16_OFF[_n] = _o16
    _o16 += _w
BLOB16_W = _o16

_BLOB16L_LAYOUT = [
    ("b16_ca1_a", 64, 64), ("b16_ca1_b", 128, 64), ("b16_ca2", 64, 192),
    ("b16_x1a", 64, 256), ("b16_x1b", 128, 256),
    ("b16_x2a", 128, 512), ("b16_x2b", 128, 512),
]
_BLOB16L_OFF = {}
_o16l = 0
for _n, _pp, _w in _BLOB16L_LAYOUT:
    _BLOB16L_OFF[_n] = _o16l
    _o16l += _w
BLOB16L_W = _o16l


def _np_consts(iw):
    """All constant tensors (iota + host-prepped weights)."""
    f = np.float32
    c = {}
    iota = np.arange(64, dtype=f)
    c["iota_col"] = np.concatenate([iota, iota]).reshape(128, 1)
    c["neg_iota_col"] = -c["iota_col"]
    selg = np.zeros((2, 128), dtype=f)
    selg[0, :64] = 1.0
    selg[1, 64:] = 1.0
    c["selg"] = selg
    c["ones_row"] = np.ones((1, 1024), dtype=f)
    # sel96[c*32+blk, blk] = -0.5: one matmul turns xsq96 (96,128) into
    # -xx/2 for all 4096 points as a (32,128) psum tile
    sel96 = np.zeros((96, 32), dtype=f)
    for _c in range(3):
        for _b in range(32):
            sel96[_c * 32 + _b, _b] = -0.5
    c["sel96"] = sel96
    # replication matrix: R2[p, n*16+k] = (n == p % 64); v-fold accumuland
    _R = np.repeat(np.eye(64, dtype=f), KNN, axis=1)
    c["R2"] = np.vstack([_R, _R])

    e1_w = iw["e1_w"].astype(f)
    W1, W2 = e1_w[:, :64], e1_w[:, 64:]
    c["wu1"] = W1.T.copy()
    c["wv1"] = (W2 - W1).T.copy()
    e2_w = iw["e2_w"].astype(f)
    W21, W22 = e2_w[:, :64], e2_w[:, 64:]
    c["wu2"] = W21.T.copy()
    c["wv2"] = (W22 - W21).T.copy()

    def bn_sb(g, b):
        return (g / np.sqrt(1.0 + EPS)).astype(f), b.astype(f)

    def pair_col(v):
        return np.concatenate([v, v]).reshape(128, 1).astype(f)

    s1, b1 = bn_sb(iw["e1_g"], iw["e1_b"])
    c["s1"], c["b1"] = pair_col(s1), pair_col(b1)
    s2, b2 = bn_sb(iw["e2_g"], iw["e2_b"])
    c["s2"], c["b2"] = s2.reshape(128, 1), b2.reshape(128, 1)

    cal1_w = iw["cal1_w"].astype(f)
    c["ca1_a"] = cal1_w[:, :64].T.copy()
    c["ca1_b"] = cal1_w[:, 64:].T.copy()
    cs, cbv = bn_sb(iw["cal1_g"], iw["cal1_b"])
    c["ca1_s"], c["ca1_bias"] = cs.reshape(64, 1), cbv.reshape(64, 1)

    c["ca2"] = iw["cal2_w"].astype(f).T.copy()
    cb2 = iw["cal2_bias"].astype(f)
    c["cb2_blk1"] = cb2[:128].reshape(128, 1)
    c["cb2_blk2"] = cb2[128:].reshape(64, 1)

    exp1_w = iw["exp1_w"].astype(f)
    c["x1a"] = exp1_w[:, :64].T.copy()
    c["x1b"] = exp1_w[:, 64:].T.copy()
    es, eb = bn_sb(iw["exp1_g"], iw["exp1_b"])
    c["e1s"] = es.reshape(2, 128).T.copy()
    c["e1bias"] = eb.reshape(2, 128).T.copy()

    exp2_w = iw["exp2_w"].astype(f)
    c["x2a"] = exp2_w[:, :128].T.copy()
    c["x2b"] = exp2_w[:, 128:].T.copy()
    es2, eb2 = bn_sb(iw["exp2_g"], iw["exp2_b"])
    c["e2s"] = es2.reshape(4, 128).T.copy()
    c["e2bias"] = eb2.reshape(4, 128).T.copy()

    rdT = iw["red_w"].astype(f).T.reshape(4, 128, 256)
    for i in range(4):
        c[f"rd{i}"] = rdT[i].copy()
    rs, rb = bn_sb(iw["red_g"], iw["red_b"])
    c["rds"] = rs.reshape(2, 128).T.copy()
    c["rdb"] = rb.reshape(2, 128).T.copy()

    sc1T = iw["sc1_w"].astype(f).T.reshape(2, 128, 256)
    c["sc1_0"], c["sc1_1"] = sc1T[0].copy(), sc1T[1].copy()
    c["sc1b"] = iw["sc1_b"].astype(f).reshape(2, 128).T.copy()
    sc2T = iw["sc2_w"].astype(f).T.reshape(2, 128, 256)
    c["sc2_0"], c["sc2_1"] = sc2T[0].copy(), sc2T[1].copy()
    c["sc2b"] = iw["sc2_b"].astype(f).reshape(2, 128).T.copy()

    n1s, n1b = bn_sb(iw["sc_n1_g"], iw["sc_n1_b"])
    c["n1s"] = (2.0 * n1s).reshape(2, 128).T.copy()
    c["n1b"] = n1b.reshape(2, 128).T.copy()
    n2s, n2b = bn_sb(iw["sc_n2_g"], iw["sc_n2_b"])
    c["n2s"] = n2s.reshape(2, 128).T.copy()
    c["n2b"] = n2b.reshape(2, 128).T.copy()
    return c


def _pack_blob(c):
    blob = np.zeros((128, BLOB_W), dtype=np.float32)
    for name, p, w in _BLOB_LAYOUT:
        v = c[name]
        assert v.shape == (p, w), (name, v.shape, (p, w))
        blob[:p, _BLOB_OFF[name]:_BLOB_OFF[name] + w] = v
    return blob


def _pack_blob_late(c):
    blob = np.zeros((128, BLOBL_W), dtype=np.float32)
    for name, p, w in _BLOBL_LAYOUT:
        v = c[name]
        assert v.shape == (p, w), (name, v.shape, (p, w))
        blob[:p, _BLOBL_OFF[name]:_BLOBL_OFF[name] + w] = v
    return blob


def _pack_blob16(c):
    import ml_dtypes
    src16 = {"b16_R2": c["R2"],
             "b16_wu1": c["wu1"], "b16_wv1": c["wv1"],
             "b16_wu2": c["wu2"], "b16_wv2": c["wv2"],
             "b16_ca1_a": c["ca1_a"], "b16_ca1_b": c["ca1_b"],
             "b16_ca2": c["ca2"], "b16_x1a": c["x1a"], "b16_x1b": c["x1b"],
             "b16_x2a": c["x2a"], "b16_x2b": c["x2b"]}
    blob = np.zeros((128, BLOB16_W), dtype=ml_dtypes.bfloat16)
    for name, p, w in _BLOB16_LAYOUT:
        v = src16[name]
        assert v.shape == (p, w), (name, v.shape, (p, w))
        blob[:p, _BLOB16_OFF[name]:_BLOB16_OFF[name] + w] = v.astype(
            ml_dtypes.bfloat16)
    blobl = np.zeros((128, BLOB16L_W), dtype=ml_dtypes.bfloat16)
    for name, p, w in _BLOB16L_LAYOUT:
        v = src16[name]
        assert v.shape == (p, w), (name, v.shape, (p, w))
        blobl[:p, _BLOB16L_OFF[name]:_BLOB16L_OFF[name] + w] = v.astype(
            ml_dtypes.bfloat16)
    return blob, blobl


def _emit(tc, I, out_ap, ctx):
    nc = tc.nc

    cp = ctx.enter_context(tc.tile_pool(name="const", bufs=1))
    wide = ctx.enter_context(tc.tile_pool(name="wide", bufs=1))
    MERGE = int(os.environ.get("K_MERGE", "1"))
    nbig = int(os.environ.get("K_NBIG", "2")) if MERGE else 6
    pp_big = ctx.enter_context(
        tc.tile_pool(name="ps_big", bufs=nbig, space="PSUM"))
    pp_big2 = ctx.enter_context(
        tc.tile_pool(name="ps_big2", bufs=int(os.environ.get("K_NBIG2", "2")),
                     space="PSUM"))
    pp_med = ctx.enter_context(
        tc.tile_pool(name="ps_med", bufs=int(os.environ.get("K_NMED", "2")),
                     space="PSUM"))
    wk = ctx.enter_context(
        tc.tile_pool(name="work", bufs=int(os.environ.get("K_WK", "10"))))
    wk2 = ctx.enter_context(
        tc.tile_pool(name="work2", bufs=int(os.environ.get("K_WK2", "8"))))
    ring = ctx.enter_context(tc.tile_pool(name="ring", bufs=1))

    # ---- DMA order = HWDGE processing order: the knn-critical pieces
    # (x96 for xx, xt3 rows, f32 blob with sel96/iota) go first so the
    # distance/top-k chain starts ~4us earlier; bf16 weights and the input
    # x (first needed by the e1 u/v matmuls) follow; late-stage weights
    # last.
    x96 = wide.tile([96, 128], F32, tag="w96")
    nc.sync.dma_start(out=x96,
                      in_=bass.AP(tensor=I["xt3"].tensor, offset=0,
                                  ap=[[4096, 3], [128, 32], [1, 128]]))
    blob = cp.tile([128, BLOB_W], F32, tag="blob")
    nc.sync.dma_start(out=blob, in_=I["blob"])
    sb = {}
    for name, p, w in _BLOB_LAYOUT:
        sb[name] = blob[0:p, _BLOB_OFF[name]:_BLOB_OFF[name] + w]
    blob16 = cp.tile([128, BLOB16_W], BF16, tag="blob16")
    for name, p, w in _BLOB16_LAYOUT:
        sb[name] = blob16[0:p, _BLOB16_OFF[name]:_BLOB16_OFF[name] + w]
    blob16l = cp.tile([128, BLOB16L_W], BF16, tag="blob16l")
    for name, p, w in _BLOB16L_LAYOUT:
        sb[name] = blob16l[0:p, _BLOB16L_OFF[name]:_BLOB16L_OFF[name] + w]
    xsq96 = wide.tile([96, 128], F32, tag="w96b")
    nc.scalar.activation(out=xsq96, in_=x96, func=AF.Square)
    nxp32 = pp_med.tile([32, 128], F32, tag="med")
    nc.tensor.matmul(nxp32, sb["sel96"], xsq96)
    nxs = wide.tile([32, 128], F32, tag="w96c")
    nc.scalar.activation(out=nxs, in_=nxp32, func=AF.Copy)

    B4 = wide.tile([4, 4096], F32, tag="wC")
    A4 = wide.tile([4, 4096], F32, tag="wB")
    nc.sync.dma_start(out=B4[0:3, :], in_=I["xt3"])
    nc.sync.dma_start(out=B4[3:4, :], in_=nxs)
    nc.sync.dma_start(out=A4[0:3, :], in_=I["xt3"])
    nc.sync.dma_start(out=A4[3:4, :],
                      in_=bass.AP(tensor=I["blob"].tensor,
                                  offset=_BLOB_OFF["ones_row"],
                                  ap=[[0, 1], [0, 4], [1, 1024]]))

    # bf16 weights, input x, then late-stage weights
    nc.sync.dma_start(out=blob16, in_=I["blob16"])
    x = wide.tile([64, 4096], BF16, tag="wD")
    for t in range(8):
        nc.sync.dma_start(out=x[:, t * 512:(t + 1) * 512],
                          in_=I["xt16"][:, t * 512:(t + 1) * 512])
    blobl = cp.tile([128, BLOBL_W], F32, tag="blobl")
    nc.sync.dma_start(out=blobl, in_=I["blobl"])
    nc.sync.dma_start(out=blob16l, in_=I["blob16l"])
    for name, p, w in _BLOBL_LAYOUT:
        sb[name] = blobl[0:p, _BLOBL_OFF[name]:_BLOBL_OFF[name] + w]

    # all pairwise-distance tiles upfront (prologue is DMA-bound, engines
    # idle): qt_all[:, pi*64:...] = q for pair pi; frees the psum med ring
    # and the ACT stream from per-pair distance work
    QTALL = int(os.environ.get("K_QTALL", "1"))
    qt_all = None
    if QTALL:
        qt_all = wide.tile([128, NPAIR * 64], F32, tag="wQT")
        for pi in range(NPAIR):
            _cs1 = slice((2 * pi) * 64, (2 * pi + 1) * 64)
            _cs2 = slice((2 * pi + 1) * 64, (2 * pi + 2) * 64)
            _pdp = pp_med.tile([128, 64], F32, tag="med")
            nc.tensor.matmul(_pdp[0:64, :], A4[:, _cs1], B4[:, _cs1])
            nc.tensor.matmul(_pdp[64:128, :], A4[:, _cs2], B4[:, _cs2])
            nc.scalar.activation(out=qt_all[:, pi * 64:(pi + 1) * 64],
                                 in_=_pdp, func=AF.Copy)

    # gated activations accumulated across all groups (for batched convs)
    x1all = wide.tile([64, 4096], BF16, tag="wE")
    x2all = wide.tile([128, 4096], BF16, tag="wF")
    # final per-group features (512ch as 4 blocks x 64 groups)
    xfin = cp.tile([128, 4, G], F32, tag="xfin")

    BDM = int(os.environ.get("K_BDM", "1"))
    bd_ring = []
    bdv_ring = []
    bdd_ring = []
    for ri in range(int(os.environ.get("K_BD", "6"))):
        if BDM:
            bddt = cp.tile([128, 256], BF16, tag=f"bddring{ri}")
            nc.gpsimd.memset(bddt, 0.0)
            bdd_ring.append(bddt)
            bd_ring.append(bddt[:, 0:128])
            bdv_ring.append(bddt[:, 128:256])
        else:
            bdt = cp.tile([128, 128], BF16, tag=f"bdring{ri}")
            nc.gpsimd.memset(bdt, 0.0)
            bd_ring.append(bdt)
            bdvt = cp.tile([128, 128], BF16, tag=f"bdvring{ri}")
            nc.gpsimd.memset(bdvt, 0.0)
            bdv_ring.append(bdvt)

    ADDP = int(os.environ.get("K_ADDP", "0"))
    E2_MODE = os.environ.get("K_E2M", "a")  # a | b | c | bc
    PCTAIL = int(os.environ.get("K_PCTAIL", "0"))
    SPOOL = int(os.environ.get("K_SPOOL", "1"))
    RELUP = int(os.environ.get("K_RELUP", "0"))
    RELUX2 = int(os.environ.get("K_RELUX2", "0"))

    def _relu_sb(out, in_, s_col, b_col, pool=None):
        # relu(s*in + b) from sbuf: 2 pool ops, or 1 act op
        if RELUP if pool is None else pool:
            tmp = wk.tile(list(in_.shape), F32, tag="rtmp")
            nc.gpsimd.tensor_scalar(out=tmp, in0=in_, scalar1=s_col,
                                    scalar2=b_col, op0=ALU.mult, op1=ALU.add)
            nc.gpsimd.tensor_scalar(out=out, in0=tmp, scalar1=0.0,
                                    scalar2=None, op0=ALU.max)
        else:
            nc.scalar.activation(out=out, in_=in_, func=AF.Relu,
                                 bias=b_col, scale=s_col)
    F1 = int(os.environ.get("K_F1", "1"))
    F2 = int(os.environ.get("K_F2", "1"))
    QTP = int(os.environ.get("K_QTP", "0"))

    def _tadd(out, a, b):
        if ADDP:
            nc.gpsimd.tensor_tensor(out, a, b, op=ALU.add)
        else:
            nc.vector.tensor_add(out, a, b)

    # windowed-max placement: offload part of the (128, 32n, 16k) max blocks
    # from DVE (TensorReduce) to the mostly-idle Pool engine (pairwise-max
    # tree, in place on the psum tile)
    E1_POOL = int(os.environ.get("K_E1P", "0"))
    E2_POOL_ROUNDS = tuple(
        int(v) for v in os.environ.get("K_E2P", "0,0").split(","))
    PC_POOL = int(os.environ.get("K_PCP", "0"))

    def _wmax(gp, out, pool_rounds, k=KNN):
        """max over k of gp (128, 512) viewed as (p, n, k) -> out (128, 512/k).

        pool_rounds pairwise-max rounds run on the Pool engine (in place on
        the psum tile); the remaining window is reduced on DVE."""
        g3 = gp.rearrange("p (n k) -> p n k", k=k)
        w = k
        for _ in range(pool_rounds):
            h = w // 2
            nc.gpsimd.tensor_tensor(g3[:, :, 0:h], g3[:, :, 0:h],
                                    g3[:, :, h:w], op=ALU.max)
            w = h
        nc.vector.reduce_max(out=out, in_=g3[:, :, 0:w],
                             axis=mybir.AxisListType.X)

    # ---------------- per-pair loops (chunked for DMA latency hiding) ----
    CHUNK = int(os.environ.get("K_CHUNK", "8"))
    NIXB = int(os.environ.get("K_NIXB", "8"))
    ixb_ring = [None] * NIXB
    ixr_all = ring.tile([2 * CHUNK, 1024], U16, tag="ixr_all")

    def phase_a(pi):
        g1, g2 = 2 * pi, 2 * pi + 1
        cs1 = slice(g1 * 64, (g1 + 1) * 64)
        cs2 = slice(g2 * 64, (g2 + 1) * 64)

        # q = x_m.x_n - xx_n/2 for both groups -> (128, 64)
        # (row-constant -xx_m/2 term dropped: doesn't change row top-k)
        if QTALL:
            qt = qt_all[:, pi * 64:(pi + 1) * 64]
        else:
            pdp = pp_med.tile([128, 64], F32, tag="med")
            for h, cs in ((0, cs1), (1, cs2)):
                nc.tensor.matmul(pdp[h * 64:(h + 1) * 64, :],
                                 A4[:, cs], B4[:, cs])
            qt = wk.tile([128, 64], F32, tag="qt")
            if QTP:
                nc.gpsimd.tensor_copy(qt, pdp)
            else:
                nc.scalar.activation(out=qt, in_=pdp, func=AF.Copy)

        # top-16 indices per point row
        mx = wk.tile([128, 16], F32, tag="mx")
        ix = wk.tile([128, 16], U16, tag="ix")
        qt2 = wk.tile([128, 64], F32, tag="qt2")
        nc.vector.max(out=mx[:, 0:8], in_=qt)
        nc.vector.max_index(out=ix[:, 0:8], in_max=mx[:, 0:8], in_values=qt)
        nc.vector.match_replace(out=qt2, in_to_replace=mx[:, 0:8],
                                in_values=qt, imm_value=NEG)
        nc.vector.max(out=mx[:, 8:16], in_=qt2)
        nc.vector.max_index(out=ix[:, 8:16], in_max=mx[:, 8:16], in_values=qt2)

        # idx row form (2, 1024) u16, then broadcast to all 128 partitions so
        # the one-hot compare can run at 4x (2-byte sbuf in/out)
        s2 = 2 * (pi % CHUNK)
        ixrows = ixr_all[s2:s2 + 2, :]
        nc.sync.dma_start(out=ixrows, in_=ix)
        ixb = ring.tile([128, 1024], U16, tag=f"ixb{pi % NIXB}")
        ixb_ring[pi % NIXB] = ixb
        nc.sync.dma_start(
            out=ixb, in_=ixrows.unsqueeze(1).broadcast_to((2, 64, 1024)))

    S01_ring = [None] * CHUNK

    def phase_b1(pi):
        """S-matrix + e1 edge conv for pair pi."""
        g1, g2 = 2 * pi, 2 * pi + 1
        cs1 = slice(g1 * 64, (g1 + 1) * 64)
        cs2 = slice(g2 * 64, (g2 + 1) * 64)
        ixb = ixb_ring[pi % NIXB]
        S01 = wk2.tile([128, 1024], BF16, tag="S01")
        S01_ring[pi % CHUNK] = S01
        # S01 = (ix_bcast == iota_p): 4x dve op (all operands 2-byte sbuf)
        seng = nc.gpsimd if (SPOOL == 1 or (SPOOL == 2 and pi % 2)) \
            else nc.vector
        seng.tensor_scalar(out=S01, in0=ixb,
                           scalar1=sb["iota_col"], scalar2=None,
                           op0=ALU.is_equal)

        bd = bd_ring[pi % len(bd_ring)]
        if F1:
            # u and v-fold matmuls into one psum tile; merged copies
            uvv = pp_med.tile([128, 256], F32, tag="med")
            nc.tensor.matmul(uvv[0:64, 0:64], x[:, cs1], sb["b16_wu1"])
            nc.tensor.matmul(uvv[64:128, 64:128], x[:, cs2], sb["b16_wu1"])
            nc.tensor.matmul(uvv[0:64, 128:192], x[:, cs1], sb["b16_wv1"])
            nc.tensor.matmul(uvv[64:128, 192:256], x[:, cs2], sb["b16_wv1"])
            bdv = bdv_ring[pi % len(bdv_ring)]
            if BDM:
                # one copy per partition half: {u block, v block} as a
                # strided access pattern on both sides.
                # col = a*128 + b*64 + c: a selects u/v, b selects group
                bdd = bdd_ring[pi % len(bdd_ring)]
                sv = uvv.rearrange("p (a b c) -> p a b c", a=2, b=2)
                dv = bdd.rearrange("p (a b c) -> p a b c", a=2, b=2)
                nc.scalar.activation(out=dv[0:64, :, 0:1, :],
                                     in_=sv[0:64, :, 0:1, :], func=AF.Copy)
                nc.scalar.activation(out=dv[64:128, :, 1:2, :],
                                     in_=sv[64:128, :, 1:2, :], func=AF.Copy)
            else:
                nc.scalar.activation(
                    out=bd[0:64, 0:64], in_=uvv[0:64, 0:64], func=AF.Copy)
                nc.scalar.activation(
                    out=bd[64:128, 64:128], in_=uvv[64:128, 64:128],
                    func=AF.Copy)
                nc.scalar.activation(
                    out=bdv[0:64, 0:64], in_=uvv[0:64, 128:192], func=AF.Copy)
                nc.scalar.activation(
                    out=bdv[64:128, 64:128], in_=uvv[64:128, 192:256],
                    func=AF.Copy)
        else:
            uv1 = pp_med.tile([128, 128], F32, tag="med")
            nc.tensor.matmul(uv1[0:64, 0:64], x[:, cs1], sb["b16_wu1"])
            nc.tensor.matmul(uv1[64:128, 64:128], x[:, cs2], sb["b16_wu1"])
            nc.scalar.activation(out=bd[0:64, 0:64], in_=uv1[0:64, 0:64],
                                 func=AF.Copy)
            nc.scalar.activation(out=bd[64:128, 64:128],
                                 in_=uv1[64:128, 64:128], func=AF.Copy)
        m1 = wk.tile([128, 64], F32, tag="m1")
        if MERGE:
            g1p = pp_big2.tile([128, 1024], F32, tag="big2")
            for half in range(2):
                csl = slice(half * 512, (half + 1) * 512)
                gh = g1p[:, csl]
                if F1:
                    nc.tensor.matmul(gh, bd, S01[:, csl], start=True,
                                     stop=False)
                    nc.tensor.matmul(gh, bdv, sb["b16_R2"][:, csl],
                                     start=False, stop=True)
                else:
                    nc.tensor.matmul(gh, bd, S01[:, csl])
            nc.vector.reduce_max(
                out=m1, in_=g1p.rearrange("p (n k) -> p n k", k=KNN),
                axis=mybir.AxisListType.X)
        else:
            for half in range(2):
                csl = slice(half * 512, (half + 1) * 512)
                g1p = pp_big.tile([128, 512], F32, tag="big")
                if F1:
                    nc.tensor.matmul(g1p, bd, S01[:, csl], start=True,
                                     stop=False)
                    nc.tensor.matmul(g1p, bdv, sb["b16_R2"][:, csl],
                                     start=False, stop=True)
                else:
                    nc.tensor.matmul(g1p, bd, S01[:, csl])
                _wmax(g1p, m1[:, half * 32:(half + 1) * 32], E1_POOL)
        if F1:
            _relu_sb(x1all[:, cs1], m1[0:64, :], sb["s1"][0:64],
                     sb["b1"][0:64])
            _relu_sb(x1all[:, cs2], m1[64:128, :], sb["s1"][64:128],
                     sb["b1"][64:128])
        else:
            v1 = pp_med.tile([128, 64], F32, tag="med")
            nc.tensor.matmul(v1[0:64, :], sb["b16_wv1"], x[:, cs1])
            nc.tensor.matmul(v1[64:128, :], sb["b16_wv1"], x[:, cs2])
            t1a = wk.tile([64, 64], F32, tag="t1a")
            _tadd(t1a, m1[0:64, :], v1[0:64, :])
            t1b = wk.tile([128, 64], F32, tag="t1b")
            _tadd(t1b[64:128, :], m1[64:128, :], v1[64:128, :])
            nc.scalar.activation(out=x1all[:, cs1], in_=t1a, func=AF.Relu,
                                 bias=sb["b1"][0:64], scale=sb["s1"][0:64])
            nc.scalar.activation(out=x1all[:, cs2], in_=t1b[64:128, :],
                                 func=AF.Relu,
                                 bias=sb["b1"][64:128], scale=sb["s1"][64:128])

    def phase_b2(pi):
        """e2 edge conv for pair pi (consumes x1all + S01)."""
        g1, g2 = 2 * pi, 2 * pi + 1
        cs1 = slice(g1 * 64, (g1 + 1) * 64)
        cs2 = slice(g2 * 64, (g2 + 1) * 64)
        S01 = S01_ring[pi % CHUNK]
        for h, cs in ((0, cs1), (1, cs2)):
            xg = x1all[:, cs]
            psl = slice(h * 64, (h + 1) * 64)
            if E2_MODE == "a" and F2:
                uvp = pp_med.tile([64, 256], F32, tag="med")
                nc.tensor.matmul(uvp[:, 0:128], xg, sb["b16_wu2"])
                nc.tensor.matmul(uvp[:, 128:256], xg, sb["b16_wv2"])
                uvs = wk.tile([128, 256], BF16, tag="uvs")
                nc.scalar.activation(out=uvs[psl, :], in_=uvp, func=AF.Copy)
                uT2 = uvs[:, 0:128]
                vv2 = uvs[:, 128:256]
            else:
                uT2 = wk.tile([128, 128], BF16, tag="uT2")
                uT2p = pp_med.tile([64, 128], F32, tag="med")
                nc.tensor.matmul(uT2p, xg, sb["b16_wu2"])
                nc.scalar.activation(out=uT2[psl, :], in_=uT2p, func=AF.Copy)
            if E2_MODE == "a":
                m2 = wk.tile([128, 64], F32, tag="m2")
                if MERGE:
                    g2p = pp_big2.tile([128, 1024], F32, tag="big2")
                    for half in range(2):
                        csl = slice(half * 512, (half + 1) * 512)
                        gh = g2p[:, csl]
                        if F2:
                            nc.tensor.matmul(gh, uT2[psl, :], S01[psl, csl],
                                             start=True, stop=False)
                            nc.tensor.matmul(gh, vv2[psl, :],
                                             sb["b16_R2"][psl, csl],
                                             start=False, stop=True)
                        else:
                            nc.tensor.matmul(gh, uT2[psl, :], S01[psl, csl])
                    nc.vector.reduce_max(
                        out=m2, in_=g2p.rearrange("p (n k) -> p n k", k=KNN),
                        axis=mybir.AxisListType.X)
                else:
                    for half in range(2):
                        csl = slice(half * 512, (half + 1) * 512)
                        g2p = pp_big.tile([128, 512], F32, tag="big")
                        if F2:
                            nc.tensor.matmul(g2p, uT2[psl, :], S01[psl, csl],
                                             start=True, stop=False)
                            nc.tensor.matmul(g2p, vv2[psl, :],
                                             sb["b16_R2"][psl, csl],
                                             start=False, stop=True)
                        else:
                            nc.tensor.matmul(g2p, uT2[psl, :], S01[psl, csl])
                        _wmax(g2p, m2[:, half * 32:(half + 1) * 32],
                              E2_POOL_ROUNDS[half])
                if F2:
                    _relu_sb(x2all[:, cs], m2, sb["s2"], sb["b2"],
                             pool=RELUX2)
                else:
                    v2 = pp_med.tile([128, 64], F32, tag="med")
                    nc.tensor.matmul(v2, sb["b16_wv2"], xg)
                    t2 = wk.tile([128, 64], F32, tag="t2")
                    _tadd(t2, m2, v2)
                    nc.scalar.activation(out=x2all[:, cs], in_=t2,
                                         func=AF.Relu,
                                         bias=sb["b2"], scale=sb["s2"])
                continue
            # v-folded path: g2p = uT2.S + vT2.R, then relu(bn) at psum
            # egress (valid pre-max: bn scale > 0), max-tree on sbuf bf16
            vv2p = pp_med.tile([64, 128], F32, tag="med")
            nc.tensor.matmul(vv2p, xg, sb["b16_wv2"])
            vv2 = wk.tile([64, 128], BF16, tag="vv2")
            nc.scalar.activation(out=vv2, in_=vv2p, func=AF.Copy)
            for half in range(2):
                csl = slice(half * 512, (half + 1) * 512)
                g2p = pp_big.tile([128, 512], F32, tag="big")
                nc.tensor.matmul(g2p, uT2[psl, :], S01[psl, csl],
                                 start=True, stop=False)
                nc.tensor.matmul(g2p, vv2, sb["b16_R2"][0:64, csl],
                                 start=False, stop=True)
                x2pre = wk.tile([128, 512], BF16, tag="x2pre")
                nc.scalar.activation(out=x2pre, in_=g2p, func=AF.Relu,
                                     bias=sb["b2"], scale=sb["s2"])
                p3 = x2pre.rearrange("p (n k) -> p n k", k=KNN)
                eng = nc.gpsimd if E2_MODE == "c" or (
                    E2_MODE == "bc" and half == 1) else nc.vector
                eng.tensor_tensor(p3[:, :, 0:8], p3[:, :, 0:8],
                                  p3[:, :, 8:16], op=ALU.max)
                eng.tensor_tensor(p3[:, :, 0:4], p3[:, :, 0:4],
                                  p3[:, :, 4:8], op=ALU.max)
                eng.tensor_tensor(p3[:, :, 0:2], p3[:, :, 0:2],
                                  p3[:, :, 2:4], op=ALU.max)
                osub = slice(cs.start + half * 32, cs.start + half * 32 + 32)
                eng.tensor_tensor(x2all[:, osub], p3[:, :, 0:1].squeeze(-1),
                                  p3[:, :, 1:2].squeeze(-1), op=ALU.max)


    # ---------------- batched calib/gate/expansion (per 512-col window) --
    c1all = wide.tile([64, 4096], BF16, tag="wG")
    sigA = wide.tile([64, 4096], BF16, tag="wA")
    sigX2 = wide.tile([128, 4096], BF16, tag="wH")
    pcr = ctx.enter_context(
        tc.tile_pool(name="pcring", bufs=int(os.environ.get("K_NPC", "3"))))
    ee0 = wide.tile([128, 4096], BF16, tag="wK")
    ee1 = wide.tile([128, 4096], BF16, tag="wL")
    ee = [ee0, ee1]

    def phase_c(j):
        if PCSPLIT:
            phase_c_part(j, 0)
            phase_c_part(j, 1)
        else:
            phase_c_part(j, None)

    def phase_c_part(j, part):
        if part is None:
            csl = slice(j * 512, (j + 1) * 512)
            fsl = slice(j * 8, (j + 1) * 8)
        else:
            csl = slice(j * 512 + part * 256, j * 512 + part * 256 + 256)
            fsl = slice(j * 8 + part * 4, j * 8 + part * 4 + 4)
        W = csl.stop - csl.start
        c1p = pp_big.tile([64, W], F32, tag="big")
        nc.tensor.matmul(c1p, sb["b16_ca1_a"], x1all[:, csl], start=True,
                         stop=False)
        nc.tensor.matmul(c1p, sb["b16_ca1_b"], x2all[:, csl], start=False,
                         stop=True)
        nc.scalar.activation(out=c1all[:, csl], in_=c1p, func=AF.Relu,
                             bias=sb["ca1_bias"], scale=sb["ca1_s"])
        sp1 = pp_big.tile([128, W], F32, tag="big")
        nc.tensor.matmul(sp1, sb["b16_ca2"][:, 0:128], c1all[:, csl])
        nc.scalar.activation(out=sigA[:, csl], in_=sp1[0:64, :],
                             func=AF.Sigmoid, bias=sb["cb2_blk1"][0:64])
        nc.scalar.activation(out=sigX2[0:64, csl], in_=sp1[64:128, :],
                             func=AF.Sigmoid, bias=sb["cb2_blk1"][64:128])
        sp2 = pp_big.tile([64, W], F32, tag="big")
        nc.tensor.matmul(sp2, sb["b16_ca2"][:, 128:192], c1all[:, csl])
        nc.scalar.activation(out=sigX2[64:128, csl], in_=sp2, func=AF.Sigmoid,
                             bias=sb["cb2_blk2"])
        p1t = pcr.tile([64, W], BF16, tag="p1r")
        p2t = pcr.tile([128, W], BF16, tag="p2r")
        nc.gpsimd.tensor_mul(p1t, x1all[:, csl], sigA[:, csl])
        nc.gpsimd.tensor_mul(p2t, x2all[:, csl], sigX2[:, csl])
        for b in range(2):
            ep = pp_big.tile([128, W], F32, tag="big")
            osl = slice(b * 128, (b + 1) * 128)
            nc.tensor.matmul(ep, sb["b16_x1a"][:, osl], p1t,
                             start=True, stop=False)
            nc.tensor.matmul(ep, sb["b16_x1b"][:, osl], p2t,
                             start=False, stop=True)
            nc.scalar.activation(out=ee[b][:, csl], in_=ep, func=AF.Relu,
                                 bias=sb["e1bias"][:, b:b + 1],
                                 scale=sb["e1s"][:, b:b + 1])
        tailwin = PCTAIL and j >= 6
        for b in range(4):
            if tailwin and b >= 2:
                xp = pp_big2.tile([128, W], F32, tag="big2")
            else:
                xp = pp_big.tile([128, W], F32, tag="big")
            osl = slice(b * 128, (b + 1) * 128)
            nc.tensor.matmul(xp, sb["b16_x2a"][:, osl], ee[0][:, csl],
                             start=True, stop=False)
            nc.tensor.matmul(xp, sb["b16_x2b"][:, osl], ee[1][:, csl],
                             start=False, stop=True)
            xm = wk2.tile([128, W // 64], F32, tag="xm")
            _wmax(xp, xm, PC_POOL, k=64)
            nc.scalar.activation(out=xfin[:, b, fsl], in_=xm,
                                 func=AF.Relu,
                                 bias=sb["e2bias"][:, b:b + 1],
                                 scale=sb["e2s"][:, b:b + 1])

    STAG = int(os.environ.get("K_STAG", "4"))
    PCSPLIT = int(os.environ.get("K_PCSPLIT", "0"))
    FLAT = int(os.environ.get("K_FLAT", "1"))
    if FLAT:
        # one continuous pipeline over all 32 pairs: phase_a leads b1 by LA,
        # b2 trails b1 by STAG, each phase_c window fires as its 4 pairs
        # complete. No chunk boundaries, so the pipeline never drains.
        # Ring safety: ixb/ixr slots (8) are rewritten 8-LA b1-steps after
        # their reader; S01 slots (WK2) rewritten WK2-STAG steps after.
        LA = int(os.environ.get("K_LA", "4"))
        for pi in range(LA):
            phase_a(pi)

        PCD = int(os.environ.get("K_PCD", "5"))

        def _after_b2(done):
            # window w is emitted at done == 4*w + 3 + PCD
            if done >= PCD + 3 and (done - PCD - 3) % 4 == 0:
                phase_c((done - PCD - 3) // 4)
            if done == NPAIR - 1:
                for w in range((done - PCD - 3) // 4 + 1, NPAIR // 4):
                    phase_c(w)
        for pi in range(NPAIR):
            phase_b1(pi)
            if pi + LA < NPAIR:
                phase_a(pi + LA)
            if pi >= STAG:
                phase_b2(pi - STAG)
                _after_b2(pi - STAG)
        for pi in range(NPAIR - STAG, NPAIR):
            phase_b2(pi)
            _after_b2(pi)
    else:
        nwin = CHUNK // 4
        pending_c = []
        for chunk in range(NPAIR // CHUNK):
            base = chunk * CHUNK
            for pi in range(base, base + CHUNK):
                phase_a(pi)
            for w in pending_c:
                phase_c(w)
            pending_c = []
            for i in range(CHUNK):
                phase_b1(base + i)
                if i >= STAG:
                    phase_b2(base + i - STAG)
            for i in range(CHUNK - STAG, CHUNK):
                phase_b2(base + i)
            pending_c = list(range(nwin * chunk, nwin * chunk + nwin))
        for w in pending_c:
            phase_c(w)

    # ---------------- final stage (256ch x 64 group-cols) ---------------
    tt = wk.tile([128, 2, G], F32, tag="tt")
    FSPLIT = int(os.environ.get("K_FSPLIT", "0"))
    for b in range(2):
        osl = slice(b * 128, (b + 1) * 128)
        rp = pp_med.tile([128, G], F32, tag="med")
        rngs = (slice(0, 48), slice(48, 64)) if FSPLIT else (slice(0, G),)
        for rng in rngs:
            for cb in range(4):
                nc.tensor.matmul(rp[:, rng], sb[f"rd{cb}"][:, osl],
                                 xfin[:, cb, rng],
                                 start=(cb == 0), stop=(cb == 3))
        rr = wk.tile([128, G], F32, tag="rr")
        nc.scalar.activation(out=rr, in_=rp, func=AF.Relu,
                             bias=sb["rdb"][:, b:b + 1],
                             scale=sb["rds"][:, b:b + 1])
        nc.vector.tensor_scalar(out=tt[:, b, :], in0=rr,
                                scalar1=sb["n1s"][:, b:b + 1],
                                scalar2=sb["n1b"][:, b:b + 1],
                                op0=ALU.mult, op1=ALU.add)
    hh = wk.tile([128, 2, G], F32, tag="hh")
    for b in range(2):
        osl = slice(b * 128, (b + 1) * 128)
        hp = pp_med.tile([128, G], F32, tag="med")
        for cb in range(2):
            nc.tensor.matmul(hp, sb[f"sc1_{cb}"][:, osl], tt[:, cb, :],
                             start=(cb == 0), stop=(cb == 1))
        nc.scalar.activation(out=hh[:, b, :], in_=hp, func=AF.Relu,
                             bias=sb["sc1b"][:, b:b + 1])
    for b in range(2):
        osl = slice(b * 128, (b + 1) * 128)
        h2p = pp_med.tile([128, G], F32, tag="med")
        for cb in range(2):
            nc.tensor.matmul(h2p, sb[f"sc2_{cb}"][:, osl], hh[:, cb, :],
                             start=(cb == 0), stop=(cb == 1))
        s2sum = wk.tile([128, G], F32, tag="s2sum")
        nc.vector.tensor_scalar(out=s2sum, in0=h2p,
                                scalar1=sb["sc2b"][:, b:b + 1], scalar2=None,
                                op0=ALU.add)
        s2t = wk.tile([128, G], F32, tag="s2t")
        nc.vector.tensor_add(s2t, s2sum, tt[:, b, :])
        osb = wk.tile([128, G], F32, tag="osb")
        nc.vector.tensor_scalar(out=osb, in0=s2t,
                                scalar1=sb["n2s"][:, b:b + 1],
                                scalar2=sb["n2b"][:, b:b + 1],
                                op0=ALU.mult, op1=ALU.add)
        nc.sync.dma_start(out=out_ap[b * 128:(b + 1) * 128, :], in_=osb)


@functools.lru_cache(maxsize=1)
def _build():
    nc = bacc.Bacc("TRN2", target_bir_lowering=False, debug=False,
                   num_devices=NCORES)
    I = {}
    I["xt16"] = nc.dram_tensor("xt16", (64, 4096), BF16,
                               kind="ExternalInput").ap()
    I["xt3"] = nc.dram_tensor("xt3", (3, 4096), F32,
                              kind="ExternalInput").ap()
    I["blob"] = nc.dram_tensor("blob", (128, BLOB_W), F32,
                               kind="ExternalInput").ap()
    I["blobl"] = nc.dram_tensor("blobl", (128, BLOBL_W), F32,
                                kind="ExternalInput").ap()
    I["blob16"] = nc.dram_tensor("blob16", (128, BLOB16_W), BF16,
                                 kind="ExternalInput").ap()
    I["blob16l"] = nc.dram_tensor("blob16l", (128, BLOB16L_W), BF16,
                                  kind="ExternalInput").ap()
    out_ap = nc.dram_tensor("out", (256, G), F32, kind="ExternalOutput").ap()
    from contextlib import ExitStack
    with tile.TileContext(nc) as tc, ExitStack() as ctx:
        _emit(tc, I, out_ap, ctx)
    nc.compile()
    return nc


def kernel(**inputs):
    nc = _build()
    consts = _np_consts(inputs)
    blob = _pack_blob(consts)
    blobl = _pack_blob_late(consts)
    blob16v, blob16lv = _pack_blob16(consts)

    xyz = inputs["xyz"].astype(np.float32)      # (2, 256, 64, 3)
    feats = inputs["feats"].astype(np.float32)  # (2, 256, 64, 61)
    xf_full = np.concatenate([xyz, feats], axis=-1).reshape(512 * 64, 64)

    in_maps = []
    for c in range(NCORES):
        import ml_dtypes
        sh = xf_full[c * 4096:(c + 1) * 4096, :]
        in_maps.append({
            "blob": blob,
            "blobl": blobl,
            "blob16": blob16v,
            "blob16l": blob16lv,
            "xt16": np.ascontiguousarray(sh.T.astype(ml_dtypes.bfloat16)),
            "xt3": np.ascontiguousarray(sh.T[0:3, :]),
        })

    trace = bool(int(os.environ.get("KERNEL_TRACE", "0")))
    try:
        res = bass_utils.run_bass_kernel_spmd(
            nc, in_maps, core_ids=list(range(NCORES)), trace=trace)
    except ModuleNotFoundError:
        res = bass_utils.run_bass_kernel_spmd(
            nc, in_maps, core_ids=list(range(NCORES)))
    if trace and res.exec_time_ns is not None:
        print(f"HW exec time: {res.exec_time_ns} ns")
        if res.instructions_and_trace is not None:
            print(f"trace: {res.instructions_and_trace[1]}")
        kernel.last_results = res

    out = np.empty((2, 256, 256), dtype=np.float32)
    for c in range(NCORES):
        o = res.results[c]["out"]              # (256, 64)
        b, mlo = divmod(c * G, 256)
        out[b, :, mlo:mlo + G] = o
    return out


if __name__ == "__main__":
    print("building bass graph...")
    nc = _build()
    print("graph built ok")



# revision 75
# speedup vs baseline: 1.0292x; 1.0007x over previous
"""Trainium2 Bass kernel for nn_AttnGNNLayer (EdgeConv-style GNN layer).

Data-parallel over the B*M=512 group axis: 64 groups per core on 8 cores.

Per-group pipeline (K=64 points, knn=16):
  - distance proxy q = x^T x - xx/2 (one ones-row accum matmul; xx from a
    tall (96,128) Square + one f32 matmul against a packed selector)
  - top-16 neighbor indices via DVE max8 / match_replace / max_index
  - one-hot gather matrix S[j, n*16+k]: idx rows DMA'd to (2,1024) u16,
    broadcast-DMA'd to all 128 partitions, then ONE tensor_scalar is_equal
    vs a per-partition iota (4x-mode eligible; runs on the Pool engine,
    which may only touch SBUF - GPSIMD cannot access PSUM on silicon)
  - edge conv: gather matmul accumulates bd@S + bdv@R (R = kron(I,1_16)),
    folding the center term v into psum so the windowed reduce_max (DVE,
    merged (128,1024) 2-bank tiles) directly yields max_k(u[idx]+v); the
    bn+relu then applies at psum egress
  - all 1x1 convs batched over all 64*64=4096 points per core on PE

Emission is software-pipelined: per 8-pair chunk, phase_a (knn) x8, then
b1 (S + e1) / b2 (e2) interleaved with a stagger of 4 so every engine's
in-order stream has other pairs' work between dependent ops; phase_c
(calib/gate/expansion windows) is deferred past the next chunk's phase_a.
Weight constants ship in early/late blobs (f32 + bf16) so the pair loop
does not wait on late-stage conv weights.
"""

import functools
import os
import sys

for _p in ("/opt/trn_rl_repo", "/root/.axon_site/_ro/trn_rl_repo"):
    if os.path.isdir(_p) and _p not in sys.path:
        sys.path.append(_p)

import numpy as np

import concourse.bass as bass
import concourse.mybir as mybir
import concourse.tile as tile
from concourse import bacc, bass_utils

F32 = mybir.dt.float32
BF16 = mybir.dt.bfloat16
U16 = mybir.dt.uint16

B, M, K, KNN = 2, 256, 64, 16
G = 64            # groups per core
NPAIR = G // 2    # 32 pair tiles (2 groups packed in 128 partitions)
NCORES = 8
NEG = -1.0e30
EPS = 1e-5

AF = mybir.ActivationFunctionType
ALU = mybir.AluOpType

# (name, partitions, width) of every constant packed into the blob, in order
_BLOB_LAYOUT = [
    ("iota_col", 128, 1),
    ("neg_iota_col", 128, 1),
    ("ones_row", 1, 1024),
    ("sel96", 96, 32),
    ("s1", 128, 1), ("b1", 128, 1), ("s2", 128, 1), ("b2", 128, 1),
    ("ca1_s", 64, 1), ("ca1_bias", 64, 1),
    ("cb2_blk1", 128, 1), ("cb2_blk2", 64, 1),
    ("e1s", 128, 2), ("e1bias", 128, 2),
    ("e2s", 128, 4), ("e2bias", 128, 4),
]
_BLOB_OFF = {}
_off = 0
for _n, _pp, _w in _BLOB_LAYOUT:
    _BLOB_OFF[_n] = _off
    _off += _w
BLOB_W = _off

# late-stage weights (final 256ch x 64 stage): separate DMA issued after the
# input DMAs so the pair loop can start sooner
_BLOBL_LAYOUT = [
    ("rd0", 128, 256), ("rd1", 128, 256), ("rd2", 128, 256), ("rd3", 128, 256),
    ("rds", 128, 2), ("rdb", 128, 2),
    ("sc1_0", 128, 256), ("sc1_1", 128, 256), ("sc1b", 128, 2),
    ("sc2_0", 128, 256), ("sc2_1", 128, 256), ("sc2b", 128, 2),
    ("n1s", 128, 2), ("n1b", 128, 2), ("n2s", 128, 2), ("n2b", 128, 2),
]
_BLOBL_OFF = {}
_offl = 0
for _n, _pp, _w in _BLOBL_LAYOUT:
    _BLOBL_OFF[_n] = _offl
    _offl += _w
BLOBL_W = _offl

# bf16 constants: pair-loop weights (early) and conv weights (late)
_BLOB16_LAYOUT = [
    ("b16_R2", 128, 1024),
    ("b16_wu1", 64, 64), ("b16_wv1", 64, 64),
    ("b16_wu2", 64, 128), ("b16_wv2", 64, 128),
]
_BLOB16_OFF = {}
_o16 = 0
for _n, _pp, _w in _BLOB16_LAYOUT:
    _BLOB16_OFF[_n] = _o16
    _o16 += _w
BLOB16_W = _o16

_BLOB16L_LAYOUT = [
    ("b16_ca1_a", 64, 64), ("b16_ca1_b", 128, 64), ("b16_ca2", 64, 192),
    ("b16_x1a", 64, 256), ("b16_x1b", 128, 256),
    ("b16_x2a", 128, 512), ("b16_x2b", 128, 512),
]
_BLOB16L_OFF = {}
_o16l = 0
for _n, _pp, _w in _BLOB16L_LAYOUT:
    _BLOB16L_OFF[_n] = _o16l
    _o16l += _w
BLOB16L_W = _o16l


def _np_consts(iw):
    """All constant tensors (iota + host-prepped weights)."""
    f = np.float32
    c = {}
    iota = np.arange(64, dtype=f)
    c["iota_col"] = np.concatenate([iota, iota]).reshape(128, 1)
    c["neg_iota_col"] = -c["iota_col"]
    selg = np.zeros((2, 128), dtype=f)
    selg[0, :64] = 1.0
    selg[1, 64:] = 1.0
    c["selg"] = selg
    c["ones_row"] = np.ones((1, 1024), dtype=f)
    # sel96[c*32+blk, blk] = -0.5: one matmul turns xsq96 (96,128) into
    # -xx/2 for all 4096 points as a (32,128) psum tile
    sel96 = np.zeros((96, 32), dtype=f)
    for _c in range(3):
        for _b in range(32):
            sel96[_c * 32 + _b, _b] = -0.5
    c["sel96"] = sel96
    # replication matrix: R2[p, n*16+k] = (n == p % 64); v-fold accumuland
    _R = np.repeat(np.eye(64, dtype=f), KNN, axis=1)
    c["R2"] = np.vstack([_R, _R])

    e1_w = iw["e1_w"].astype(f)
    W1, W2 = e1_w[:, :64], e1_w[:, 64:]
    c["wu1"] = W1.T.copy()
    c["wv1"] = (W2 - W1).T.copy()
    e2_w = iw["e2_w"].astype(f)
    W21, W22 = e2_w[:, :64], e2_w[:, 64:]
    c["wu2"] = W21.T.copy()
    c["wv2"] = (W22 - W21).T.copy()

    def bn_sb(g, b):
        return (g / np.sqrt(1.0 + EPS)).astype(f), b.astype(f)

    def pair_col(v):
        return np.concatenate([v, v]).reshape(128, 1).astype(f)

    s1, b1 = bn_sb(iw["e1_g"], iw["e1_b"])
    c["s1"], c["b1"] = pair_col(s1), pair_col(b1)
    s2, b2 = bn_sb(iw["e2_g"], iw["e2_b"])
    c["s2"], c["b2"] = s2.reshape(128, 1), b2.reshape(128, 1)

    cal1_w = iw["cal1_w"].astype(f)
    c["ca1_a"] = cal1_w[:, :64].T.copy()
    c["ca1_b"] = cal1_w[:, 64:].T.copy()
    cs, cbv = bn_sb(iw["cal1_g"], iw["cal1_b"])
    c["ca1_s"], c["ca1_bias"] = cs.reshape(64, 1), cbv.reshape(64, 1)

    c["ca2"] = iw["cal2_w"].astype(f).T.copy()
    cb2 = iw["cal2_bias"].astype(f)
    c["cb2_blk1"] = cb2[:128].reshape(128, 1)
    c["cb2_blk2"] = cb2[128:].reshape(64, 1)

    exp1_w = iw["exp1_w"].astype(f)
    c["x1a"] = exp1_w[:, :64].T.copy()
    c["x1b"] = exp1_w[:, 64:].T.copy()
    es, eb = bn_sb(iw["exp1_g"], iw["exp1_b"])
    c["e1s"] = es.reshape(2, 128).T.copy()
    c["e1bias"] = eb.reshape(2, 128).T.copy()

    exp2_w = iw["exp2_w"].astype(f)
    c["x2a"] = exp2_w[:, :128].T.copy()
    c["x2b"] = exp2_w[:, 128:].T.copy()
    es2, eb2 = bn_sb(iw["exp2_g"], iw["exp2_b"])
    c["e2s"] = es2.reshape(4, 128).T.copy()
    c["e2bias"] = eb2.reshape(4, 128).T.copy()

    rdT = iw["red_w"].astype(f).T.reshape(4, 128, 256)
    for i in range(4):
        c[f"rd{i}"] = rdT[i].copy()
    rs, rb = bn_sb(iw["red_g"], iw["red_b"])
    c["rds"] = rs.reshape(2, 128).T.copy()
    c["rdb"] = rb.reshape(2, 128).T.copy()

    sc1T = iw["sc1_w"].astype(f).T.reshape(2, 128, 256)
    c["sc1_0"], c["sc1_1"] = sc1T[0].copy(), sc1T[1].copy()
    c["sc1b"] = iw["sc1_b"].astype(f).reshape(2, 128).T.copy()
    sc2T = iw["sc2_w"].astype(f).T.reshape(2, 128, 256)
    c["sc2_0"], c["sc2_1"] = sc2T[0].copy(), sc2T[1].copy()
    c["sc2b"] = iw["sc2_b"].astype(f).reshape(2, 128).T.copy()

    n1s, n1b = bn_sb(iw["sc_n1_g"], iw["sc_n1_b"])
    c["n1s"] = (2.0 * n1s).reshape(2, 128).T.copy()
    c["n1b"] = n1b.reshape(2, 128).T.copy()
    n2s, n2b = bn_sb(iw["sc_n2_g"], iw["sc_n2_b"])
    c["n2s"] = n2s.reshape(2, 128).T.copy()
    c["n2b"] = n2b.reshape(2, 128).T.copy()
    return c


def _pack_blob(c):
    blob = np.zeros((128, BLOB_W), dtype=np.float32)
    for name, p, w in _BLOB_LAYOUT:
        v = c[name]
        assert v.shape == (p, w), (name, v.shape, (p, w))
        blob[:p, _BLOB_OFF[name]:_BLOB_OFF[name] + w] = v
    return blob


def _pack_blob_late(c):
    blob = np.zeros((128, BLOBL_W), dtype=np.float32)
    for name, p, w in _BLOBL_LAYOUT:
        v = c[name]
        assert v.shape == (p, w), (name, v.shape, (p, w))
        blob[:p, _BLOBL_OFF[name]:_BLOBL_OFF[name] + w] = v
    return blob


def _pack_blob16(c):
    import ml_dtypes
    src16 = {"b16_R2": c["R2"],
             "b16_wu1": c["wu1"], "b16_wv1": c["wv1"],
             "b16_wu2": c["wu2"], "b16_wv2": c["wv2"],
             "b16_ca1_a": c["ca1_a"], "b16_ca1_b": c["ca1_b"],
             "b16_ca2": c["ca2"], "b16_x1a": c["x1a"], "b16_x1b": c["x1b"],
             "b16_x2a": c["x2a"], "b16_x2b": c["x2b"]}
    blob = np.zeros((128, BLOB16_W), dtype=ml_dtypes.bfloat16)
    for name, p, w in _BLOB16_LAYOUT:
        v = src16[name]
        assert v.shape == (p, w), (name, v.shape, (p, w))
        blob[:p, _BLOB16_OFF[name]:_BLOB16_OFF[name] + w] = v.astype(
            ml_dtypes.bfloat16)
    blobl = np.zeros((128, BLOB16L_W), dtype=ml_dtypes.bfloat16)
    for name, p, w in _BLOB16L_LAYOUT:
        v = src16[name]
        assert v.shape == (p, w), (name, v.shape, (p, w))
        blobl[:p, _BLOB16L_OFF[name]:_BLOB16L_OFF[name] + w] = v.astype(
            ml_dtypes.bfloat16)
    return blob, blobl


def _emit(tc, I, out_ap, ctx):
    nc = tc.nc

    cp = ctx.enter_context(tc.tile_pool(name="const", bufs=1))
    wide = ctx.enter_context(tc.tile_pool(name="wide", bufs=1))
    MERGE = int(os.environ.get("K_MERGE", "1"))
    nbig = int(os.environ.get("K_NBIG", "2")) if MERGE else 6
    pp_big = ctx.enter_context(
        tc.tile_pool(name="ps_big", bufs=nbig, space="PSUM"))
    pp_big2 = ctx.enter_context(
        tc.tile_pool(name="ps_big2", bufs=int(os.environ.get("K_NBIG2", "2")),
                     space="PSUM"))
    pp_med = ctx.enter_context(
        tc.tile_pool(name="ps_med", bufs=int(os.environ.get("K_NMED", "2")),
                     space="PSUM"))
    wk = ctx.enter_context(
        tc.tile_pool(name="work", bufs=int(os.environ.get("K_WK", "10"))))
    wk2 = ctx.enter_context(
        tc.tile_pool(name="work2", bufs=int(os.environ.get("K_WK2", "8"))))
    ring = ctx.enter_context(tc.tile_pool(name="ring", bufs=1))

    # ---- DMA order = HWDGE processing order: the knn-critical pieces
    # (x96 for xx, xt3 rows, f32 blob with sel96/iota) go first so the
    # distance/top-k chain starts ~4us earlier; bf16 weights and the input
    # x (first needed by the e1 u/v matmuls) follow; late-stage weights
    # last.
    x96 = wide.tile([96, 128], F32, tag="w96")
    nc.sync.dma_start(out=x96,
                      in_=bass.AP(tensor=I["xt3"].tensor, offset=0,
                                  ap=[[4096, 3], [128, 32], [1, 128]]))
    blob = cp.tile([128, BLOB_W], F32, tag="blob")
    nc.sync.dma_start(out=blob, in_=I["blob"])
    sb = {}
    for name, p, w in _BLOB_LAYOUT:
        sb[name] = blob[0:p, _BLOB_OFF[name]:_BLOB_OFF[name] + w]
    blob16 = cp.tile([128, BLOB16_W], BF16, tag="blob16")
    for name, p, w in _BLOB16_LAYOUT:
        sb[name] = blob16[0:p, _BLOB16_OFF[name]:_BLOB16_OFF[name] + w]
    blob16l = cp.tile([128, BLOB16L_W], BF16, tag="blob16l")
    for name, p, w in _BLOB16L_LAYOUT:
        sb[name] = blob16l[0:p, _BLOB16L_OFF[name]:_BLOB16L_OFF[name] + w]
    xsq96 = wide.tile([96, 128], F32, tag="w96b")
    nc.scalar.activation(out=xsq96, in_=x96, func=AF.Square)
    nxp32 = pp_med.tile([32, 128], F32, tag="med")
    nc.tensor.matmul(nxp32, sb["sel96"], xsq96)
    nxs = wide.tile([32, 128], F32, tag="w96c")
    nc.scalar.activation(out=nxs, in_=nxp32, func=AF.Copy)

    B4 = wide.tile([4, 4096], F32, tag="wC")
    A4 = wide.tile([4, 4096], F32, tag="wB")
    nc.sync.dma_start(out=B4[0:3, :], in_=I["xt3"])
    nc.sync.dma_start(out=B4[3:4, :], in_=nxs)
    nc.sync.dma_start(out=A4[0:3, :], in_=I["xt3"])
    nc.sync.dma_start(out=A4[3:4, :],
                      in_=bass.AP(tensor=I["blob"].tensor,
                                  offset=_BLOB_OFF["ones_row"],
                                  ap=[[0, 1], [0, 4], [1, 1024]]))

    # bf16 weights, input x, then late-stage weights
    nc.sync.dma_start(out=blob16, in_=I["blob16"])
    x = wide.tile([64, 4096], BF16, tag="wD")
    for t in range(8):
        nc.sync.dma_start(out=x[:, t * 512:(t + 1) * 512],
                          in_=I["xt16"][:, t * 512:(t + 1) * 512])
    blobl = cp.tile([128, BLOBL_W], F32, tag="blobl")
    nc.sync.dma_start(out=blobl, in_=I["blobl"])
    nc.sync.dma_start(out=blob16l, in_=I["blob16l"])
    for name, p, w in _BLOBL_LAYOUT:
        sb[name] = blobl[0:p, _BLOBL_OFF[name]:_BLOBL_OFF[name] + w]

    # all pairwise-distance tiles upfront (prologue is DMA-bound, engines
    # idle): qt_all[:, pi*64:...] = q for pair pi; frees the psum med ring
    # and the ACT stream from per-pair distance work
    QTALL = int(os.environ.get("K_QTALL", "1"))
    qt_all = None
    if QTALL:
        qt_all = wide.tile([128, NPAIR * 64], F32, tag="wQT")
        for pi in range(NPAIR):
            _cs1 = slice((2 * pi) * 64, (2 * pi + 1) * 64)
            _cs2 = slice((2 * pi + 1) * 64, (2 * pi + 2) * 64)
            _pdp = pp_med.tile([128, 64], F32, tag="med")
            nc.tensor.matmul(_pdp[0:64, :], A4[:, _cs1], B4[:, _cs1])
            nc.tensor.matmul(_pdp[64:128, :], A4[:, _cs2], B4[:, _cs2])
            nc.scalar.activation(out=qt_all[:, pi * 64:(pi + 1) * 64],
                                 in_=_pdp, func=AF.Copy)

    # gated activations accumulated across all groups (for batched convs)
    x1all = wide.tile([64, 4096], BF16, tag="wE")
    x2all = wide.tile([128, 4096], BF16, tag="wF")
    # final per-group features (512ch as 4 blocks x 64 groups)
    xfin = cp.tile([128, 4, G], F32, tag="xfin")

    BDM = int(os.environ.get("K_BDM", "1"))
    bd_ring = []
    bdv_ring = []
    bdd_ring = []
    for ri in range(int(os.environ.get("K_BD", "6"))):
        if BDM:
            bddt = cp.tile([128, 256], BF16, tag=f"bddring{ri}")
            nc.gpsimd.memset(bddt, 0.0)
            bdd_ring.append(bddt)
            bd_ring.append(bddt[:, 0:128])
            bdv_ring.append(bddt[:, 128:256])
        else:
            bdt = cp.tile([128, 128], BF16, tag=f"bdring{ri}")
            nc.gpsimd.memset(bdt, 0.0)
            bd_ring.append(bdt)
            bdvt = cp.tile([128, 128], BF16, tag=f"bdvring{ri}")
            nc.gpsimd.memset(bdvt, 0.0)
            bdv_ring.append(bdvt)

    ADDP = int(os.environ.get("K_ADDP", "0"))
    E2_MODE = os.environ.get("K_E2M", "a")  # a | b | c | bc
    PCTAIL = int(os.environ.get("K_PCTAIL", "0"))
    SPOOL = int(os.environ.get("K_SPOOL", "1"))
    RELUP = int(os.environ.get("K_RELUP", "0"))
    RELUX2 = int(os.environ.get("K_RELUX2", "0"))

    def _relu_sb(out, in_, s_col, b_col, pool=None):
        # relu(s*in + b) from sbuf: 2 pool ops, or 1 act op
        if RELUP if pool is None else pool:
            tmp = wk.tile(list(in_.shape), F32, tag="rtmp")
            nc.gpsimd.tensor_scalar(out=tmp, in0=in_, scalar1=s_col,
                                    scalar2=b_col, op0=ALU.mult, op1=ALU.add)
            nc.gpsimd.tensor_scalar(out=out, in0=tmp, scalar1=0.0,
                                    scalar2=None, op0=ALU.max)
        else:
            nc.scalar.activation(out=out, in_=in_, func=AF.Relu,
                                 bias=b_col, scale=s_col)
    F1 = int(os.environ.get("K_F1", "1"))
    F2 = int(os.environ.get("K_F2", "1"))
    QTP = int(os.environ.get("K_QTP", "0"))

    def _tadd(out, a, b):
        if ADDP:
            nc.gpsimd.tensor_tensor(out, a, b, op=ALU.add)
        else:
            nc.vector.tensor_add(out, a, b)

    # windowed-max placement: offload part of the (128, 32n, 16k) max blocks
    # from DVE (TensorReduce) to the mostly-idle Pool engine (pairwise-max
    # tree, in place on the psum tile)
    E1_POOL = int(os.environ.get("K_E1P", "0"))
    E2_POOL_ROUNDS = tuple(
        int(v) for v in os.environ.get("K_E2P", "0,0").split(","))
    PC_POOL = int(os.environ.get("K_PCP", "0"))

    def _wmax(gp, out, pool_rounds, k=KNN):
        """max over k of gp (128, 512) viewed as (p, n, k) -> out (128, 512/k).

        pool_rounds pairwise-max rounds run on the Pool engine (in place on
        the psum tile); the remaining window is reduced on DVE."""
        g3 = gp.rearrange("p (n k) -> p n k", k=k)
        w = k
        for _ in range(pool_rounds):
            h = w // 2
            nc.gpsimd.tensor_tensor(g3[:, :, 0:h], g3[:, :, 0:h],
                                    g3[:, :, h:w], op=ALU.max)
            w = h
        nc.vector.reduce_max(out=out, in_=g3[:, :, 0:w],
                             axis=mybir.AxisListType.X)

    # ---------------- per-pair loops (chunked for DMA latency hiding) ----
    CHUNK = int(os.environ.get("K_CHUNK", "8"))
    NIXB = int(os.environ.get("K_NIXB", "8"))
    ixb_ring = [None] * NIXB
    ixr_all = ring.tile([2 * CHUNK, 1024], U16, tag="ixr_all")

    def phase_a(pi):
        g1, g2 = 2 * pi, 2 * pi + 1
        cs1 = slice(g1 * 64, (g1 + 1) * 64)
        cs2 = slice(g2 * 64, (g2 + 1) * 64)

        # q = x_m.x_n - xx_n/2 for both groups -> (128, 64)
        # (row-constant -xx_m/2 term dropped: doesn't change row top-k)
        if QTALL:
            qt = qt_all[:, pi * 64:(pi + 1) * 64]
        else:
            pdp = pp_med.tile([128, 64], F32, tag="med")
            for h, cs in ((0, cs1), (1, cs2)):
                nc.tensor.matmul(pdp[h * 64:(h + 1) * 64, :],
                                 A4[:, cs], B4[:, cs])
            qt = wk.tile([128, 64], F32, tag="qt")
            if QTP:
                nc.gpsimd.tensor_copy(qt, pdp)
            else:
                nc.scalar.activation(out=qt, in_=pdp, func=AF.Copy)

        # top-16 indices per point row
        mx = wk.tile([128, 16], F32, tag="mx")
        ix = wk.tile([128, 16], U16, tag="ix")
        qt2 = wk.tile([128, 64], F32, tag="qt2")
        nc.vector.max(out=mx[:, 0:8], in_=qt)
        nc.vector.max_index(out=ix[:, 0:8], in_max=mx[:, 0:8], in_values=qt)
        nc.vector.match_replace(out=qt2, in_to_replace=mx[:, 0:8],
                                in_values=qt, imm_value=NEG)
        nc.vector.max(out=mx[:, 8:16], in_=qt2)
        nc.vector.max_index(out=ix[:, 8:16], in_max=mx[:, 8:16], in_values=qt2)

        # idx row form (2, 1024) u16, then broadcast to all 128 partitions so
        # the one-hot compare can run at 4x (2-byte sbuf in/out)
        s2 = 2 * (pi % CHUNK)
        ixrows = ixr_all[s2:s2 + 2, :]
        nc.sync.dma_start(out=ixrows, in_=ix)
        ixb = ring.tile([128, 1024], U16, tag=f"ixb{pi % NIXB}")
        ixb_ring[pi % NIXB] = ixb
        nc.sync.dma_start(
            out=ixb, in_=ixrows.unsqueeze(1).broadcast_to((2, 64, 1024)))

    S01_ring = [None] * CHUNK

    def phase_b1(pi):
        """S-matrix + e1 edge conv for pair pi."""
        g1, g2 = 2 * pi, 2 * pi + 1
        cs1 = slice(g1 * 64, (g1 + 1) * 64)
        cs2 = slice(g2 * 64, (g2 + 1) * 64)
        ixb = ixb_ring[pi % NIXB]
        S01 = wk2.tile([128, 1024], BF16, tag="S01")
        S01_ring[pi % CHUNK] = S01
        # S01 = (ix_bcast == iota_p): 4x dve op (all operands 2-byte sbuf)
        seng = nc.gpsimd if (SPOOL == 1 or (SPOOL == 2 and pi % 2)) \
            else nc.vector
        seng.tensor_scalar(out=S01, in0=ixb,
                           scalar1=sb["iota_col"], scalar2=None,
                           op0=ALU.is_equal)

        bd = bd_ring[pi % len(bd_ring)]
        if F1:
            # u and v-fold matmuls into one psum tile; merged copies
            uvv = pp_med.tile([128, 256], F32, tag="med")
            nc.tensor.matmul(uvv[0:64, 0:64], x[:, cs1], sb["b16_wu1"])
            nc.tensor.matmul(uvv[64:128, 64:128], x[:, cs2], sb["b16_wu1"])
            nc.tensor.matmul(uvv[0:64, 128:192], x[:, cs1], sb["b16_wv1"])
            nc.tensor.matmul(uvv[64:128, 192:256], x[:, cs2], sb["b16_wv1"])
            bdv = bdv_ring[pi % len(bdv_ring)]
            if BDM:
                # one copy per partition half: {u block, v block} as a
                # strided access pattern on both sides.
                # col = a*128 + b*64 + c: a selects u/v, b selects group
                bdd = bdd_ring[pi % len(bdd_ring)]
                sv = uvv.rearrange("p (a b c) -> p a b c", a=2, b=2)
                dv = bdd.rearrange("p (a b c) -> p a b c", a=2, b=2)
                nc.scalar.activation(out=dv[0:64, :, 0:1, :],
                                     in_=sv[0:64, :, 0:1, :], func=AF.Copy)
                nc.scalar.activation(out=dv[64:128, :, 1:2, :],
                                     in_=sv[64:128, :, 1:2, :], func=AF.Copy)
            else:
                nc.scalar.activation(
                    out=bd[0:64, 0:64], in_=uvv[0:64, 0:64], func=AF.Copy)
                nc.scalar.activation(
                    out=bd[64:128, 64:128], in_=uvv[64:128, 64:128],
                    func=AF.Copy)
                nc.scalar.activation(
                    out=bdv[0:64, 0:64], in_=uvv[0:64, 128:192], func=AF.Copy)
                nc.scalar.activation(
                    out=bdv[64:128, 64:128], in_=uvv[64:128, 192:256],
                    func=AF.Copy)
        else:
            uv1 = pp_med.tile([128, 128], F32, tag="med")
            nc.tensor.matmul(uv1[0:64, 0:64], x[:, cs1], sb["b16_wu1"])
            nc.tensor.matmul(uv1[64:128, 64:128], x[:, cs2], sb["b16_wu1"])
            nc.scalar.activation(out=bd[0:64, 0:64], in_=uv1[0:64, 0:64],
                                 func=AF.Copy)
            nc.scalar.activation(out=bd[64:128, 64:128],
                                 in_=uv1[64:128, 64:128], func=AF.Copy)
        m1 = wk.tile([128, 64], F32, tag="m1")
        if MERGE:
            g1p = pp_big2.tile([128, 1024], F32, tag="big2")
            for half in range(2):
                csl = slice(half * 512, (half + 1) * 512)
                gh = g1p[:, csl]
                if F1:
                    nc.tensor.matmul(gh, bd, S01[:, csl], start=True,
                                     stop=False)
                    nc.tensor.matmul(gh, bdv, sb["b16_R2"][:, csl],
                                     start=False, stop=True)
                else:
                    nc.tensor.matmul(gh, bd, S01[:, csl])
            nc.vector.reduce_max(
                out=m1, in_=g1p.rearrange("p (n k) -> p n k", k=KNN),
                axis=mybir.AxisListType.X)
        else:
            for half in range(2):
                csl = slice(half * 512, (half + 1) * 512)
                g1p = pp_big.tile([128, 512], F32, tag="big")
                if F1:
                    nc.tensor.matmul(g1p, bd, S01[:, csl], start=True,
                                     stop=False)
                    nc.tensor.matmul(g1p, bdv, sb["b16_R2"][:, csl],
                                     start=False, stop=True)
                else:
                    nc.tensor.matmul(g1p, bd, S01[:, csl])
                _wmax(g1p, m1[:, half * 32:(half + 1) * 32], E1_POOL)
        if F1:
            _relu_sb(x1all[:, cs1], m1[0:64, :], sb["s1"][0:64],
                     sb["b1"][0:64])
            _relu_sb(x1all[:, cs2], m1[64:128, :], sb["s1"][64:128],
                     sb["b1"][64:128])
        else:
            v1 = pp_med.tile([128, 64], F32, tag="med")
            nc.tensor.matmul(v1[0:64, :], sb["b16_wv1"], x[:, cs1])
            nc.tensor.matmul(v1[64:128, :], sb["b16_wv1"], x[:, cs2])
            t1a = wk.tile([64, 64], F32, tag="t1a")
            _tadd(t1a, m1[0:64, :], v1[0:64, :])
            t1b = wk.tile([128, 64], F32, tag="t1b")
            _tadd(t1b[64:128, :], m1[64:128, :], v1[64:128, :])
            nc.scalar.activation(out=x1all[:, cs1], in_=t1a, func=AF.Relu,
                                 bias=sb["b1"][0:64], scale=sb["s1"][0:64])
            nc.scalar.activation(out=x1all[:, cs2], in_=t1b[64:128, :],
                                 func=AF.Relu,
                                 bias=sb["b1"][64:128], scale=sb["s1"][64:128])

    def phase_b2(pi):
        """e2 edge conv for pair pi (consumes x1all + S01)."""
        g1, g2 = 2 * pi, 2 * pi + 1
        cs1 = slice(g1 * 64, (g1 + 1) * 64)
        cs2 = slice(g2 * 64, (g2 + 1) * 64)
        S01 = S01_ring[pi % CHUNK]
        for h, cs in ((0, cs1), (1, cs2)):
            xg = x1all[:, cs]
            psl = slice(h * 64, (h + 1) * 64)
            if E2_MODE == "a" and F2:
                uvp = pp_med.tile([64, 256], F32, tag="med")
                nc.tensor.matmul(uvp[:, 0:128], xg, sb["b16_wu2"])
                nc.tensor.matmul(uvp[:, 128:256], xg, sb["b16_wv2"])
                uvs = wk.tile([128, 256], BF16, tag="uvs")
                nc.scalar.activation(out=uvs[psl, :], in_=uvp, func=AF.Copy)
                uT2 = uvs[:, 0:128]
                vv2 = uvs[:, 128:256]
            else:
                uT2 = wk.tile([128, 128], BF16, tag="uT2")
                uT2p = pp_med.tile([64, 128], F32, tag="med")
                nc.tensor.matmul(uT2p, xg, sb["b16_wu2"])
                nc.scalar.activation(out=uT2[psl, :], in_=uT2p, func=AF.Copy)
            if E2_MODE == "a":
                m2 = wk.tile([128, 64], F32, tag="m2")
                if MERGE:
                    g2p = pp_big2.tile([128, 1024], F32, tag="big2")
                    for half in range(2):
                        csl = slice(half * 512, (half + 1) * 512)
                        gh = g2p[:, csl]
                        if F2:
                            nc.tensor.matmul(gh, uT2[psl, :], S01[psl, csl],
                                             start=True, stop=False)
                            nc.tensor.matmul(gh, vv2[psl, :],
                                             sb["b16_R2"][psl, csl],
                                             start=False, stop=True)
                        else:
                            nc.tensor.matmul(gh, uT2[psl, :], S01[psl, csl])
                    nc.vector.reduce_max(
                        out=m2, in_=g2p.rearrange("p (n k) -> p n k", k=KNN),
                        axis=mybir.AxisListType.X)
                else:
                    for half in range(2):
                        csl = slice(half * 512, (half + 1) * 512)
                        g2p = pp_big.tile([128, 512], F32, tag="big")
                        if F2:
                            nc.tensor.matmul(g2p, uT2[psl, :], S01[psl, csl],
                                             start=True, stop=False)
                            nc.tensor.matmul(g2p, vv2[psl, :],
                                             sb["b16_R2"][psl, csl],
                                             start=False, stop=True)
                        else:
                            nc.tensor.matmul(g2p, uT2[psl, :], S01[psl, csl])
                        _wmax(g2p, m2[:, half * 32:(half + 1) * 32],
                              E2_POOL_ROUNDS[half])
                if F2:
                    _relu_sb(x2all[:, cs], m2, sb["s2"], sb["b2"],
                             pool=RELUX2)
                else:
                    v2 = pp_med.tile([128, 64], F32, tag="med")
                    nc.tensor.matmul(v2, sb["b16_wv2"], xg)
                    t2 = wk.tile([128, 64], F32, tag="t2")
                    _tadd(t2, m2, v2)
                    nc.scalar.activation(out=x2all[:, cs], in_=t2,
                                         func=AF.Relu,
                                         bias=sb["b2"], scale=sb["s2"])
                continue
            # v-folded path: g2p = uT2.S + vT2.R, then relu(bn) at psum
            # egress (valid pre-max: bn scale > 0), max-tree on sbuf bf16
            vv2p = pp_med.tile([64, 128], F32, tag="med")
            nc.tensor.matmul(vv2p, xg, sb["b16_wv2"])
            vv2 = wk.tile([64, 128], BF16, tag="vv2")
            nc.scalar.activation(out=vv2, in_=vv2p, func=AF.Copy)
            for half in range(2):
                csl = slice(half * 512, (half + 1) * 512)
                g2p = pp_big.tile([128, 512], F32, tag="big")
                nc.tensor.matmul(g2p, uT2[psl, :], S01[psl, csl],
                                 start=True, stop=False)
                nc.tensor.matmul(g2p, vv2, sb["b16_R2"][0:64, csl],
                                 start=False, stop=True)
                x2pre = wk.tile([128, 512], BF16, tag="x2pre")
                nc.scalar.activation(out=x2pre, in_=g2p, func=AF.Relu,
                                     bias=sb["b2"], scale=sb["s2"])
                p3 = x2pre.rearrange("p (n k) -> p n k", k=KNN)
                eng = nc.gpsimd if E2_MODE == "c" or (
                    E2_MODE == "bc" and half == 1) else nc.vector
                eng.tensor_tensor(p3[:, :, 0:8], p3[:, :, 0:8],
                                  p3[:, :, 8:16], op=ALU.max)
                eng.tensor_tensor(p3[:, :, 0:4], p3[:, :, 0:4],
                                  p3[:, :, 4:8], op=ALU.max)
                eng.tensor_tensor(p3[:, :, 0:2], p3[:, :, 0:2],
                                  p3[:, :, 2:4], op=ALU.max)
                osub = slice(cs.start + half * 32, cs.start + half * 32 + 32)
                eng.tensor_tensor(x2all[:, osub], p3[:, :, 0:1].squeeze(-1),
                                  p3[:, :, 1:2].squeeze(-1), op=ALU.max)


    # ---------------- batched calib/gate/expansion (per 512-col window) --
    c1all = wide.tile([64, 4096], BF16, tag="wG")
    sigA = wide.tile([64, 4096], BF16, tag="wA")
    sigX2 = wide.tile([128, 4096], BF16, tag="wH")
    pcr = ctx.enter_context(
        tc.tile_pool(name="pcring", bufs=int(os.environ.get("K_NPC", "3"))))
    ee0 = wide.tile([128, 4096], BF16, tag="wK")
    ee1 = wide.tile([128, 4096], BF16, tag="wL")
    ee = [ee0, ee1]

    def phase_c(j):
        if PCSPLIT:
            phase_c_part(j, 0)
            phase_c_part(j, 1)
        else:
            phase_c_part(j, None)

    def phase_c_part(j, part):
        if part is None:
            csl = slice(j * 512, (j + 1) * 512)
            fsl = slice(j * 8, (j + 1) * 8)
        else:
            csl = slice(j * 512 + part * 256, j * 512 + part * 256 + 256)
            fsl = slice(j * 8 + part * 4, j * 8 + part * 4 + 4)
        W = csl.stop - csl.start
        c1p = pp_big.tile([64, W], F32, tag="big")
        nc.tensor.matmul(c1p, sb["b16_ca1_a"], x1all[:, csl], start=True,
                         stop=False)
        nc.tensor.matmul(c1p, sb["b16_ca1_b"], x2all[:, csl], start=False,
                         stop=True)
        nc.scalar.activation(out=c1all[:, csl], in_=c1p, func=AF.Relu,
                             bias=sb["ca1_bias"], scale=sb["ca1_s"])
        sp1 = pp_big.tile([128, W], F32, tag="big")
        nc.tensor.matmul(sp1, sb["b16_ca2"][:, 0:128], c1all[:, csl])
        nc.scalar.activation(out=sigA[:, csl], in_=sp1[0:64, :],
                             func=AF.Sigmoid, bias=sb["cb2_blk1"][0:64])
        nc.scalar.activation(out=sigX2[0:64, csl], in_=sp1[64:128, :],
                             func=AF.Sigmoid, bias=sb["cb2_blk1"][64:128])
        sp2 = pp_big.tile([64, W], F32, tag="big")
        nc.tensor.matmul(sp2, sb["b16_ca2"][:, 128:192], c1all[:, csl])
        nc.scalar.activation(out=sigX2[64:128, csl], in_=sp2, func=AF.Sigmoid,
                             bias=sb["cb2_blk2"])
        p1t = pcr.tile([64, W], BF16, tag="p1r")
        p2t = pcr.tile([128, W], BF16, tag="p2r")
        nc.gpsimd.tensor_mul(p1t, x1all[:, csl], sigA[:, csl])
        nc.gpsimd.tensor_mul(p2t, x2all[:, csl], sigX2[:, csl])
        for b in range(2):
            ep = pp_big.tile([128, W], F32, tag="big")
            osl = slice(b * 128, (b + 1) * 128)
            nc.tensor.matmul(ep, sb["b16_x1a"][:, osl], p1t,
                             start=True, stop=False)
            nc.tensor.matmul(ep, sb["b16_x1b"][:, osl], p2t,
                             start=False, stop=True)
            nc.scalar.activation(out=ee[b][:, csl], in_=ep, func=AF.Relu,
                                 bias=sb["e1bias"][:, b:b + 1],
                                 scale=sb["e1s"][:, b:b + 1])
        tailwin = PCTAIL and j >= 6
        for b in range(4):
            if tailwin and b >= 2:
                xp = pp_big2.tile([128, W], F32, tag="big2")
            else:
                xp = pp_big.tile([128, W], F32, tag="big")
            osl = slice(b * 128, (b + 1) * 128)
            nc.tensor.matmul(xp, sb["b16_x2a"][:, osl], ee[0][:, csl],
                             start=True, stop=False)
            nc.tensor.matmul(xp, sb["b16_x2b"][:, osl], ee[1][:, csl],
                             start=False, stop=True)
            xm = wk2.tile([128, W // 64], F32, tag="xm")
            _wmax(xp, xm, PC_POOL, k=64)
            nc.scalar.activation(out=xfin[:, b, fsl], in_=xm,
                                 func=AF.Relu,
                                 bias=sb["e2bias"][:, b:b + 1],
                                 scale=sb["e2s"][:, b:b + 1])

    STAG = int(os.environ.get("K_STAG", "4"))
    PCSPLIT = int(os.environ.get("K_PCSPLIT", "0"))
    FLAT = int(os.environ.get("K_FLAT", "1"))
    if FLAT:
        # one continuous pipeline over all 32 pairs: phase_a leads b1 by LA,
        # b2 trails b1 by STAG, each phase_c window fires as its 4 pairs
        # complete. No chunk boundaries, so the pipeline never drains.
        # Ring safety: ixb/ixr slots (8) are rewritten 8-LA b1-steps after
        # their reader; S01 slots (WK2) rewritten WK2-STAG steps after.
        LA = int(os.environ.get("K_LA", "4"))
        for pi in range(LA):
            phase_a(pi)

        PCD = int(os.environ.get("K_PCD", "4"))

        def _after_b2(done):
            # window w is emitted at done == 4*w + 3 + PCD
            if done >= PCD + 3 and (done - PCD - 3) % 4 == 0:
                phase_c((done - PCD - 3) // 4)
            if done == NPAIR - 1:
                for w in range((done - PCD - 3) // 4 + 1, NPAIR // 4):
                    phase_c(w)
        for pi in range(NPAIR):
            phase_b1(pi)
            if pi + LA < NPAIR:
                phase_a(pi + LA)
            if pi >= STAG:
                phase_b2(pi - STAG)
                _after_b2(pi - STAG)
        for pi in range(NPAIR - STAG, NPAIR):
            phase_b2(pi)
            _after_b2(pi)
    else:
        nwin = CHUNK // 4
        pending_c = []
        for chunk in range(NPAIR // CHUNK):
            base = chunk * CHUNK
            for pi in range(base, base + CHUNK):
                phase_a(pi)
            for w in pending_c:
                phase_c(w)
            pending_c = []
            for i in range(CHUNK):
                phase_b1(base + i)
                if i >= STAG:
                    phase_b2(base + i - STAG)
            for i in range(CHUNK - STAG, CHUNK):
                phase_b2(base + i)
            pending_c = list(range(nwin * chunk, nwin * chunk + nwin))
        for w in pending_c:
            phase_c(w)

    # ---------------- final stage (256ch x 64 group-cols) ---------------
    tt = wk.tile([128, 2, G], F32, tag="tt")
    FSPLIT = int(os.environ.get("K_FSPLIT", "0"))
    for b in range(2):
        osl = slice(b * 128, (b + 1) * 128)
        rp = pp_med.tile([128, G], F32, tag="med")
        rngs = (slice(0, 48), slice(48, 64)) if FSPLIT else (slice(0, G),)
        for rng in rngs:
            for cb in range(4):
                nc.tensor.matmul(rp[:, rng], sb[f"rd{cb}"][:, osl],
                                 xfin[:, cb, rng],
                                 start=(cb == 0), stop=(cb == 3))
        rr = wk.tile([128, G], F32, tag="rr")
        nc.scalar.activation(out=rr, in_=rp, func=AF.Relu,
                             bias=sb["rdb"][:, b:b + 1],
                             scale=sb["rds"][:, b:b + 1])
        nc.vector.tensor_scalar(out=tt[:, b, :], in0=rr,
                                scalar1=sb["n1s"][:, b:b + 1],
                                scalar2=sb["n1b"][:, b:b + 1],
                                op0=ALU.mult, op1=ALU.add)
    hh = wk.tile([128, 2, G], F32, tag="hh")
    for b in range(2):
        osl = slice(b * 128, (b + 1) * 128)
        hp = pp_med.tile([128, G], F32, tag="med")
        for cb in range(2):
            nc.tensor.matmul(hp, sb[f"sc1_{cb}"][:, osl], tt[:, cb, :],
                             start=(cb == 0), stop=(cb == 1))
        nc.scalar.activation(out=hh[:, b, :], in_=hp, func=AF.Relu,
                             bias=sb["sc1b"][:, b:b + 1])
    for b in range(2):
        osl = slice(b * 128, (b + 1) * 128)
        h2p = pp_med.tile([128, G], F32, tag="med")
        for cb in range(2):
            nc.tensor.matmul(h2p, sb[f"sc2_{cb}"][:, osl], hh[:, cb, :],
                             start=(cb == 0), stop=(cb == 1))
        s2sum = wk.tile([128, G], F32, tag="s2sum")
        nc.vector.tensor_scalar(out=s2sum, in0=h2p,
                                scalar1=sb["sc2b"][:, b:b + 1], scalar2=None,
                                op0=ALU.add)
        s2t = wk.tile([128, G], F32, tag="s2t")
        nc.vector.tensor_add(s2t, s2sum, tt[:, b, :])
        osb = wk.tile([128, G], F32, tag="osb")
        nc.vector.tensor_scalar(out=osb, in0=s2t,
                                scalar1=sb["n2s"][:, b:b + 1],
                                scalar2=sb["n2b"][:, b:b + 1],
                                op0=ALU.mult, op1=ALU.add)
        nc.sync.dma_start(out=out_ap[b * 128:(b + 1) * 128, :], in_=osb)


@functools.lru_cache(maxsize=1)
def _build():
    nc = bacc.Bacc("TRN2", target_bir_lowering=False, debug=False,
                   num_devices=NCORES)
    I = {}
    I["xt16"] = nc.dram_tensor("xt16", (64, 4096), BF16,
                               kind="ExternalInput").ap()
    I["xt3"] = nc.dram_tensor("xt3", (3, 4096), F32,
                              kind="ExternalInput").ap()
    I["blob"] = nc.dram_tensor("blob", (128, BLOB_W), F32,
                               kind="ExternalInput").ap()
    I["blobl"] = nc.dram_tensor("blobl", (128, BLOBL_W), F32,
                                kind="ExternalInput").ap()
    I["blob16"] = nc.dram_tensor("blob16", (128, BLOB16_W), BF16,
                                 kind="ExternalInput").ap()
    I["blob16l"] = nc.dram_tensor("blob16l", (128, BLOB16L_W), BF16,
                                  kind="ExternalInput").ap()
    out_ap = nc.dram_tensor("out", (256, G), F32, kind="ExternalOutput").ap()
    from contextlib import ExitStack
    with tile.TileContext(nc) as tc, ExitStack() as ctx:
        _emit(tc, I, out_ap, ctx)
    nc.compile()
    return nc


def kernel(**inputs):
    nc = _build()
    consts = _np_consts(inputs)
    blob = _pack_blob(consts)
    blobl = _pack_blob_late(consts)
    blob16v, blob16lv = _pack_blob16(consts)

    xyz = inputs["xyz"].astype(np.float32)      # (2, 256, 64, 3)
    feats = inputs["feats"].astype(np.float32)  # (2, 256, 64, 61)
    xf_full = np.concatenate([xyz, feats], axis=-1).reshape(512 * 64, 64)

    in_maps = []
    for c in range(NCORES):
        import ml_dtypes
        sh = xf_full[c * 4096:(c + 1) * 4096, :]
        in_maps.append({
            "blob": blob,
            "blobl": blobl,
            "blob16": blob16v,
            "blob16l": blob16lv,
            "xt16": np.ascontiguousarray(sh.T.astype(ml_dtypes.bfloat16)),
            "xt3": np.ascontiguousarray(sh.T[0:3, :]),
        })

    trace = bool(int(os.environ.get("KERNEL_TRACE", "0")))
    try:
        res = bass_utils.run_bass_kernel_spmd(
            nc, in_maps, core_ids=list(range(NCORES)), trace=trace)
    except ModuleNotFoundError:
        res = bass_utils.run_bass_kernel_spmd(
            nc, in_maps, core_ids=list(range(NCORES)))
    if trace and res.exec_time_ns is not None:
        print(f"HW exec time: {res.exec_time_ns} ns")
        if res.instructions_and_trace is not None:
            print(f"trace: {res.instructions_and_trace[1]}")
        kernel.last_results = res

    out = np.empty((2, 256, 256), dtype=np.float32)
    for c in range(NCORES):
        o = res.results[c]["out"]              # (256, 64)
        b, mlo = divmod(c * G, 256)
        out[b, :, mlo:mlo + G] = o
    return out


if __name__ == "__main__":
    print("building bass graph...")
    nc = _build()
    print("graph built ok")



# revision 76
# speedup vs baseline: 1.0295x; 1.0003x over previous
"""Trainium2 Bass kernel for nn_AttnGNNLayer (EdgeConv-style GNN layer).

Data-parallel over the B*M=512 group axis: 64 groups per core on 8 cores.

Per-group pipeline (K=64 points, knn=16):
  - distance proxy q = x^T x - xx/2 (one ones-row accum matmul; xx from a
    tall (96,128) Square + one f32 matmul against a packed selector)
  - top-16 neighbor indices via DVE max8 / match_replace / max_index
  - one-hot gather matrix S[j, n*16+k]: idx rows DMA'd to (2,1024) u16,
    broadcast-DMA'd to all 128 partitions, then ONE tensor_scalar is_equal
    vs a per-partition iota (4x-mode eligible; runs on the Pool engine,
    which may only touch SBUF - GPSIMD cannot access PSUM on silicon)
  - edge conv: gather matmul accumulates bd@S + bdv@R (R = kron(I,1_16)),
    folding the center term v into psum so the windowed reduce_max (DVE,
    merged (128,1024) 2-bank tiles) directly yields max_k(u[idx]+v); the
    bn+relu then applies at psum egress
  - all 1x1 convs batched over all 64*64=4096 points per core on PE

Emission is software-pipelined: per 8-pair chunk, phase_a (knn) x8, then
b1 (S + e1) / b2 (e2) interleaved with a stagger of 4 so every engine's
in-order stream has other pairs' work between dependent ops; phase_c
(calib/gate/expansion windows) is deferred past the next chunk's phase_a.
Weight constants ship in early/late blobs (f32 + bf16) so the pair loop
does not wait on late-stage conv weights.
"""

import functools
import os
import sys

for _p in ("/opt/trn_rl_repo", "/root/.axon_site/_ro/trn_rl_repo"):
    if os.path.isdir(_p) and _p not in sys.path:
        sys.path.append(_p)

import numpy as np

import concourse.bass as bass
import concourse.mybir as mybir
import concourse.tile as tile
from concourse import bacc, bass_utils

F32 = mybir.dt.float32
BF16 = mybir.dt.bfloat16
U16 = mybir.dt.uint16

B, M, K, KNN = 2, 256, 64, 16
G = 64            # groups per core
NPAIR = G // 2    # 32 pair tiles (2 groups packed in 128 partitions)
NCORES = 8
NEG = -1.0e30
EPS = 1e-5

AF = mybir.ActivationFunctionType
ALU = mybir.AluOpType

# (name, partitions, width) of every constant packed into the blob, in order
_BLOB_LAYOUT = [
    ("iota_col", 128, 1),
    ("neg_iota_col", 128, 1),
    ("ones_row", 1, 1024),
    ("sel96", 96, 32),
    ("s1", 128, 1), ("b1", 128, 1), ("s2", 128, 1), ("b2", 128, 1),
    ("ca1_s", 64, 1), ("ca1_bias", 64, 1),
    ("cb2_blk1", 128, 1), ("cb2_blk2", 64, 1),
    ("e1s", 128, 2), ("e1bias", 128, 2),
    ("e2s", 128, 4), ("e2bias", 128, 4),
]
_BLOB_OFF = {}
_off = 0
for _n, _pp, _w in _BLOB_LAYOUT:
    _BLOB_OFF[_n] = _off
    _off += _w
BLOB_W = _off

# late-stage weights (final 256ch x 64 stage): separate DMA issued after the
# input DMAs so the pair loop can start sooner
_BLOBL_LAYOUT = [
    ("rd0", 128, 256), ("rd1", 128, 256), ("rd2", 128, 256), ("rd3", 128, 256),
    ("rds", 128, 2), ("rdb", 128, 2),
    ("sc1_0", 128, 256), ("sc1_1", 128, 256), ("sc1b", 128, 2),
    ("sc2_0", 128, 256), ("sc2_1", 128, 256), ("sc2b", 128, 2),
    ("n1s", 128, 2), ("n1b", 128, 2), ("n2s", 128, 2), ("n2b", 128, 2),
]
_BLOBL_OFF = {}
_offl = 0
for _n, _pp, _w in _BLOBL_LAYOUT:
    _BLOBL_OFF[_n] = _offl
    _offl += _w
BLOBL_W = _offl

# bf16 constants: pair-loop weights (early) and conv weights (late)
_BLOB16_LAYOUT = [
    ("b16_R2", 128, 1024),
    ("b16_wu1", 64, 64), ("b16_wv1", 64, 64),
    ("b16_wu2", 64, 128), ("b16_wv2", 64, 128),
]
_BLOB16_OFF = {}
_o16 = 0
for _n, _pp, _w in _BLOB16_LAYOUT:
    _BLOB16_OFF[_n] = _o16
    _o16 += _w
BLOB16_W = _o16

_BLOB16L_LAYOUT = [
    ("b16_ca1_a", 64, 64), ("b16_ca1_b", 128, 64), ("b16_ca2", 64, 192),
    ("b16_x1a", 64, 256), ("b16_x1b", 128, 256),
    ("b16_x2a", 128, 512), ("b16_x2b", 128, 512),
]
_BLOB16L_OFF = {}
_o16l = 0
for _n, _pp, _w in _BLOB16L_LAYOUT:
    _BLOB16L_OFF[_n] = _o16l
    _o16l += _w
BLOB16L_W = _o16l


def _np_consts(iw):
    """All constant tensors (iota + host-prepped weights)."""
    f = np.float32
    c = {}
    iota = np.arange(64, dtype=f)
    c["iota_col"] = np.concatenate([iota, iota]).reshape(128, 1)
    c["neg_iota_col"] = -c["iota_col"]
    selg = np.zeros((2, 128), dtype=f)
    selg[0, :64] = 1.0
    selg[1, 64:] = 1.0
    c["selg"] = selg
    c["ones_row"] = np.ones((1, 1024), dtype=f)
    # sel96[c*32+blk, blk] = -0.5: one matmul turns xsq96 (96,128) into
    # -xx/2 for all 4096 points as a (32,128) psum tile
    sel96 = np.zeros((96, 32), dtype=f)
    for _c in range(3):
        for _b in range(32):
            sel96[_c * 32 + _b, _b] = -0.5
    c["sel96"] = sel96
    # replication matrix: R2[p, n*16+k] = (n == p % 64); v-fold accumuland
    _R = np.repeat(np.eye(64, dtype=f), KNN, axis=1)
    c["R2"] = np.vstack([_R, _R])

    e1_w = iw["e1_w"].astype(f)
    W1, W2 = e1_w[:, :64], e1_w[:, 64:]
    c["wu1"] = W1.T.copy()
    c["wv1"] = (W2 - W1).T.copy()
    e2_w = iw["e2_w"].astype(f)
    W21, W22 = e2_w[:, :64], e2_w[:, 64:]
    c["wu2"] = W21.T.copy()
    c["wv2"] = (W22 - W21).T.copy()

    def bn_sb(g, b):
        return (g / np.sqrt(1.0 + EPS)).astype(f), b.astype(f)

    def pair_col(v):
        return np.concatenate([v, v]).reshape(128, 1).astype(f)

    s1, b1 = bn_sb(iw["e1_g"], iw["e1_b"])
    c["s1"], c["b1"] = pair_col(s1), pair_col(b1)
    s2, b2 = bn_sb(iw["e2_g"], iw["e2_b"])
    c["s2"], c["b2"] = s2.reshape(128, 1), b2.reshape(128, 1)

    cal1_w = iw["cal1_w"].astype(f)
    c["ca1_a"] = cal1_w[:, :64].T.copy()
    c["ca1_b"] = cal1_w[:, 64:].T.copy()
    cs, cbv = bn_sb(iw["cal1_g"], iw["cal1_b"])
    c["ca1_s"], c["ca1_bias"] = cs.reshape(64, 1), cbv.reshape(64, 1)

    c["ca2"] = iw["cal2_w"].astype(f).T.copy()
    cb2 = iw["cal2_bias"].astype(f)
    c["cb2_blk1"] = cb2[:128].reshape(128, 1)
    c["cb2_blk2"] = cb2[128:].reshape(64, 1)

    exp1_w = iw["exp1_w"].astype(f)
    c["x1a"] = exp1_w[:, :64].T.copy()
    c["x1b"] = exp1_w[:, 64:].T.copy()
    es, eb = bn_sb(iw["exp1_g"], iw["exp1_b"])
    c["e1s"] = es.reshape(2, 128).T.copy()
    c["e1bias"] = eb.reshape(2, 128).T.copy()

    exp2_w = iw["exp2_w"].astype(f)
    c["x2a"] = exp2_w[:, :128].T.copy()
    c["x2b"] = exp2_w[:, 128:].T.copy()
    es2, eb2 = bn_sb(iw["exp2_g"], iw["exp2_b"])
    c["e2s"] = es2.reshape(4, 128).T.copy()
    c["e2bias"] = eb2.reshape(4, 128).T.copy()

    rdT = iw["red_w"].astype(f).T.reshape(4, 128, 256)
    for i in range(4):
        c[f"rd{i}"] = rdT[i].copy()
    rs, rb = bn_sb(iw["red_g"], iw["red_b"])
    c["rds"] = rs.reshape(2, 128).T.copy()
    c["rdb"] = rb.reshape(2, 128).T.copy()

    sc1T = iw["sc1_w"].astype(f).T.reshape(2, 128, 256)
    c["sc1_0"], c["sc1_1"] = sc1T[0].copy(), sc1T[1].copy()
    c["sc1b"] = iw["sc1_b"].astype(f).reshape(2, 128).T.copy()
    sc2T = iw["sc2_w"].astype(f).T.reshape(2, 128, 256)
    c["sc2_0"], c["sc2_1"] = sc2T[0].copy(), sc2T[1].copy()
    c["sc2b"] = iw["sc2_b"].astype(f).reshape(2, 128).T.copy()

    n1s, n1b = bn_sb(iw["sc_n1_g"], iw["sc_n1_b"])
    c["n1s"] = (2.0 * n1s).reshape(2, 128).T.copy()
    c["n1b"] = n1b.reshape(2, 128).T.copy()
    n2s, n2b = bn_sb(iw["sc_n2_g"], iw["sc_n2_b"])
    c["n2s"] = n2s.reshape(2, 128).T.copy()
    c["n2b"] = n2b.reshape(2, 128).T.copy()
    return c


def _pack_blob(c):
    blob = np.zeros((128, BLOB_W), dtype=np.float32)
    for name, p, w in _BLOB_LAYOUT:
        v = c[name]
        assert v.shape == (p, w), (name, v.shape, (p, w))
        blob[:p, _BLOB_OFF[name]:_BLOB_OFF[name] + w] = v
    return blob


def _pack_blob_late(c):
    blob = np.zeros((128, BLOBL_W), dtype=np.float32)
    for name, p, w in _BLOBL_LAYOUT:
        v = c[name]
        assert v.shape == (p, w), (name, v.shape, (p, w))
        blob[:p, _BLOBL_OFF[name]:_BLOBL_OFF[name] + w] = v
    return blob


def _pack_blob16(c):
    import ml_dtypes
    src16 = {"b16_R2": c["R2"],
             "b16_wu1": c["wu1"], "b16_wv1": c["wv1"],
             "b16_wu2": c["wu2"], "b16_wv2": c["wv2"],
             "b16_ca1_a": c["ca1_a"], "b16_ca1_b": c["ca1_b"],
             "b16_ca2": c["ca2"], "b16_x1a": c["x1a"], "b16_x1b": c["x1b"],
             "b16_x2a": c["x2a"], "b16_x2b": c["x2b"]}
    blob = np.zeros((128, BLOB16_W), dtype=ml_dtypes.bfloat16)
    for name, p, w in _BLOB16_LAYOUT:
        v = src16[name]
        assert v.shape == (p, w), (name, v.shape, (p, w))
        blob[:p, _BLOB16_OFF[name]:_BLOB16_OFF[name] + w] = v.astype(
            ml_dtypes.bfloat16)
    blobl = np.zeros((128, BLOB16L_W), dtype=ml_dtypes.bfloat16)
    for name, p, w in _BLOB16L_LAYOUT:
        v = src16[name]
        assert v.shape == (p, w), (name, v.shape, (p, w))
        blobl[:p, _BLOB16L_OFF[name]:_BLOB16L_OFF[name] + w] = v.astype(
            ml_dtypes.bfloat16)
    return blob, blobl


def _emit(tc, I, out_ap, ctx):
    nc = tc.nc

    cp = ctx.enter_context(tc.tile_pool(name="const", bufs=1))
    wide = ctx.enter_context(tc.tile_pool(name="wide", bufs=1))
    MERGE = int(os.environ.get("K_MERGE", "1"))
    nbig = int(os.environ.get("K_NBIG", "2")) if MERGE else 6
    pp_big = ctx.enter_context(
        tc.tile_pool(name="ps_big", bufs=nbig, space="PSUM"))
    pp_big2 = ctx.enter_context(
        tc.tile_pool(name="ps_big2", bufs=int(os.environ.get("K_NBIG2", "2")),
                     space="PSUM"))
    pp_med = ctx.enter_context(
        tc.tile_pool(name="ps_med", bufs=int(os.environ.get("K_NMED", "2")),
                     space="PSUM"))
    wk = ctx.enter_context(
        tc.tile_pool(name="work", bufs=int(os.environ.get("K_WK", "10"))))
    wk2 = ctx.enter_context(
        tc.tile_pool(name="work2", bufs=int(os.environ.get("K_WK2", "8"))))
    ring = ctx.enter_context(tc.tile_pool(name="ring", bufs=1))

    # ---- DMA order = HWDGE processing order: the knn-critical pieces
    # (x96 for xx, xt3 rows, f32 blob with sel96/iota) go first so the
    # distance/top-k chain starts ~4us earlier; bf16 weights and the input
    # x (first needed by the e1 u/v matmuls) follow; late-stage weights
    # last.
    x96 = wide.tile([96, 128], F32, tag="w96")
    nc.sync.dma_start(out=x96,
                      in_=bass.AP(tensor=I["xt3"].tensor, offset=0,
                                  ap=[[4096, 3], [128, 32], [1, 128]]))
    blob = cp.tile([128, BLOB_W], F32, tag="blob")
    nc.sync.dma_start(out=blob, in_=I["blob"])
    sb = {}
    for name, p, w in _BLOB_LAYOUT:
        sb[name] = blob[0:p, _BLOB_OFF[name]:_BLOB_OFF[name] + w]
    blob16 = cp.tile([128, BLOB16_W], BF16, tag="blob16")
    for name, p, w in _BLOB16_LAYOUT:
        sb[name] = blob16[0:p, _BLOB16_OFF[name]:_BLOB16_OFF[name] + w]
    blob16l = cp.tile([128, BLOB16L_W], BF16, tag="blob16l")
    for name, p, w in _BLOB16L_LAYOUT:
        sb[name] = blob16l[0:p, _BLOB16L_OFF[name]:_BLOB16L_OFF[name] + w]
    xsq96 = wide.tile([96, 128], F32, tag="w96b")
    nc.scalar.activation(out=xsq96, in_=x96, func=AF.Square)
    nxp32 = pp_med.tile([32, 128], F32, tag="med")
    nc.tensor.matmul(nxp32, sb["sel96"], xsq96)
    nxs = wide.tile([32, 128], F32, tag="w96c")
    nc.scalar.activation(out=nxs, in_=nxp32, func=AF.Copy)

    B4 = wide.tile([4, 4096], F32, tag="wC")
    A4 = wide.tile([4, 4096], F32, tag="wB")
    nc.sync.dma_start(out=B4[0:3, :], in_=I["xt3"])
    nc.sync.dma_start(out=B4[3:4, :], in_=nxs)
    nc.sync.dma_start(out=A4[0:3, :], in_=I["xt3"])
    nc.sync.dma_start(out=A4[3:4, :],
                      in_=bass.AP(tensor=I["blob"].tensor,
                                  offset=_BLOB_OFF["ones_row"],
                                  ap=[[0, 1], [0, 4], [1, 1024]]))

    # bf16 weights, input x, then late-stage weights
    nc.sync.dma_start(out=blob16, in_=I["blob16"])
    x = wide.tile([64, 4096], BF16, tag="wD")
    for t in range(8):
        nc.sync.dma_start(out=x[:, t * 512:(t + 1) * 512],
                          in_=I["xt16"][:, t * 512:(t + 1) * 512])
    blobl = cp.tile([128, BLOBL_W], F32, tag="blobl")
    nc.sync.dma_start(out=blobl, in_=I["blobl"])
    nc.sync.dma_start(out=blob16l, in_=I["blob16l"])
    for name, p, w in _BLOBL_LAYOUT:
        sb[name] = blobl[0:p, _BLOBL_OFF[name]:_BLOBL_OFF[name] + w]

    # all pairwise-distance tiles upfront (prologue is DMA-bound, engines
    # idle): qt_all[:, pi*64:...] = q for pair pi; frees the psum med ring
    # and the ACT stream from per-pair distance work
    QTALL = int(os.environ.get("K_QTALL", "1"))
    qt_all = None
    if QTALL:
        qt_all = wide.tile([128, NPAIR * 64], F32, tag="wQT")
        for pi in range(NPAIR):
            _cs1 = slice((2 * pi) * 64, (2 * pi + 1) * 64)
            _cs2 = slice((2 * pi + 1) * 64, (2 * pi + 2) * 64)
            _pdp = pp_med.tile([128, 64], F32, tag="med")
            nc.tensor.matmul(_pdp[0:64, :], A4[:, _cs1], B4[:, _cs1])
            nc.tensor.matmul(_pdp[64:128, :], A4[:, _cs2], B4[:, _cs2])
            nc.scalar.activation(out=qt_all[:, pi * 64:(pi + 1) * 64],
                                 in_=_pdp, func=AF.Copy)

    # gated activations accumulated across all groups (for batched convs)
    x1all = wide.tile([64, 4096], BF16, tag="wE")
    x2all = wide.tile([128, 4096], BF16, tag="wF")
    # final per-group features (512ch as 4 blocks x 64 groups)
    xfin = cp.tile([128, 4, G], F32, tag="xfin")

    BDM = int(os.environ.get("K_BDM", "1"))
    bd_ring = []
    bdv_ring = []
    bdd_ring = []
    for ri in range(int(os.environ.get("K_BD", "6"))):
        if BDM:
            bddt = cp.tile([128, 256], BF16, tag=f"bddring{ri}")
            nc.gpsimd.memset(bddt, 0.0)
            bdd_ring.append(bddt)
            bd_ring.append(bddt[:, 0:128])
            bdv_ring.append(bddt[:, 128:256])
        else:
            bdt = cp.tile([128, 128], BF16, tag=f"bdring{ri}")
            nc.gpsimd.memset(bdt, 0.0)
            bd_ring.append(bdt)
            bdvt = cp.tile([128, 128], BF16, tag=f"bdvring{ri}")
            nc.gpsimd.memset(bdvt, 0.0)
            bdv_ring.append(bdvt)

    ADDP = int(os.environ.get("K_ADDP", "0"))
    E2_MODE = os.environ.get("K_E2M", "a")  # a | b | c | bc
    PCTAIL = int(os.environ.get("K_PCTAIL", "0"))
    SPOOL = int(os.environ.get("K_SPOOL", "1"))
    RELUP = int(os.environ.get("K_RELUP", "0"))
    RELUX2 = int(os.environ.get("K_RELUX2", "0"))

    def _relu_sb(out, in_, s_col, b_col, pool=None):
        # relu(s*in + b) from sbuf: 2 pool ops, or 1 act op
        if RELUP if pool is None else pool:
            tmp = wk.tile(list(in_.shape), F32, tag="rtmp")
            nc.gpsimd.tensor_scalar(out=tmp, in0=in_, scalar1=s_col,
                                    scalar2=b_col, op0=ALU.mult, op1=ALU.add)
            nc.gpsimd.tensor_scalar(out=out, in0=tmp, scalar1=0.0,
                                    scalar2=None, op0=ALU.max)
        else:
            nc.scalar.activation(out=out, in_=in_, func=AF.Relu,
                                 bias=b_col, scale=s_col)
    F1 = int(os.environ.get("K_F1", "1"))
    F2 = int(os.environ.get("K_F2", "1"))
    QTP = int(os.environ.get("K_QTP", "0"))

    def _tadd(out, a, b):
        if ADDP:
            nc.gpsimd.tensor_tensor(out, a, b, op=ALU.add)
        else:
            nc.vector.tensor_add(out, a, b)

    # windowed-max placement: offload part of the (128, 32n, 16k) max blocks
    # from DVE (TensorReduce) to the mostly-idle Pool engine (pairwise-max
    # tree, in place on the psum tile)
    E1_POOL = int(os.environ.get("K_E1P", "0"))
    E2_POOL_ROUNDS = tuple(
        int(v) for v in os.environ.get("K_E2P", "0,0").split(","))
    PC_POOL = int(os.environ.get("K_PCP", "0"))

    def _wmax(gp, out, pool_rounds, k=KNN):
        """max over k of gp (128, 512) viewed as (p, n, k) -> out (128, 512/k).

        pool_rounds pairwise-max rounds run on the Pool engine (in place on
        the psum tile); the remaining window is reduced on DVE."""
        g3 = gp.rearrange("p (n k) -> p n k", k=k)
        w = k
        for _ in range(pool_rounds):
            h = w // 2
            nc.gpsimd.tensor_tensor(g3[:, :, 0:h], g3[:, :, 0:h],
                                    g3[:, :, h:w], op=ALU.max)
            w = h
        nc.vector.reduce_max(out=out, in_=g3[:, :, 0:w],
                             axis=mybir.AxisListType.X)

    # ---------------- per-pair loops (chunked for DMA latency hiding) ----
    CHUNK = int(os.environ.get("K_CHUNK", "8"))
    NIXB = int(os.environ.get("K_NIXB", "8"))
    ixb_ring = [None] * NIXB
    ixr_all = ring.tile([2 * CHUNK, 1024], U16, tag="ixr_all")

    def phase_a(pi):
        g1, g2 = 2 * pi, 2 * pi + 1
        cs1 = slice(g1 * 64, (g1 + 1) * 64)
        cs2 = slice(g2 * 64, (g2 + 1) * 64)

        # q = x_m.x_n - xx_n/2 for both groups -> (128, 64)
        # (row-constant -xx_m/2 term dropped: doesn't change row top-k)
        if QTALL:
            qt = qt_all[:, pi * 64:(pi + 1) * 64]
        else:
            pdp = pp_med.tile([128, 64], F32, tag="med")
            for h, cs in ((0, cs1), (1, cs2)):
                nc.tensor.matmul(pdp[h * 64:(h + 1) * 64, :],
                                 A4[:, cs], B4[:, cs])
            qt = wk.tile([128, 64], F32, tag="qt")
            if QTP:
                nc.gpsimd.tensor_copy(qt, pdp)
            else:
                nc.scalar.activation(out=qt, in_=pdp, func=AF.Copy)

        # top-16 indices per point row
        mx = wk.tile([128, 16], F32, tag="mx")
        ix = wk.tile([128, 16], U16, tag="ix")
        qt2 = wk.tile([128, 64], F32, tag="qt2")
        nc.vector.max(out=mx[:, 0:8], in_=qt)
        nc.vector.max_index(out=ix[:, 0:8], in_max=mx[:, 0:8], in_values=qt)
        nc.vector.match_replace(out=qt2, in_to_replace=mx[:, 0:8],
                                in_values=qt, imm_value=NEG)
        nc.vector.max(out=mx[:, 8:16], in_=qt2)
        nc.vector.max_index(out=ix[:, 8:16], in_max=mx[:, 8:16], in_values=qt2)

        # idx row form (2, 1024) u16, then broadcast to all 128 partitions so
        # the one-hot compare can run at 4x (2-byte sbuf in/out)
        s2 = 2 * (pi % CHUNK)
        ixrows = ixr_all[s2:s2 + 2, :]
        nc.sync.dma_start(out=ixrows, in_=ix)
        ixb = ring.tile([128, 1024], U16, tag=f"ixb{pi % NIXB}")
        ixb_ring[pi % NIXB] = ixb
        nc.sync.dma_start(
            out=ixb, in_=ixrows.unsqueeze(1).broadcast_to((2, 64, 1024)))

    S01_ring = [None] * CHUNK

    def phase_b1(pi):
        """S-matrix + e1 edge conv for pair pi."""
        g1, g2 = 2 * pi, 2 * pi + 1
        cs1 = slice(g1 * 64, (g1 + 1) * 64)
        cs2 = slice(g2 * 64, (g2 + 1) * 64)
        ixb = ixb_ring[pi % NIXB]
        S01 = wk2.tile([128, 1024], BF16, tag="S01")
        S01_ring[pi % CHUNK] = S01
        # S01 = (ix_bcast == iota_p): 4x dve op (all operands 2-byte sbuf)
        seng = nc.gpsimd if (SPOOL == 1 or (SPOOL == 2 and pi % 2)) \
            else nc.vector
        seng.tensor_scalar(out=S01, in0=ixb,
                           scalar1=sb["iota_col"], scalar2=None,
                           op0=ALU.is_equal)

        bd = bd_ring[pi % len(bd_ring)]
        if F1:
            # u and v-fold matmuls into one psum tile; merged copies
            uvv = pp_med.tile([128, 256], F32, tag="med")
            nc.tensor.matmul(uvv[0:64, 0:64], x[:, cs1], sb["b16_wu1"])
            nc.tensor.matmul(uvv[64:128, 64:128], x[:, cs2], sb["b16_wu1"])
            nc.tensor.matmul(uvv[0:64, 128:192], x[:, cs1], sb["b16_wv1"])
            nc.tensor.matmul(uvv[64:128, 192:256], x[:, cs2], sb["b16_wv1"])
            bdv = bdv_ring[pi % len(bdv_ring)]
            if BDM:
                # one copy per partition half: {u block, v block} as a
                # strided access pattern on both sides.
                # col = a*128 + b*64 + c: a selects u/v, b selects group
                bdd = bdd_ring[pi % len(bdd_ring)]
                sv = uvv.rearrange("p (a b c) -> p a b c", a=2, b=2)
                dv = bdd.rearrange("p (a b c) -> p a b c", a=2, b=2)
                nc.scalar.activation(out=dv[0:64, :, 0:1, :],
                                     in_=sv[0:64, :, 0:1, :], func=AF.Copy)
                nc.scalar.activation(out=dv[64:128, :, 1:2, :],
                                     in_=sv[64:128, :, 1:2, :], func=AF.Copy)
            else:
                nc.scalar.activation(
                    out=bd[0:64, 0:64], in_=uvv[0:64, 0:64], func=AF.Copy)
                nc.scalar.activation(
                    out=bd[64:128, 64:128], in_=uvv[64:128, 64:128],
                    func=AF.Copy)
                nc.scalar.activation(
                    out=bdv[0:64, 0:64], in_=uvv[0:64, 128:192], func=AF.Copy)
                nc.scalar.activation(
                    out=bdv[64:128, 64:128], in_=uvv[64:128, 192:256],
                    func=AF.Copy)
        else:
            uv1 = pp_med.tile([128, 128], F32, tag="med")
            nc.tensor.matmul(uv1[0:64, 0:64], x[:, cs1], sb["b16_wu1"])
            nc.tensor.matmul(uv1[64:128, 64:128], x[:, cs2], sb["b16_wu1"])
            nc.scalar.activation(out=bd[0:64, 0:64], in_=uv1[0:64, 0:64],
                                 func=AF.Copy)
            nc.scalar.activation(out=bd[64:128, 64:128],
                                 in_=uv1[64:128, 64:128], func=AF.Copy)
        m1 = wk.tile([128, 64], F32, tag="m1")
        if MERGE:
            g1p = pp_big2.tile([128, 1024], F32, tag="big2")
            for half in range(2):
                csl = slice(half * 512, (half + 1) * 512)
                gh = g1p[:, csl]
                if F1:
                    nc.tensor.matmul(gh, bd, S01[:, csl], start=True,
                                     stop=False)
                    nc.tensor.matmul(gh, bdv, sb["b16_R2"][:, csl],
                                     start=False, stop=True)
                else:
                    nc.tensor.matmul(gh, bd, S01[:, csl])
            nc.vector.reduce_max(
                out=m1, in_=g1p.rearrange("p (n k) -> p n k", k=KNN),
                axis=mybir.AxisListType.X)
        else:
            for half in range(2):
                csl = slice(half * 512, (half + 1) * 512)
                g1p = pp_big.tile([128, 512], F32, tag="big")
                if F1:
                    nc.tensor.matmul(g1p, bd, S01[:, csl], start=True,
                                     stop=False)
                    nc.tensor.matmul(g1p, bdv, sb["b16_R2"][:, csl],
                                     start=False, stop=True)
                else:
                    nc.tensor.matmul(g1p, bd, S01[:, csl])
                _wmax(g1p, m1[:, half * 32:(half + 1) * 32], E1_POOL)
        if F1:
            _relu_sb(x1all[:, cs1], m1[0:64, :], sb["s1"][0:64],
                     sb["b1"][0:64])
            _relu_sb(x1all[:, cs2], m1[64:128, :], sb["s1"][64:128],
                     sb["b1"][64:128])
        else:
            v1 = pp_med.tile([128, 64], F32, tag="med")
            nc.tensor.matmul(v1[0:64, :], sb["b16_wv1"], x[:, cs1])
            nc.tensor.matmul(v1[64:128, :], sb["b16_wv1"], x[:, cs2])
            t1a = wk.tile([64, 64], F32, tag="t1a")
            _tadd(t1a, m1[0:64, :], v1[0:64, :])
            t1b = wk.tile([128, 64], F32, tag="t1b")
            _tadd(t1b[64:128, :], m1[64:128, :], v1[64:128, :])
            nc.scalar.activation(out=x1all[:, cs1], in_=t1a, func=AF.Relu,
                                 bias=sb["b1"][0:64], scale=sb["s1"][0:64])
            nc.scalar.activation(out=x1all[:, cs2], in_=t1b[64:128, :],
                                 func=AF.Relu,
                                 bias=sb["b1"][64:128], scale=sb["s1"][64:128])

    def phase_b2(pi):
        """e2 edge conv for pair pi (consumes x1all + S01)."""
        g1, g2 = 2 * pi, 2 * pi + 1
        cs1 = slice(g1 * 64, (g1 + 1) * 64)
        cs2 = slice(g2 * 64, (g2 + 1) * 64)
        S01 = S01_ring[pi % CHUNK]
        for h, cs in ((0, cs1), (1, cs2)):
            xg = x1all[:, cs]
            psl = slice(h * 64, (h + 1) * 64)
            if E2_MODE == "a" and F2:
                uvp = pp_med.tile([64, 256], F32, tag="med")
                nc.tensor.matmul(uvp[:, 0:128], xg, sb["b16_wu2"])
                nc.tensor.matmul(uvp[:, 128:256], xg, sb["b16_wv2"])
                uvs = wk.tile([128, 256], BF16, tag="uvs")
                nc.scalar.activation(out=uvs[psl, :], in_=uvp, func=AF.Copy)
                uT2 = uvs[:, 0:128]
                vv2 = uvs[:, 128:256]
            else:
                uT2 = wk.tile([128, 128], BF16, tag="uT2")
                uT2p = pp_med.tile([64, 128], F32, tag="med")
                nc.tensor.matmul(uT2p, xg, sb["b16_wu2"])
                nc.scalar.activation(out=uT2[psl, :], in_=uT2p, func=AF.Copy)
            if E2_MODE == "a":
                m2 = wk.tile([128, 64], F32, tag="m2")
                if MERGE:
                    g2p = pp_big2.tile([128, 1024], F32, tag="big2")
                    for half in range(2):
                        csl = slice(half * 512, (half + 1) * 512)
                        gh = g2p[:, csl]
                        if F2:
                            nc.tensor.matmul(gh, uT2[psl, :], S01[psl, csl],
                                             start=True, stop=False)
                            nc.tensor.matmul(gh, vv2[psl, :],
                                             sb["b16_R2"][psl, csl],
                                             start=False, stop=True)
                        else:
                            nc.tensor.matmul(gh, uT2[psl, :], S01[psl, csl])
                    nc.vector.reduce_max(
                        out=m2, in_=g2p.rearrange("p (n k) -> p n k", k=KNN),
                        axis=mybir.AxisListType.X)
                else:
                    for half in range(2):
                        csl = slice(half * 512, (half + 1) * 512)
                        g2p = pp_big.tile([128, 512], F32, tag="big")
                        if F2:
                            nc.tensor.matmul(g2p, uT2[psl, :], S01[psl, csl],
                                             start=True, stop=False)
                            nc.tensor.matmul(g2p, vv2[psl, :],
                                             sb["b16_R2"][psl, csl],
                                             start=False, stop=True)
                        else:
                            nc.tensor.matmul(g2p, uT2[psl, :], S01[psl, csl])
                        _wmax(g2p, m2[:, half * 32:(half + 1) * 32],
                              E2_POOL_ROUNDS[half])
                if F2:
                    _relu_sb(x2all[:, cs], m2, sb["s2"], sb["b2"],
                             pool=RELUX2)
                else:
                    v2 = pp_med.tile([128, 64], F32, tag="med")
                    nc.tensor.matmul(v2, sb["b16_wv2"], xg)
                    t2 = wk.tile([128, 64], F32, tag="t2")
                    _tadd(t2, m2, v2)
                    nc.scalar.activation(out=x2all[:, cs], in_=t2,
                                         func=AF.Relu,
                                         bias=sb["b2"], scale=sb["s2"])
                continue
            # v-folded path: g2p = uT2.S + vT2.R, then relu(bn) at psum
            # egress (valid pre-max: bn scale > 0), max-tree on sbuf bf16
            vv2p = pp_med.tile([64, 128], F32, tag="med")
            nc.tensor.matmul(vv2p, xg, sb["b16_wv2"])
            vv2 = wk.tile([64, 128], BF16, tag="vv2")
            nc.scalar.activation(out=vv2, in_=vv2p, func=AF.Copy)
            for half in range(2):
                csl = slice(half * 512, (half + 1) * 512)
                g2p = pp_big.tile([128, 512], F32, tag="big")
                nc.tensor.matmul(g2p, uT2[psl, :], S01[psl, csl],
                                 start=True, stop=False)
                nc.tensor.matmul(g2p, vv2, sb["b16_R2"][0:64, csl],
                                 start=False, stop=True)
                x2pre = wk.tile([128, 512], BF16, tag="x2pre")
                nc.scalar.activation(out=x2pre, in_=g2p, func=AF.Relu,
                                     bias=sb["b2"], scale=sb["s2"])
                p3 = x2pre.rearrange("p (n k) -> p n k", k=KNN)
                eng = nc.gpsimd if E2_MODE == "c" or (
                    E2_MODE == "bc" and half == 1) else nc.vector
                eng.tensor_tensor(p3[:, :, 0:8], p3[:, :, 0:8],
                                  p3[:, :, 8:16], op=ALU.max)
                eng.tensor_tensor(p3[:, :, 0:4], p3[:, :, 0:4],
                                  p3[:, :, 4:8], op=ALU.max)
                eng.tensor_tensor(p3[:, :, 0:2], p3[:, :, 0:2],
                                  p3[:, :, 2:4], op=ALU.max)
                osub = slice(cs.start + half * 32, cs.start + half * 32 + 32)
                eng.tensor_tensor(x2all[:, osub], p3[:, :, 0:1].squeeze(-1),
                                  p3[:, :, 1:2].squeeze(-1), op=ALU.max)


    # ---------------- batched calib/gate/expansion (per 512-col window) --
    c1all = wide.tile([64, 4096], BF16, tag="wG")
    sigA = wide.tile([64, 4096], BF16, tag="wA")
    sigX2 = wide.tile([128, 4096], BF16, tag="wH")
    pcr = ctx.enter_context(
        tc.tile_pool(name="pcring", bufs=int(os.environ.get("K_NPC", "4"))))
    ee0 = wide.tile([128, 4096], BF16, tag="wK")
    ee1 = wide.tile([128, 4096], BF16, tag="wL")
    ee = [ee0, ee1]

    def phase_c(j):
        if PCSPLIT:
            phase_c_part(j, 0)
            phase_c_part(j, 1)
        else:
            phase_c_part(j, None)

    def phase_c_part(j, part):
        if part is None:
            csl = slice(j * 512, (j + 1) * 512)
            fsl = slice(j * 8, (j + 1) * 8)
        else:
            csl = slice(j * 512 + part * 256, j * 512 + part * 256 + 256)
            fsl = slice(j * 8 + part * 4, j * 8 + part * 4 + 4)
        W = csl.stop - csl.start
        c1p = pp_big.tile([64, W], F32, tag="big")
        nc.tensor.matmul(c1p, sb["b16_ca1_a"], x1all[:, csl], start=True,
                         stop=False)
        nc.tensor.matmul(c1p, sb["b16_ca1_b"], x2all[:, csl], start=False,
                         stop=True)
        nc.scalar.activation(out=c1all[:, csl], in_=c1p, func=AF.Relu,
                             bias=sb["ca1_bias"], scale=sb["ca1_s"])
        sp1 = pp_big.tile([128, W], F32, tag="big")
        nc.tensor.matmul(sp1, sb["b16_ca2"][:, 0:128], c1all[:, csl])
        nc.scalar.activation(out=sigA[:, csl], in_=sp1[0:64, :],
                             func=AF.Sigmoid, bias=sb["cb2_blk1"][0:64])
        nc.scalar.activation(out=sigX2[0:64, csl], in_=sp1[64:128, :],
                             func=AF.Sigmoid, bias=sb["cb2_blk1"][64:128])
        sp2 = pp_big.tile([64, W], F32, tag="big")
        nc.tensor.matmul(sp2, sb["b16_ca2"][:, 128:192], c1all[:, csl])
        nc.scalar.activation(out=sigX2[64:128, csl], in_=sp2, func=AF.Sigmoid,
                             bias=sb["cb2_blk2"])
        p1t = pcr.tile([64, W], BF16, tag="p1r")
        p2t = pcr.tile([128, W], BF16, tag="p2r")
        nc.gpsimd.tensor_mul(p1t, x1all[:, csl], sigA[:, csl])
        nc.gpsimd.tensor_mul(p2t, x2all[:, csl], sigX2[:, csl])
        for b in range(2):
            ep = pp_big.tile([128, W], F32, tag="big")
            osl = slice(b * 128, (b + 1) * 128)
            nc.tensor.matmul(ep, sb["b16_x1a"][:, osl], p1t,
                             start=True, stop=False)
            nc.tensor.matmul(ep, sb["b16_x1b"][:, osl], p2t,
                             start=False, stop=True)
            nc.scalar.activation(out=ee[b][:, csl], in_=ep, func=AF.Relu,
                                 bias=sb["e1bias"][:, b:b + 1],
                                 scale=sb["e1s"][:, b:b + 1])
        tailwin = PCTAIL and j >= 6
        for b in range(4):
            if tailwin and b >= 2:
                xp = pp_big2.tile([128, W], F32, tag="big2")
            else:
                xp = pp_big.tile([128, W], F32, tag="big")
            osl = slice(b * 128, (b + 1) * 128)
            nc.tensor.matmul(xp, sb["b16_x2a"][:, osl], ee[0][:, csl],
                             start=True, stop=False)
            nc.tensor.matmul(xp, sb["b16_x2b"][:, osl], ee[1][:, csl],
                             start=False, stop=True)
            xm = wk2.tile([128, W // 64], F32, tag="xm")
            _wmax(xp, xm, PC_POOL, k=64)
            nc.scalar.activation(out=xfin[:, b, fsl], in_=xm,
                                 func=AF.Relu,
                                 bias=sb["e2bias"][:, b:b + 1],
                                 scale=sb["e2s"][:, b:b + 1])

    STAG = int(os.environ.get("K_STAG", "4"))
    PCSPLIT = int(os.environ.get("K_PCSPLIT", "0"))
    FLAT = int(os.environ.get("K_FLAT", "1"))
    if FLAT:
        # one continuous pipeline over all 32 pairs: phase_a leads b1 by LA,
        # b2 trails b1 by STAG, each phase_c window fires as its 4 pairs
        # complete. No chunk boundaries, so the pipeline never drains.
        # Ring safety: ixb/ixr slots (8) are rewritten 8-LA b1-steps after
        # their reader; S01 slots (WK2) rewritten WK2-STAG steps after.
        LA = int(os.environ.get("K_LA", "4"))
        for pi in range(LA):
            phase_a(pi)

        PCD = int(os.environ.get("K_PCD", "4"))

        def _after_b2(done):
            # window w is emitted at done == 4*w + 3 + PCD
            if done >= PCD + 3 and (done - PCD - 3) % 4 == 0:
                phase_c((done - PCD - 3) // 4)
            if done == NPAIR - 1:
                for w in range((done - PCD - 3) // 4 + 1, NPAIR // 4):
                    phase_c(w)
        for pi in range(NPAIR):
            phase_b1(pi)
            if pi + LA < NPAIR:
                phase_a(pi + LA)
            if pi >= STAG:
                phase_b2(pi - STAG)
                _after_b2(pi - STAG)
        for pi in range(NPAIR - STAG, NPAIR):
            phase_b2(pi)
            _after_b2(pi)
    else:
        nwin = CHUNK // 4
        pending_c = []
        for chunk in range(NPAIR // CHUNK):
            base = chunk * CHUNK
            for pi in range(base, base + CHUNK):
                phase_a(pi)
            for w in pending_c:
                phase_c(w)
            pending_c = []
            for i in range(CHUNK):
                phase_b1(base + i)
                if i >= STAG:
                    phase_b2(base + i - STAG)
            for i in range(CHUNK - STAG, CHUNK):
                phase_b2(base + i)
            pending_c = list(range(nwin * chunk, nwin * chunk + nwin))
        for w in pending_c:
            phase_c(w)

    # ---------------- final stage (256ch x 64 group-cols) ---------------
    tt = wk.tile([128, 2, G], F32, tag="tt")
    FSPLIT = int(os.environ.get("K_FSPLIT", "0"))
    for b in range(2):
        osl = slice(b * 128, (b + 1) * 128)
        rp = pp_med.tile([128, G], F32, tag="med")
        rngs = (slice(0, 48), slice(48, 64)) if FSPLIT else (slice(0, G),)
        for rng in rngs:
            for cb in range(4):
                nc.tensor.matmul(rp[:, rng], sb[f"rd{cb}"][:, osl],
                                 xfin[:, cb, rng],
                                 start=(cb == 0), stop=(cb == 3))
        rr = wk.tile([128, G], F32, tag="rr")
        nc.scalar.activation(out=rr, in_=rp, func=AF.Relu,
                             bias=sb["rdb"][:, b:b + 1],
                             scale=sb["rds"][:, b:b + 1])
        nc.vector.tensor_scalar(out=tt[:, b, :], in0=rr,
                                scalar1=sb["n1s"][:, b:b + 1],
                                scalar2=sb["n1b"][:, b:b + 1],
                                op0=ALU.mult, op1=ALU.add)
    hh = wk.tile([128, 2, G], F32, tag="hh")
    for b in range(2):
        osl = slice(b * 128, (b + 1) * 128)
        hp = pp_med.tile([128, G], F32, tag="med")
        for cb in range(2):
            nc.tensor.matmul(hp, sb[f"sc1_{cb}"][:, osl], tt[:, cb, :],
                             start=(cb == 0), stop=(cb == 1))
        nc.scalar.activation(out=hh[:, b, :], in_=hp, func=AF.Relu,
                             bias=sb["sc1b"][:, b:b + 1])
    for b in range(2):
        osl = slice(b * 128, (b + 1) * 128)
        h2p = pp_med.tile([128, G], F32, tag="med")
        for cb in range(2):
            nc.tensor.matmul(h2p, sb[f"sc2_{cb}"][:, osl], hh[:, cb, :],
                             start=(cb == 0), stop=(cb == 1))
        s2sum = wk.tile([128, G], F32, tag="s2sum")
        nc.vector.tensor_scalar(out=s2sum, in0=h2p,
                                scalar1=sb["sc2b"][:, b:b + 1], scalar2=None,
                                op0=ALU.add)
        s2t = wk.tile([128, G], F32, tag="s2t")
        nc.vector.tensor_add(s2t, s2sum, tt[:, b, :])
        osb = wk.tile([128, G], F32, tag="osb")
        nc.vector.tensor_scalar(out=osb, in0=s2t,
                                scalar1=sb["n2s"][:, b:b + 1],
                                scalar2=sb["n2b"][:, b:b + 1],
                                op0=ALU.mult, op1=ALU.add)
        nc.sync.dma_start(out=out_ap[b * 128:(b + 1) * 128, :], in_=osb)


@functools.lru_cache(maxsize=1)
def _build():
    nc = bacc.Bacc("TRN2", target_bir_lowering=False, debug=False,
                   num_devices=NCORES)
    I = {}
    I["xt16"] = nc.dram_tensor("xt16", (64, 4096), BF16,
                               kind="ExternalInput").ap()
    I["xt3"] = nc.dram_tensor("xt3", (3, 4096), F32,
                              kind="ExternalInput").ap()
    I["blob"] = nc.dram_tensor("blob", (128, BLOB_W), F32,
                               kind="ExternalInput").ap()
    I["blobl"] = nc.dram_tensor("blobl", (128, BLOBL_W), F32,
                                kind="ExternalInput").ap()
    I["blob16"] = nc.dram_tensor("blob16", (128, BLOB16_W), BF16,
                                 kind="ExternalInput").ap()
    I["blob16l"] = nc.dram_tensor("blob16l", (128, BLOB16L_W), BF16,
                                  kind="ExternalInput").ap()
    out_ap = nc.dram_tensor("out", (256, G), F32, kind="ExternalOutput").ap()
    from contextlib import ExitStack
    with tile.TileContext(nc) as tc, ExitStack() as ctx:
        _emit(tc, I, out_ap, ctx)
    nc.compile()
    return nc


def kernel(**inputs):
    nc = _build()
    consts = _np_consts(inputs)
    blob = _pack_blob(consts)
    blobl = _pack_blob_late(consts)
    blob16v, blob16lv = _pack_blob16(consts)

    xyz = inputs["xyz"].astype(np.float32)      # (2, 256, 64, 3)
    feats = inputs["feats"].astype(np.float32)  # (2, 256, 64, 61)
    xf_full = np.concatenate([xyz, feats], axis=-1).reshape(512 * 64, 64)

    in_maps = []
    for c in range(NCORES):
        import ml_dtypes
        sh = xf_full[c * 4096:(c + 1) * 4096, :]
        in_maps.append({
            "blob": blob,
            "blobl": blobl,
            "blob16": blob16v,
            "blob16l": blob16lv,
            "xt16": np.ascontiguousarray(sh.T.astype(ml_dtypes.bfloat16)),
            "xt3": np.ascontiguousarray(sh.T[0:3, :]),
        })

    trace = bool(int(os.environ.get("KERNEL_TRACE", "0")))
    try:
        res = bass_utils.run_bass_kernel_spmd(
            nc, in_maps, core_ids=list(range(NCORES)), trace=trace)
    except ModuleNotFoundError:
        res = bass_utils.run_bass_kernel_spmd(
            nc, in_maps, core_ids=list(range(NCORES)))
    if trace and res.exec_time_ns is not None:
        print(f"HW exec time: {res.exec_time_ns} ns")
        if res.instructions_and_trace is not None:
            print(f"trace: {res.instructions_and_trace[1]}")
        kernel.last_results = res

    out = np.empty((2, 256, 256), dtype=np.float32)
    for c in range(NCORES):
        o = res.results[c]["out"]              # (256, 64)
        b, mlo = divmod(c * G, 256)
        out[b, :, mlo:mlo + G] = o
    return out


if __name__ == "__main__":
    print("building bass graph...")
    nc = _build()
    print("graph built ok")

